# revision 1
# baseline (speedup 1.0000x reference)
"""Cross-modal attention Trainium2 kernel.

Reference computation (all 1x1 convs + folded eval-mode BN):
  q = BN(Wq @ rgb), k = BN(Wk @ edge), v = BN(Wv @ edge)
  attn = softmax(q^T k) per head; xx = relu(attn @ v); out = BN(Wp @ xx)

Shapes: B=2, C=256, H=W=64 (N=4096), heads=8, key_dim=16, d=32.

Sharding: 8 cores = (batch b in {0,1}) x (query-slice qs in {0..3}, 1024
queries each). Each core computes K/V projections for the full N of its
batch (cheap) and attention + output projection for its query slice; the
host concatenates slices. No collectives.

Per-core dataflow (scores kept transposed so softmax-sum and the AV
contraction both run on the m axis without transposing the big matrix):
  sT[m, q] = sum_kd kk[kd, m] qq[kd, q]     (PE, fp32r, 32x128 row-tiled:
                                             2 heads concurrently)
  e = exp(sT)                               (ScalarE PSUM->SBUF bf16; max-
                                             subtraction skipped: |s|<~45)
  av[q, (h: d|den)] += e[mtile]^T @ [v|1]   (PE bf16, exp as stationary)
  xx = relu(av) * recip(den)                (VectorE, per-partition scalar)
  out = Wp^T @ xx^T + bp                    (PE transpose + fp32r matmul)
"""

import sys

for p in ("/opt/trn_rl_repo", "/opt/trn_rl_repo/concourse"):
    if p not in sys.path:
        sys.path.insert(0, p)

import numpy as np

import concourse.bass as bass
import concourse.mybir as mybir
import concourse.tile as tile
from concourse.bass_utils import run_bass_kernel_spmd
from concourse.tile_rust import add_dep_helper

F32 = mybir.dt.float32
F32R = mybir.dt.float32r
BF16 = mybir.dt.bfloat16
AF = mybir.ActivationFunctionType

NUM_HEADS, KD, D = 8, 16, 32
B, C, H, W = 2, 256, 64, 64
N = H * W            # 4096 keys per batch
QCH = 1024           # queries per core
NMT = N // 128       # 32 m-tiles
HB = 33              # per-head AV block: 32 v-cols + 1 denominator col


def build_nc(trace_scopes=False):
    nc = bass.Bass()

    rgb_s = nc.dram_tensor("rgb_s", [C, QCH], F32R, kind="ExternalInput")
    edge = nc.dram_tensor("edge", [C, N], F32R, kind="ExternalInput")
    w_qA = nc.dram_tensor("w_qA", [C, 128], F32R, kind="ExternalInput")
    w_qB = nc.dram_tensor("w_qB", [C, 128], F32R, kind="ExternalInput")
    w_kA = nc.dram_tensor("w_kA", [C, 128], F32R, kind="ExternalInput")
    w_kB = nc.dram_tensor("w_kB", [C, 128], F32R, kind="ExternalInput")
    w_v = nc.dram_tensor("w_v", [C, 264], F32R, kind="ExternalInput")
    w_p = nc.dram_tensor("w_p", [256, C], F32R, kind="ExternalInput")
    b_qA = nc.dram_tensor("b_qA", [128, 1], F32, kind="ExternalInput")
    b_qB = nc.dram_tensor("b_qB", [128, 1], F32, kind="ExternalInput")
    b_kA = nc.dram_tensor("b_kA", [128, 1], F32, kind="ExternalInput")
    b_kB = nc.dram_tensor("b_kB", [128, 1], F32, kind="ExternalInput")
    b_v = nc.dram_tensor("b_v", [128, 264], F32, kind="ExternalInput")
    b_p = nc.dram_tensor("b_p", [C, 1], F32, kind="ExternalInput")
    ident = nc.dram_tensor("ident", [128, 128], F32, kind="ExternalInput")
    out = nc.dram_tensor("out", [C, QCH], F32, kind="ExternalOutput")

    with tile.TileContext(nc) as tc:
        with tc.tile_pool(name="const", bufs=1) as cp, \
             tc.tile_pool(name="data", bufs=1) as dp:
            wq = [cp.tile([128, 256], F32R, name=f"wq{x}", tag=f"wq{x}") for x in "AB"]
            wk = [cp.tile([128, 256], F32R, name=f"wk{x}", tag=f"wk{x}") for x in "AB"]
            wv = cp.tile([128, 528], F32R, name="wv", tag="wv")
            wp = cp.tile([128, 512], F32R, name="wp", tag="wp")
            bq = [cp.tile([128, 1], F32, name=f"bq{x}", tag=f"bq{x}") for x in "AB"]
            bk = [cp.tile([128, 1], F32, name=f"bk{x}", tag=f"bk{x}") for x in "AB"]
            bv = cp.tile([128, 264], F32, name="bv", tag="bv")
            bp = cp.tile([128, 2], F32, name="bp", tag="bp")
            idn = cp.tile([128, 128], F32, name="idn", tag="idn")
            zrow = cp.tile([1, 128], BF16, name="zrow", tag="zrow")

            nc.sync.dma_start(bp[:, 0:1], b_p[0:128, :])
            nc.sync.dma_start(bp[:, 1:2], b_p[128:256, :])
            nc.sync.dma_start(bq[0][:], b_qA[:])
            nc.sync.dma_start(bq[1][:], b_qB[:])
            nc.sync.dma_start(bk[0][:], b_kA[:])
            nc.sync.dma_start(bk[1][:], b_kB[:])
            nc.sync.dma_start(bv[:], b_v[:])
            nc.vector.memset(zrow[:], 0.0)

            rgb_sb = dp.tile([128, 2 * QCH], F32R, name="rgb_sb", tag="rgb")
            edge_sb = [dp.tile([128, N], F32R, name=f"edge{k}", tag=f"edge{k}") for k in range(2)]

            # Direct DMA into the f32r tiles (f32r bits == f32; the HW
            # matmul rounds on read). Matmuls that end up with multiple
            # DMA-lane waits are legalized by the wait-splitter post-pass.
            for k in range(2):
                nc.sync.dma_start(wq[0][:, 128 * k:128 * (k + 1)], w_qA[128 * k:128 * (k + 1), :])
                nc.sync.dma_start(wq[1][:, 128 * k:128 * (k + 1)], w_qB[128 * k:128 * (k + 1), :])
                nc.sync.dma_start(wk[0][:, 128 * k:128 * (k + 1)], w_kA[128 * k:128 * (k + 1), :])
                nc.sync.dma_start(wk[1][:, 128 * k:128 * (k + 1)], w_kB[128 * k:128 * (k + 1), :])
                nc.sync.dma_start(wv[:, 264 * k:264 * (k + 1)], w_v[128 * k:128 * (k + 1), :])
                nc.sync.dma_start(wp[:, 256 * k:256 * (k + 1)], w_p[128 * k:128 * (k + 1), :])
                nc.sync.dma_start(rgb_sb[:, QCH * k:QCH * (k + 1)], rgb_s[128 * k:128 * (k + 1), :])
                nc.sync.dma_start(edge_sb[k][:], edge[128 * k:128 * (k + 1), :])
            nc.sync.dma_start(idn[:], ident[:])

            qq = [dp.tile([128, QCH], F32R, name=f"qq{x}", tag=f"qq{x}") for x in "AB"]
            kk = [dp.tile([128, N], F32R, name=f"kk{x}", tag=f"kk{x}") for x in "AB"]
            vto = dp.tile([128, NMT * 8 * HB], BF16, name="vto", tag="vto")
            outb = [dp.tile([128, QCH], F32, name=f"outb{k}", tag=f"outb{k}") for k in range(2)]

            # ---- pools (PSUM pools span the whole kernel: releasing a
            # PSUM pool makes successor tiles inherit released-zone deps
            # as multi-wait instructions, which walrus rejects) ----
            scp = tc.alloc_tile_pool(name="scp", bufs=2, space="PSUM")
            wps = tc.alloc_tile_pool(name="wps", bufs=4, space="PSUM")
            ep = tc.alloc_tile_pool(name="exp", bufs=20)
            sp = tc.alloc_tile_pool(name="stg", bufs=6)

            # ---- projections ----
            if True:
                pps = wps
                for x in range(2):          # qq (A/B head groups)
                    for j in range(2):
                        ps = pps.tile([128, 512], F32, name="ps_q", tag="w")
                        for k in range(2):
                            nc.tensor.matmul(
                                ps[:], wq[x][:, 128 * k:128 * (k + 1)],
                                rgb_sb[:, QCH * k + 512 * j:QCH * k + 512 * (j + 1)],
                                start=(k == 0), stop=(k == 1))
                        nc.vector.tensor_scalar_add(qq[x][:, 512 * j:512 * (j + 1)], ps[:], bq[x][:])
                for x in range(2):          # kk
                    for j in range(8):
                        ps = pps.tile([128, 512], F32, name="ps_k", tag="w")
                        for k in range(2):
                            nc.tensor.matmul(
                                ps[:], wk[x][:, 128 * k:128 * (k + 1)],
                                edge_sb[k][:, 512 * j:512 * (j + 1)],
                                start=(k == 0), stop=(k == 1))
                        nc.vector.tensor_scalar_add(kk[x][:, 512 * j:512 * (j + 1)], ps[:], bk[x][:])
                # v^T: vto[m, (h: 32 v | den)] per m-tile. wv has zero
                # columns at the denominator positions and bv carries the
                # 1.0s there, so no memset / strided writes are needed.
                for mt in range(NMT):
                    ps = pps.tile([128, 264], F32, name="ps_v", tag="w")
                    for k in range(2):
                        nc.tensor.matmul(
                            ps[:], edge_sb[k][:, 128 * mt:128 * (mt + 1)],
                            wv[:, 264 * k:264 * (k + 1)],
                            start=(k == 0), stop=(k == 1))
                    nc.vector.tensor_add(
                        vto[:, 8 * HB * mt:8 * HB * (mt + 1)], ps[:], bv[:])

            # ACT warmup: absorb the DVE tick (activation-bias const
            # tiles are DVE-written) so the first exp carries only the PE
            # wait — walrus allows a single wait per Activation.
            actw = dp.tile([1, 1], BF16, name="actw", tag="actw")
            nc.scalar.activation(actw[:], zrow[0:1, 0:1], AF.Exp)

            # ---- attention + output projection ----
            if True:
                for qc in range(2):
                    q0 = 512 * qc
                    av = []
                    for s in range(4):
                        a = wps.tile([128, 8 * HB], F32, name=f"av{s}", tag="w")
                        # one whole-bank start=True clear; the AV matmuls
                        # below all use start=False (first per-element write
                        # overwrites, later ones accumulate)
                        nc.tensor.matmul(a[:], zrow[:], vto[0:1, 0:8 * HB], start=True, stop=False)
                        av.append(a)
                    for x in range(2):
                        for pr in range(2):
                            # mtiles in pairs: PE switches tiling mode
                            # (32x128 QKT <-> 128x128 AV) once per pair
                            # instead of once per mtile (mode switch = PE
                            # drain)
                            for mt0 in range(0, NMT, 2):
                                ets = []
                                for mt in (mt0, mt0 + 1):
                                    sc = scp.tile([128, 1024], F32, name="sc", tag="sc")
                                    for j2 in range(2):
                                        j = 2 * pr + j2
                                        nc.tensor.matmul(
                                            sc[:, 512 * j2:512 * (j2 + 1)],
                                            kk[x][32 * j:32 * j + KD, 128 * mt:128 * (mt + 1)],
                                            qq[x][32 * j:32 * j + KD, q0:q0 + 512],
                                            start=True, stop=True,
                                            tile_position=(32 * j, 0))
                                    et = ep.tile([128, 1024], BF16, name="et", tag="et")
                                    nc.scalar.activation(et[:], sc[:], AF.Exp)
                                    ets.append(et)
                                for mt, et in zip((mt0, mt0 + 1), ets):
                                    for j2 in range(2):
                                        h = 4 * x + 2 * pr + j2
                                        for s in range(4):
                                            nc.tensor.matmul(
                                                av[s][:, HB * h:HB * (h + 1)],
                                                et[:, 512 * j2 + 128 * s:512 * j2 + 128 * (s + 1)],
                                                vto[:, 8 * HB * mt + HB * h:8 * HB * mt + HB * (h + 1)],
                                                start=False, stop=(mt == NMT - 1))
                    # normalize + relu + transpose + project
                    xxt = [sp.tile([128, 512], F32R, name=f"xxt{k}", tag=f"xxt{k}") for k in range(2)]
                    for s in range(4):
                        xxm = sp.tile([128, 8 * HB], F32, name="xxm", tag="xxm")
                        nc.vector.tensor_scalar_max(xxm[:], av[s][:], 0.0)
                        rden = sp.tile([128, 8], F32, name="rden", tag="rden")
                        nc.vector.reciprocal(
                            rden[:], xxm[:].rearrange("p (h x) -> p h x", x=HB)[:, :, 32])
                        xnm = sp.tile([128, 256], F32, name="xnm", tag="xnm")
                        for h in range(NUM_HEADS):
                            nc.vector.tensor_scalar_mul(
                                xnm[:, 32 * h:32 * (h + 1)], xxm[:, HB * h:HB * h + 32],
                                rden[:, h:h + 1])
                        for k in range(2):
                            tp = wps.tile([128, 128], F32, name="tp", tag="w")
                            nc.tensor.transpose(tp[:], xnm[:, 128 * k:128 * (k + 1)], idn[:])
                            nc.vector.tensor_copy(xxt[k][:, 128 * s:128 * (s + 1)], tp[:])
                    for ct in range(2):
                        ps = wps.tile([128, 512], F32, name="ps_p", tag="w")
                        for k in range(2):
                            nc.tensor.matmul(
                                ps[:], wp[:, 256 * k + 128 * ct:256 * k + 128 * (ct + 1)],
                                xxt[k][:], start=(k == 0), stop=(k == 1))
                        nc.vector.tensor_scalar_add(
                            outb[ct][:, q0:q0 + 512], ps[:], bp[:, ct:ct + 1])
            for ct in range(2):
                nc.gpsimd.dma_start(out[128 * ct:128 * (ct + 1), :], outb[ct][:])
            for _p in (sp, ep, wps, scp):
                _p.release()

    # walrus codegen accepts only ONE sync wait on compute instructions
    # (Matmult / Activation / TensorTensor / ...). The multi-wait cases
    # Tile emits here are all {self-engine, other}: a self-engine wait
    # orders an instruction against an earlier instruction on the SAME
    # in-order engine (WAW through PE's single PSUM write port, ACT/DVE
    # pipeline order), which the hardware already guarantees — drop it.
    _self_prefix = {
        "EngineType.PE": "PE",
        "EngineType.Activation": "Activation",
        "EngineType.DVE": "DVE",
        "EngineType.Pool": "Pool",
        "EngineType.SP": "SP",
    }
    for f in nc.m.functions:
        for bb in f.blocks:
            for inst in bb.instructions:
                si = inst.sync_info
                if si is None or not si.on_wait or len(si.on_wait) < 2:
                    continue
                pref = _self_prefix.get(str(getattr(inst, "engine", "")), None)
                if pref is None:
                    continue
                kept = [w for w in si.on_wait
                        if not str(w.ant_name).startswith(pref)]
                if not kept or len(kept) == len(si.on_wait):
                    continue
                si.on_wait = kept

    # Safety net: any instruction still carrying >1 wait gets all but its
    # last wait hoisted into preceding same-engine NoOps (1 wait each).
    uid = [0]
    for f in nc.m.functions:
        for bb in f.blocks:
            new_insts = []
            for inst in bb.instructions:
                si = inst.sync_info
                if si is not None and si.on_wait and len(si.on_wait) > 1:
                    for w in si.on_wait[:-1]:
                        uid[0] += 1
                        nop = mybir.InstNoOp(
                            name=f"I-waitsplit-{uid[0]}", ins=[], outs=[])
                        nop.engine = inst.engine
                        nop.sync_info = mybir.SyncInfo(
                            on_wait=[w], on_update=[])
                        new_insts.append(nop)
                    si.on_wait = [si.on_wait[-1]]
                new_insts.append(inst)
            bb.instructions = new_insts
    return nc


_CACHE = {}


def _prep_host(inputs):
    """Fold BN into weights; build head-split layouts shared by all cores."""
    f = np.float32
    Wq = (inputs["Wq"] * inputs["sq"][:, None]).astype(f)
    Wk = (inputs["Wk"] * inputs["sk"][:, None]).astype(f)
    Wv = (inputs["Wv"] * inputs["sv"][:, None]).astype(f)
    Wp = (inputs["Wp"] * inputs["sp"][:, None]).astype(f)

    def split(Wt, b):
        o = []
        for g in range(2):
            Wx = np.zeros((C, 128), f)
            bx = np.zeros((128, 1), f)
            for j in range(4):
                h = 4 * g + j
                Wx[:, 32 * j:32 * j + KD] = Wt[:, KD * h:KD * (h + 1)]
                bx[32 * j:32 * j + KD, 0] = b[KD * h:KD * (h + 1)]
            o.append((np.ascontiguousarray(Wx), bx))
        return o

    (wqA, bqA), (wqB, bqB) = split(Wq.T.astype(f), inputs["bq"])
    (wkA, bkA), (wkB, bkB) = split(Wk.T.astype(f), inputs["bk"])
    WvT = Wv.T.astype(f)                      # [C, 256] cols (h, d)
    wv_ext = np.zeros((C, 264), f)            # col 33h+32 stays 0
    bv_ext = np.zeros((264,), f)
    for h in range(NUM_HEADS):
        wv_ext[:, HB * h:HB * h + 32] = WvT[:, 32 * h:32 * (h + 1)]
        bv_ext[HB * h:HB * h + 32] = inputs["bv"][32 * h:32 * (h + 1)]
        bv_ext[HB * h + 32] = 1.0             # softmax denominator column
    return dict(
        w_qA=wqA, w_qB=wqB, w_kA=wkA, w_kB=wkB,
        w_v=wv_ext, w_p=np.ascontiguousarray(Wp.T),
        b_qA=bqA, b_qB=bqB, b_kA=bkA, b_kB=bkB,
        b_v=np.ascontiguousarray(np.broadcast_to(bv_ext, (128, 264))),
        b_p=inputs["bp"].astype(f).reshape(C, 1),
        ident=np.eye(128, dtype=f),
    )


def kernel(**inputs) -> np.ndarray:
    inputs = {k: np.asarray(v) for k, v in inputs.items()}
    if "nc" not in _CACHE:
        _CACHE["nc"] = build_nc()
    nc = _CACHE["nc"]

    shared = _prep_host(inputs)
    rgb = np.ascontiguousarray(inputs["rgb"].astype(np.float32).reshape(B, C, N))
    edge = np.ascontiguousarray(inputs["edge"].astype(np.float32).reshape(B, C, N))

    in_maps = []
    for core in range(8):
        b, qs = core // 4, core % 4
        m = dict(shared)
        m["rgb_s"] = np.ascontiguousarray(rgb[b][:, QCH * qs:QCH * (qs + 1)])
        m["edge"] = edge[b]
        in_maps.append(m)

    res = run_bass_kernel_spmd(nc, in_maps, core_ids=list(range(8)))
    full = np.zeros((B, C, N), np.float32)
    for core in range(8):
        b, qs = core // 4, core % 4
        full[b][:, QCH * qs:QCH * (qs + 1)] = res.results[core]["out"]
    return full.reshape(B, C, H, W)



# revision 7
# speedup vs baseline: 1.0119x; 1.0119x over previous
"""Cross-modal attention Trainium2 kernel.

Reference computation (all 1x1 convs + folded eval-mode BN):
  q = BN(Wq @ rgb), k = BN(Wk @ edge), v = BN(Wv @ edge)
  attn = softmax(q^T k) per head; xx = relu(attn @ v); out = BN(Wp @ xx)

Shapes: B=2, C=256, H=W=64 (N=4096), heads=8, key_dim=16, d=32.

Sharding: 8 cores = (batch b in {0,1}) x (query-slice qs in {0..3}, 1024
queries each). Each core computes K/V projections for the full N of its
batch (cheap) and attention + output projection for its query slice; the
host concatenates slices. No collectives.

Per-core dataflow (scores kept transposed so softmax-sum and the AV
contraction both run on the m axis without transposing the big matrix):
  sT[m, q] = sum_kd kk[kd, m] qq[kd, q]     (PE, fp32r, 32x128 row-tiled)
  e = exp(sT)        split across two engines, balanced ~53/47:
      ACT:  exp activation (PSUM->SBUF bf16)
      DVE:  Schraudolph bitwise exp: bf16_bits(int16(s*184.665 + 16248.6))
            = 2^(s*log2 e) with ~+-3% multiplicative error that cancels in
            softmax normalization (max |s| ~= 45, safe window (-88, +89)).
  av[q, (h: d|den)] += e[mtile]^T @ [v|1]   (PE bf16, exp as stationary)
  xx = relu(av), xnm = xx * recip(den)      (ACT relu; Pool per-head scale)
  out = Wp^T @ xxt + bp                     (PE bf16 transpose, DMA PSUM->SBUF
                                             move, matmul, ACT bias add)

Engine budget per core (TimelineSim): PE ~163us (QK^T 109 + AV 30 + proj/
misc 24), ACT ~158us (exp share + bias adds + relu), DVE ~158us (exp share
+ v moves), Pool/SP/DMA light. Baseline (all exp on ACT) was 315us.
"""

import sys

for p in ("/opt/trn_rl_repo", "/opt/trn_rl_repo/concourse"):
    if p not in sys.path:
        sys.path.insert(0, p)

import numpy as np

import concourse.bass as bass
import concourse.mybir as mybir
import concourse.tile as tile
from concourse.bass_utils import run_bass_kernel_spmd

F32 = mybir.dt.float32
F32R = mybir.dt.float32r
BF16 = mybir.dt.bfloat16
I16 = mybir.dt.int16
AF = mybir.ActivationFunctionType
ALU = mybir.AluOpType

NUM_HEADS, KD, D = 8, 16, 32
B, C, H, W = 2, 256, 64, 64
N = H * W            # 4096 keys per batch
QCH = 1024           # queries per core
NMT = N // 128       # 32 m-tiles
HB = 33              # per-head AV block: 32 v-cols + 1 denominator col

# Schraudolph exp: bf16_bits(int16(s*SCH_C1 + SCH_C2)) ~= e^s.
SCH_C1 = 184.66496580927726     # 128 * log2(e)
SCH_C2 = 16248.6                # 127*128 minus mean-zeroing interp bias
ACT_FRAC = 0.5266               # share of exp tiles on ACT (rest on DVE)


def build_nc(act_frac=ACT_FRAC):
    nc = bass.Bass()

    rgb_s = nc.dram_tensor("rgb_s", [C, QCH], F32R, kind="ExternalInput")
    edge = nc.dram_tensor("edge", [C, N], F32R, kind="ExternalInput")
    w_qA = nc.dram_tensor("w_qA", [C, 128], F32R, kind="ExternalInput")
    w_qB = nc.dram_tensor("w_qB", [C, 128], F32R, kind="ExternalInput")
    w_kA = nc.dram_tensor("w_kA", [C, 128], F32R, kind="ExternalInput")
    w_kB = nc.dram_tensor("w_kB", [C, 128], F32R, kind="ExternalInput")
    w_v = nc.dram_tensor("w_v", [C, 264], F32R, kind="ExternalInput")
    w_p = nc.dram_tensor("w_p", [256, C], BF16, kind="ExternalInput")
    b_qA = nc.dram_tensor("b_qA", [128, 1], F32, kind="ExternalInput")
    b_qB = nc.dram_tensor("b_qB", [128, 1], F32, kind="ExternalInput")
    b_kA = nc.dram_tensor("b_kA", [128, 1], F32, kind="ExternalInput")
    b_kB = nc.dram_tensor("b_kB", [128, 1], F32, kind="ExternalInput")
    b_v = nc.dram_tensor("b_v", [128, 264], F32, kind="ExternalInput")
    b_p = nc.dram_tensor("b_p", [C, 1], F32, kind="ExternalInput")
    ident = nc.dram_tensor("ident", [128, 128], BF16, kind="ExternalInput")
    out = nc.dram_tensor("out", [C, QCH], F32, kind="ExternalOutput")

    with tile.TileContext(nc) as tc:
        with tc.tile_pool(name="const", bufs=1) as cp, \
             tc.tile_pool(name="data", bufs=1) as dp:
            wq = [cp.tile([128, 256], F32R, name=f"wq{x}", tag=f"wq{x}") for x in "AB"]
            wk = [cp.tile([128, 256], F32R, name=f"wk{x}", tag=f"wk{x}") for x in "AB"]
            wv = cp.tile([128, 528], F32R, name="wv", tag="wv")
            wp = cp.tile([128, 512], BF16, name="wp", tag="wp")
            bq = [cp.tile([128, 1], F32, name=f"bq{x}", tag=f"bq{x}") for x in "AB"]
            bk = [cp.tile([128, 1], F32, name=f"bk{x}", tag=f"bk{x}") for x in "AB"]
            bv = cp.tile([128, 264], F32, name="bv", tag="bv")
            bp = cp.tile([128, 2], F32, name="bp", tag="bp")
            idn = cp.tile([128, 128], BF16, name="idn", tag="idn")
            zrow = cp.tile([1, 128], BF16, name="zrow", tag="zrow")

            nc.sync.dma_start(bp[:, 0:1], b_p[0:128, :])
            nc.sync.dma_start(bp[:, 1:2], b_p[128:256, :])
            nc.sync.dma_start(bq[0][:], b_qA[:])
            nc.sync.dma_start(bq[1][:], b_qB[:])
            nc.sync.dma_start(bk[0][:], b_kA[:])
            nc.sync.dma_start(bk[1][:], b_kB[:])
            nc.sync.dma_start(bv[:], b_v[:])
            nc.vector.memset(zrow[:], 0.0)

            rgb_sb = dp.tile([128, 2 * QCH], F32R, name="rgb_sb", tag="rgb")
            edge_sb = [dp.tile([128, N], F32R, name=f"edge{k}", tag=f"edge{k}") for k in range(2)]

            # Direct DMA into the f32r tiles (f32r bits == f32; the HW
            # matmul rounds on read). Matmuls that end up with multiple
            # DMA-lane waits are legalized by the wait-splitter post-pass.
            for k in range(2):
                nc.sync.dma_start(wq[0][:, 128 * k:128 * (k + 1)], w_qA[128 * k:128 * (k + 1), :])
                nc.sync.dma_start(wq[1][:, 128 * k:128 * (k + 1)], w_qB[128 * k:128 * (k + 1), :])
                nc.sync.dma_start(wk[0][:, 128 * k:128 * (k + 1)], w_kA[128 * k:128 * (k + 1), :])
                nc.sync.dma_start(wk[1][:, 128 * k:128 * (k + 1)], w_kB[128 * k:128 * (k + 1), :])
                nc.sync.dma_start(wv[:, 264 * k:264 * (k + 1)], w_v[128 * k:128 * (k + 1), :])
                nc.sync.dma_start(wp[:, 256 * k:256 * (k + 1)], w_p[128 * k:128 * (k + 1), :])
                nc.sync.dma_start(rgb_sb[:, QCH * k:QCH * (k + 1)], rgb_s[128 * k:128 * (k + 1), :])
                nc.sync.dma_start(edge_sb[k][:], edge[128 * k:128 * (k + 1), :])
            nc.sync.dma_start(idn[:], ident[:])

            qq = [dp.tile([128, QCH], F32R, name=f"qq{x}", tag=f"qq{x}") for x in "AB"]
            kk = [dp.tile([128, N], F32R, name=f"kk{x}", tag=f"kk{x}") for x in "AB"]
            vto = dp.tile([128, NMT * 8 * HB], BF16, name="vto", tag="vto")
            outb = [dp.tile([128, QCH], F32, name=f"outb{k}", tag=f"outb{k}") for k in range(2)]
            xxt = [dp.tile([128, 512], BF16, name=f"xxt{k}", tag=f"xxt{k}") for k in range(2)]

            # ---- pools (PSUM pools span the whole kernel: releasing a
            # PSUM pool makes successor tiles inherit released-zone deps
            # as multi-wait instructions, which walrus rejects) ----
            scp = tc.alloc_tile_pool(name="scp", bufs=2, space="PSUM")
            wps = tc.alloc_tile_pool(name="wps", bufs=4, space="PSUM")
            ep = tc.alloc_tile_pool(name="exp", bufs=10)
            sp = tc.alloc_tile_pool(name="stg", bufs=6)

            # ---- projections ----
            # qq/kk bias adds ride on ACT (Identity+bias, per-partition
            # scalar); vto's bias varies along the free dim so it stays a
            # DVE tensor_tensor add.
            for x in range(2):          # qq (A/B head groups)
                for j in range(2):
                    ps = wps.tile([128, 512], F32, name="ps_q", tag="w")
                    for k in range(2):
                        nc.tensor.matmul(
                            ps[:], wq[x][:, 128 * k:128 * (k + 1)],
                            rgb_sb[:, QCH * k + 512 * j:QCH * k + 512 * (j + 1)],
                            start=(k == 0), stop=(k == 1))
                    nc.scalar.activation(
                        qq[x][:, 512 * j:512 * (j + 1)], ps[:],
                        AF.Identity, bias=bq[x][:])
            for x in range(2):          # kk
                for j in range(8):
                    ps = wps.tile([128, 512], F32, name="ps_k", tag="w")
                    for k in range(2):
                        nc.tensor.matmul(
                            ps[:], wk[x][:, 128 * k:128 * (k + 1)],
                            edge_sb[k][:, 512 * j:512 * (j + 1)],
                            start=(k == 0), stop=(k == 1))
                    nc.scalar.activation(
                        kk[x][:, 512 * j:512 * (j + 1)], ps[:],
                        AF.Identity, bias=bk[x][:])
            # v^T: vto[m, (h: 32 v | den)] per m-tile. wv has zero
            # columns at the denominator positions and bv carries the
            # 1.0s there, so no memset / strided writes are needed.
            for mt in range(NMT):
                ps = wps.tile([128, 264], F32, name="ps_v", tag="w")
                for k in range(2):
                    nc.tensor.matmul(
                        ps[:], edge_sb[k][:, 128 * mt:128 * (mt + 1)],
                        wv[:, 264 * k:264 * (k + 1)],
                        start=(k == 0), stop=(k == 1))
                nc.vector.tensor_add(
                    vto[:, 8 * HB * mt:8 * HB * (mt + 1)], ps[:], bv[:])

            # ACT warmup: absorb the DVE tick (activation-bias const
            # tiles are DVE-written) so the first exp carries only the PE
            # wait — walrus allows a single wait per Activation.
            actw = dp.tile([1, 1], BF16, name="actw", tag="actw")
            nc.scalar.activation(actw[:], zrow[0:1, 0:1], AF.Exp)

            # Bresenham dispatcher: which engine computes exp of each
            # score tile (ACT exact exp vs DVE bitwise 2^x).
            exp_acc = [0.0]

            def emit_exp(et, sc):
                exp_acc[0] += act_frac
                if exp_acc[0] >= 1.0:
                    exp_acc[0] -= 1.0
                    nc.scalar.activation(et[:], sc[:], AF.Exp)
                else:
                    nc.vector.tensor_scalar(
                        et[:].bitcast(I16), sc[:], SCH_C1, SCH_C2,
                        ALU.mult, ALU.add)

            # ---- attention + output projection ----
            for qc in range(2):
                q0 = 512 * qc
                av = []
                for s in range(4):
                    a = wps.tile([128, 8 * HB], F32, name=f"av{s}", tag="w")
                    # one whole-bank start=True clear; the AV matmuls
                    # below all use start=False (first per-element write
                    # overwrites, later ones accumulate)
                    nc.tensor.matmul(a[:], zrow[:], vto[0:1, 0:8 * HB], start=True, stop=False)
                    av.append(a)
                for x in range(2):
                    for pr in range(2):
                        # mtiles in pairs: PE switches tiling mode
                        # (32x128 QKT <-> 128x128 AV) once per pair
                        # instead of once per mtile (mode switch = PE
                        # drain)
                        for mt0 in range(0, NMT, 2):
                            ets = []
                            for mt in (mt0, mt0 + 1):
                                sc = scp.tile([128, 1024], F32, name="sc", tag="sc")
                                for j2 in range(2):
                                    j = 2 * pr + j2
                                    nc.tensor.matmul(
                                        sc[:, 512 * j2:512 * (j2 + 1)],
                                        kk[x][32 * j:32 * j + KD, 128 * mt:128 * (mt + 1)],
                                        qq[x][32 * j:32 * j + KD, q0:q0 + 512],
                                        start=True, stop=True,
                                        tile_position=(32 * j, 0))
                                et = ep.tile([128, 1024], BF16, name="et", tag="et")
                                emit_exp(et, sc)
                                ets.append(et)
                            for mt, et in zip((mt0, mt0 + 1), ets):
                                for j2 in range(2):
                                    h = 4 * x + 2 * pr + j2
                                    for s in range(4):
                                        nc.tensor.matmul(
                                            av[s][:, HB * h:HB * (h + 1)],
                                            et[:, 512 * j2 + 128 * s:512 * j2 + 128 * (s + 1)],
                                            vto[:, 8 * HB * mt + HB * h:8 * HB * mt + HB * (h + 1)],
                                            start=False, stop=(mt == NMT - 1))
                # normalize + relu + transpose + project
                for s in range(4):
                    xxm = sp.tile([128, 8 * HB], F32, name="xxm", tag="xxm")
                    nc.scalar.activation(xxm[:], av[s][:], AF.Relu)
                    rden = sp.tile([128, 8], F32, name="rden", tag="rden")
                    nc.vector.reciprocal(
                        rden[:], xxm[:].rearrange("p (h x) -> p h x", x=HB)[:, :, 32])
                    xnm = sp.tile([128, 256], BF16, name="xnm", tag="xnm")
                    for hh in range(NUM_HEADS):
                        nc.gpsimd.tensor_scalar_mul(
                            xnm[:, 32 * hh:32 * (hh + 1)],
                            xxm[:, HB * hh:HB * hh + 32],
                            rden[:, hh:hh + 1])
                    for k in range(2):
                        tp = wps.tile([128, 128], BF16, name="tp", tag="w")
                        nc.tensor.transpose(tp[:], xnm[:, 128 * k:128 * (k + 1)], idn[:])
                        nc.vector.tensor_copy(xxt[k][:, 128 * s:128 * (s + 1)], tp[:])
                for ct in range(2):
                    ps = wps.tile([128, 512], F32, name="ps_p", tag="w")
                    for k in range(2):
                        nc.tensor.matmul(
                            ps[:], wp[:, 256 * k + 128 * ct:256 * k + 128 * (ct + 1)],
                            xxt[k][:], start=(k == 0), stop=(k == 1))
                    nc.scalar.activation(
                        outb[ct][:, q0:q0 + 512], ps[:],
                        AF.Identity, bias=bp[:, ct:ct + 1])
            for ct in range(2):
                nc.gpsimd.dma_start(out[128 * ct:128 * (ct + 1), :], outb[ct][:])
            for _p in (sp, ep, wps, scp):
                _p.release()

    # walrus codegen accepts only ONE sync wait on compute instructions
    # (Matmult / Activation / TensorTensor / ...). The multi-wait cases
    # Tile emits here are all {self-engine, other}: a self-engine wait
    # orders an instruction against an earlier instruction on the SAME
    # in-order engine (WAW through PE's single PSUM write port, ACT/DVE
    # pipeline order), which the hardware already guarantees — drop it.
    _self_prefix = {
        "EngineType.PE": "PE",
        "EngineType.Activation": "Activation",
        "EngineType.DVE": "DVE",
        "EngineType.Pool": "Pool",
        "EngineType.SP": "SP",
    }
    for f in nc.m.functions:
        for bb in f.blocks:
            for inst in bb.instructions:
                si = inst.sync_info
                if si is None or not si.on_wait or len(si.on_wait) < 2:
                    continue
                pref = _self_prefix.get(str(getattr(inst, "engine", "")), None)
                if pref is None:
                    continue
                kept = [w for w in si.on_wait
                        if not str(w.ant_name).startswith(pref)]
                if not kept or len(kept) == len(si.on_wait):
                    continue
                si.on_wait = kept

    # Safety net: any instruction still carrying >1 wait gets all but its
    # last wait hoisted into preceding same-engine NoOps (1 wait each).
    uid = [0]
    for f in nc.m.functions:
        for bb in f.blocks:
            new_insts = []
            for inst in bb.instructions:
                si = inst.sync_info
                if si is not None and si.on_wait and len(si.on_wait) > 1:
                    for w in si.on_wait[:-1]:
                        uid[0] += 1
                        nop = mybir.InstNoOp(
                            name=f"I-waitsplit-{uid[0]}", ins=[], outs=[])
                        nop.engine = inst.engine
                        nop.sync_info = mybir.SyncInfo(
                            on_wait=[w], on_update=[])
                        new_insts.append(nop)
                    si.on_wait = [si.on_wait[-1]]
                new_insts.append(inst)
            bb.instructions = new_insts
    return nc


_CACHE = {}


def _prep_host(inputs):
    """Fold BN into weights; build head-split layouts shared by all cores."""
    import ml_dtypes
    f = np.float32
    Wq = (inputs["Wq"] * inputs["sq"][:, None]).astype(f)
    Wk = (inputs["Wk"] * inputs["sk"][:, None]).astype(f)
    Wv = (inputs["Wv"] * inputs["sv"][:, None]).astype(f)
    Wp = (inputs["Wp"] * inputs["sp"][:, None]).astype(f)

    def split(Wt, b):
        o = []
        for g in range(2):
            Wx = np.zeros((C, 128), f)
            bx = np.zeros((128, 1), f)
            for j in range(4):
                h = 4 * g + j
                Wx[:, 32 * j:32 * j + KD] = Wt[:, KD * h:KD * (h + 1)]
                bx[32 * j:32 * j + KD, 0] = b[KD * h:KD * (h + 1)]
            o.append((np.ascontiguousarray(Wx), bx))
        return o

    (wqA, bqA), (wqB, bqB) = split(Wq.T.astype(f), inputs["bq"])
    (wkA, bkA), (wkB, bkB) = split(Wk.T.astype(f), inputs["bk"])
    WvT = Wv.T.astype(f)                      # [C, 256] cols (h, d)
    wv_ext = np.zeros((C, 264), f)            # col 33h+32 stays 0
    bv_ext = np.zeros((264,), f)
    for h in range(NUM_HEADS):
        wv_ext[:, HB * h:HB * h + 32] = WvT[:, 32 * h:32 * (h + 1)]
        bv_ext[HB * h:HB * h + 32] = inputs["bv"][32 * h:32 * (h + 1)]
        bv_ext[HB * h + 32] = 1.0             # softmax denominator column
    ident_bf16 = np.eye(128, dtype=ml_dtypes.bfloat16)
    return dict(
        w_qA=wqA, w_qB=wqB, w_kA=wkA, w_kB=wkB,
        w_v=wv_ext, w_p=np.ascontiguousarray(Wp.T).astype(ml_dtypes.bfloat16),
        b_qA=bqA, b_qB=bqB, b_kA=bkA, b_kB=bkB,
        b_v=np.ascontiguousarray(np.broadcast_to(bv_ext, (128, 264))),
        b_p=inputs["bp"].astype(f).reshape(C, 1),
        ident=ident_bf16,
    )


def kernel(**inputs) -> np.ndarray:
    inputs = {k: np.asarray(v) for k, v in inputs.items()}
    if "nc" not in _CACHE:
        _CACHE["nc"] = build_nc()
    nc = _CACHE["nc"]

    shared = _prep_host(inputs)
    rgb = np.ascontiguousarray(inputs["rgb"].astype(np.float32).reshape(B, C, N))
    edge = np.ascontiguousarray(inputs["edge"].astype(np.float32).reshape(B, C, N))

    in_maps = []
    for core in range(8):
        b, qs = core // 4, core % 4
        m = dict(shared)
        m["rgb_s"] = np.ascontiguousarray(rgb[b][:, QCH * qs:QCH * (qs + 1)])
        m["edge"] = edge[b]
        in_maps.append(m)

    res = run_bass_kernel_spmd(nc, in_maps, core_ids=list(range(8)))
    full = np.zeros((B, C, N), np.float32)
    for core in range(8):
        b, qs = core // 4, core % 4
        full[b][:, QCH * qs:QCH * (qs + 1)] = res.results[core]["out"]
    return full.reshape(B, C, H, W)


# revision 9
# speedup vs baseline: 1.3062x; 1.2908x over previous
"""Cross-modal attention Trainium2 kernel.

Reference computation (all 1x1 convs + folded eval-mode BN):
  q = BN(Wq @ rgb), k = BN(Wk @ edge), v = BN(Wv @ edge)
  attn = softmax(q^T k) per head; xx = relu(attn @ v); out = BN(Wp @ xx)

Shapes: B=2, C=256, H=W=64 (N=4096), heads=8, key_dim=16, d=32.

Sharding: 8 cores = (batch b in {0,1}) x (query-slice qs in {0..3}, 1024
queries each). Each core computes K/V projections for the full N of its
batch (cheap) and attention + output projection for its query slice; the
host concatenates slices. No collectives.

Per-core dataflow (scores kept transposed so softmax-sum and the AV
contraction both run on the m axis without transposing the big matrix):
  sT[m, q] = sum_kd kk[kd, m] qq[kd, q]     (PE, fp32r, 32x128 row-tiled)
  e = exp(sT)        split across two engines, balanced ~53/47:
      ACT:  exp activation (PSUM->SBUF bf16)
      DVE:  Schraudolph bitwise exp: bf16_bits(int16(s*184.665 + 16248.6))
            = 2^(s*log2 e) with ~+-3% multiplicative error that cancels in
            softmax normalization (max |s| ~= 45, safe window (-88, +89)).
  av[q, (h: d|den)] += e[mtile]^T @ [v|1]   (PE bf16, exp as stationary)
  xx = relu(av), xnm = xx * recip(den)      (ACT relu; Pool per-head scale)
  out = Wp^T @ xxt + bp                     (PE bf16 transpose, DMA PSUM->SBUF
                                             move, matmul, ACT bias add)

Engine budget per core (TimelineSim): PE ~163us (QK^T 109 + AV 30 + proj/
misc 24), ACT ~158us (exp share + bias adds + relu), DVE ~158us (exp share
+ v moves), Pool/SP/DMA light. Baseline (all exp on ACT) was 315us.
"""

import sys

for p in ("/opt/trn_rl_repo", "/opt/trn_rl_repo/concourse"):
    if p not in sys.path:
        sys.path.insert(0, p)

import numpy as np

import concourse.bass as bass
import concourse.mybir as mybir
import concourse.tile as tile
from concourse.bass_utils import run_bass_kernel_spmd

F32 = mybir.dt.float32
F32R = mybir.dt.float32r
BF16 = mybir.dt.bfloat16
I16 = mybir.dt.int16
AF = mybir.ActivationFunctionType
ALU = mybir.AluOpType

NUM_HEADS, KD, D = 8, 16, 32
B, C, H, W = 2, 256, 64, 64
N = H * W            # 4096 keys per batch
QCH = 1024           # queries per core
NMT = N // 128       # 32 m-tiles
HB = 33              # per-head AV block: 32 v-cols + 1 denominator col

# Schraudolph exp: bf16_bits(int16(s*SCH_C1 + SCH_C2)) ~= e^s.
SCH_C1 = 184.66496580927726     # 128 * log2(e)
SCH_C2 = 16248.6                # 127*128 minus mean-zeroing interp bias
ACT_FRAC = 0.5266               # share of exp tiles on ACT (rest on DVE)


def build_nc(act_frac=ACT_FRAC):
    nc = bass.Bass()

    rgb_s = nc.dram_tensor("rgb_s", [C, QCH], F32R, kind="ExternalInput")
    edge = nc.dram_tensor("edge", [C, N], F32R, kind="ExternalInput")
    w_qA = nc.dram_tensor("w_qA", [C, 128], F32R, kind="ExternalInput")
    w_qB = nc.dram_tensor("w_qB", [C, 128], F32R, kind="ExternalInput")
    w_kA = nc.dram_tensor("w_kA", [C, 128], F32R, kind="ExternalInput")
    w_kB = nc.dram_tensor("w_kB", [C, 128], F32R, kind="ExternalInput")
    w_v = nc.dram_tensor("w_v", [C, 264], F32R, kind="ExternalInput")
    w_p = nc.dram_tensor("w_p", [256, C], BF16, kind="ExternalInput")
    b_qA = nc.dram_tensor("b_qA", [128, 1], F32, kind="ExternalInput")
    b_qB = nc.dram_tensor("b_qB", [128, 1], F32, kind="ExternalInput")
    b_kA = nc.dram_tensor("b_kA", [128, 1], F32, kind="ExternalInput")
    b_kB = nc.dram_tensor("b_kB", [128, 1], F32, kind="ExternalInput")
    b_v = nc.dram_tensor("b_v", [128, 264], F32, kind="ExternalInput")
    b_p = nc.dram_tensor("b_p", [C, 1], F32, kind="ExternalInput")
    ident = nc.dram_tensor("ident", [128, 128], BF16, kind="ExternalInput")
    out = nc.dram_tensor("out", [C, QCH], F32, kind="ExternalOutput")

    with tile.TileContext(nc) as tc:
        with tc.tile_pool(name="const", bufs=1) as cp, \
             tc.tile_pool(name="data", bufs=1) as dp:
            wq = [cp.tile([128, 256], F32R, name=f"wq{x}", tag=f"wq{x}") for x in "AB"]
            wk = [cp.tile([128, 256], F32R, name=f"wk{x}", tag=f"wk{x}") for x in "AB"]
            wv = cp.tile([128, 528], F32R, name="wv", tag="wv")
            wp = cp.tile([128, 512], BF16, name="wp", tag="wp")
            bq = [cp.tile([128, 1], F32, name=f"bq{x}", tag=f"bq{x}") for x in "AB"]
            bk = [cp.tile([128, 1], F32, name=f"bk{x}", tag=f"bk{x}") for x in "AB"]
            bv = cp.tile([128, 264], F32, name="bv", tag="bv")
            bp = cp.tile([128, 2], F32, name="bp", tag="bp")
            idn = cp.tile([128, 128], BF16, name="idn", tag="idn")
            zrow = cp.tile([1, 128], BF16, name="zrow", tag="zrow")

            nc.sync.dma_start(bp[:, 0:1], b_p[0:128, :])
            nc.sync.dma_start(bp[:, 1:2], b_p[128:256, :])
            nc.sync.dma_start(bq[0][:], b_qA[:])
            nc.sync.dma_start(bq[1][:], b_qB[:])
            nc.sync.dma_start(bk[0][:], b_kA[:])
            nc.sync.dma_start(bk[1][:], b_kB[:])
            nc.sync.dma_start(bv[:], b_v[:])
            nc.vector.memset(zrow[:], 0.0)

            rgb_sb = dp.tile([128, 2 * QCH], F32R, name="rgb_sb", tag="rgb")
            edge_sb = [dp.tile([128, N], F32R, name=f"edge{k}", tag=f"edge{k}") for k in range(2)]

            # Direct DMA into the f32r tiles (f32r bits == f32; the HW
            # matmul rounds on read). Matmuls that end up with multiple
            # DMA-lane waits are legalized by the wait-splitter post-pass.
            for k in range(2):
                nc.sync.dma_start(wq[0][:, 128 * k:128 * (k + 1)], w_qA[128 * k:128 * (k + 1), :])
                nc.sync.dma_start(wq[1][:, 128 * k:128 * (k + 1)], w_qB[128 * k:128 * (k + 1), :])
                nc.sync.dma_start(wk[0][:, 128 * k:128 * (k + 1)], w_kA[128 * k:128 * (k + 1), :])
                nc.sync.dma_start(wk[1][:, 128 * k:128 * (k + 1)], w_kB[128 * k:128 * (k + 1), :])
                nc.sync.dma_start(wv[:, 264 * k:264 * (k + 1)], w_v[128 * k:128 * (k + 1), :])
                nc.sync.dma_start(wp[:, 256 * k:256 * (k + 1)], w_p[128 * k:128 * (k + 1), :])
                nc.sync.dma_start(rgb_sb[:, QCH * k:QCH * (k + 1)], rgb_s[128 * k:128 * (k + 1), :])
                nc.sync.dma_start(edge_sb[k][:], edge[128 * k:128 * (k + 1), :])
            nc.sync.dma_start(idn[:], ident[:])

            qq = [dp.tile([128, QCH], F32R, name=f"qq{x}", tag=f"qq{x}") for x in "AB"]
            kk = [dp.tile([128, N], F32R, name=f"kk{x}", tag=f"kk{x}") for x in "AB"]
            vto = dp.tile([128, NMT * 8 * HB], BF16, name="vto", tag="vto")
            outb = [dp.tile([128, QCH], F32, name=f"outb{k}", tag=f"outb{k}") for k in range(2)]
            xxt = [dp.tile([128, 512], BF16, name=f"xxt{k}", tag=f"xxt{k}") for k in range(2)]

            # ---- pools (PSUM pools span the whole kernel: releasing a
            # PSUM pool makes successor tiles inherit released-zone deps
            # as multi-wait instructions, which walrus rejects) ----
            # PSUM budget (8 banks): scp 3 x [128,1024]f32 (2 banks each)
            # for a 3-deep QKT->exp score pipeline; avp 2 x [128,264]f32
            # (1 bank) for the per-(x,pr) AV accumulator, s-chunks packed
            # along columns. Projection / transpose / out-proj psums
            # borrow scp zones transiently.
            scp = tc.alloc_tile_pool(name="scp", bufs=3, space="PSUM")
            avp = tc.alloc_tile_pool(name="avp", bufs=2, space="PSUM")
            ep = tc.alloc_tile_pool(name="exp", bufs=10)
            sp = tc.alloc_tile_pool(name="stg", bufs=6)

            # ---- projections ----
            # qq/kk bias adds ride on ACT (Identity+bias, per-partition
            # scalar); vto's bias varies along the free dim so it stays a
            # DVE tensor_tensor add.
            for x in range(2):          # qq (A/B head groups)
                for j in range(2):
                    ps = scp.tile([128, 512], F32, name="ps_q", tag="w")
                    for k in range(2):
                        nc.tensor.matmul(
                            ps[:], wq[x][:, 128 * k:128 * (k + 1)],
                            rgb_sb[:, QCH * k + 512 * j:QCH * k + 512 * (j + 1)],
                            start=(k == 0), stop=(k == 1))
                    nc.scalar.activation(
                        qq[x][:, 512 * j:512 * (j + 1)], ps[:],
                        AF.Identity, bias=bq[x][:])
            for x in range(2):          # kk
                for j in range(8):
                    ps = scp.tile([128, 512], F32, name="ps_k", tag="w")
                    for k in range(2):
                        nc.tensor.matmul(
                            ps[:], wk[x][:, 128 * k:128 * (k + 1)],
                            edge_sb[k][:, 512 * j:512 * (j + 1)],
                            start=(k == 0), stop=(k == 1))
                    nc.scalar.activation(
                        kk[x][:, 512 * j:512 * (j + 1)], ps[:],
                        AF.Identity, bias=bk[x][:])
            # v^T: vto[m, (h: 32 v | den)] per m-tile. wv has zero
            # columns at the denominator positions and bv carries the
            # 1.0s there, so no memset / strided writes are needed.
            for mt in range(NMT):
                ps = scp.tile([128, 264], F32, name="ps_v", tag="w")
                for k in range(2):
                    nc.tensor.matmul(
                        ps[:], edge_sb[k][:, 128 * mt:128 * (mt + 1)],
                        wv[:, 264 * k:264 * (k + 1)],
                        start=(k == 0), stop=(k == 1))
                nc.vector.tensor_add(
                    vto[:, 8 * HB * mt:8 * HB * (mt + 1)], ps[:], bv[:])

            # ACT warmup: absorb the DVE tick (activation-bias const
            # tiles are DVE-written) so the first exp carries only the PE
            # wait — walrus allows a single wait per Activation.
            actw = dp.tile([1, 1], BF16, name="actw", tag="actw")
            nc.scalar.activation(actw[:], zrow[0:1, 0:1], AF.Exp)

            # Bresenham dispatcher: which engine computes exp of each
            # score tile (ACT exact exp vs DVE bitwise 2^x).
            exp_acc = [0.0]

            def emit_exp(et, sc):
                exp_acc[0] += act_frac
                if exp_acc[0] >= 1.0:
                    exp_acc[0] -= 1.0
                    nc.scalar.activation(et[:], sc[:], AF.Exp)
                else:
                    nc.vector.tensor_scalar(
                        et[:].bitcast(I16), sc[:], SCH_C1, SCH_C2,
                        ALU.mult, ALU.add)

            # ---- attention + output projection ----
            # Per (qc, x, pr) head-pair group: one 1-bank PSUM accumulator
            # avt[:, 66*s + 33*j2 + c] = xx[q=128s+p, head 4x+2pr+j2, c]
            # (c==32 is the softmax denominator). QKT->exp->AV is software-
            # pipelined with AV lagging LAG mtiles so PE never waits on the
            # exp engines; sc rotates 3 PSUM zones.
            LAG = 2
            for qc in range(2):
                q0 = 512 * qc
                for x in range(2):
                    xnm_x = sp.tile([128, 512], BF16, name="xnm", tag="xnm")
                    for pr in range(2):
                        avt = avp.tile([128, 264], F32, name="av", tag="av")
                        nc.tensor.matmul(avt[:], zrow[:], vto[0:1, 0:264],
                                         start=True, stop=False)
                        pend = []

                        def emit_av(mt, et):
                            for j2 in range(2):
                                h = 4 * x + 2 * pr + j2
                                for s in range(4):
                                    nc.tensor.matmul(
                                        avt[:, 66 * s + 33 * j2:66 * s + 33 * j2 + 33],
                                        et[:, 512 * j2 + 128 * s:512 * j2 + 128 * (s + 1)],
                                        vto[:, 8 * HB * mt + HB * h:8 * HB * mt + HB * (h + 1)],
                                        start=False, stop=(mt == NMT - 1))

                        for mt in range(NMT):
                            sc = scp.tile([128, 1024], F32, name="sc", tag="w")
                            for j2 in range(2):
                                j = 2 * pr + j2
                                nc.tensor.matmul(
                                    sc[:, 512 * j2:512 * (j2 + 1)],
                                    kk[x][32 * j:32 * j + KD, 128 * mt:128 * (mt + 1)],
                                    qq[x][32 * j:32 * j + KD, q0:q0 + 512],
                                    start=True, stop=True,
                                    tile_position=(32 * j, 0))
                            et = ep.tile([128, 1024], BF16, name="et", tag="et")
                            emit_exp(et, sc)
                            pend.append((mt, et))
                            if len(pend) > LAG:
                                emit_av(*pend.pop(0))
                        while pend:
                            emit_av(*pend.pop(0))

                        # normalize this head-pair group: relu (ACT),
                        # reciprocal of denominators (DVE), per-block
                        # scale into the shared pre-transpose tile (Pool)
                        xxm = sp.tile([128, 264], F32, name="xxm", tag="xxm")
                        nc.scalar.activation(xxm[:], avt[:], AF.Relu)
                        rden = sp.tile([128, 8], F32, name="rden", tag="rden")
                        nc.vector.reciprocal(
                            rden[:],
                            xxm[:].rearrange("p (g c) -> p g c", c=33)[:, :, 32])
                        for g in range(8):      # g = 2*s + j2
                            s, j2 = g // 2, g % 2
                            c0 = 128 * s + 64 * pr + 32 * j2
                            nc.gpsimd.tensor_scalar_mul(
                                xnm_x[:, c0:c0 + 32],
                                xxm[:, 33 * g:33 * g + 32],
                                rden[:, g:g + 1])
                    for s in range(4):
                        tp = scp.tile([128, 128], BF16, name="tp", tag="w")
                        nc.tensor.transpose(tp[:], xnm_x[:, 128 * s:128 * (s + 1)], idn[:])
                        nc.vector.tensor_copy(xxt[x][:, 128 * s:128 * (s + 1)], tp[:])
                for ct in range(2):
                    ps = scp.tile([128, 512], F32, name="ps_p", tag="w")
                    for k in range(2):
                        nc.tensor.matmul(
                            ps[:], wp[:, 256 * k + 128 * ct:256 * k + 128 * (ct + 1)],
                            xxt[k][:], start=(k == 0), stop=(k == 1))
                    nc.scalar.activation(
                        outb[ct][:, q0:q0 + 512], ps[:],
                        AF.Identity, bias=bp[:, ct:ct + 1])
            for ct in range(2):
                nc.gpsimd.dma_start(out[128 * ct:128 * (ct + 1), :], outb[ct][:])
            for _p in (sp, ep, avp, scp):
                _p.release()

    # walrus codegen accepts only ONE sync wait on compute instructions
    # (Matmult / Activation / TensorTensor / ...). The multi-wait cases
    # Tile emits here are all {self-engine, other}: a self-engine wait
    # orders an instruction against an earlier instruction on the SAME
    # in-order engine (WAW through PE's single PSUM write port, ACT/DVE
    # pipeline order), which the hardware already guarantees — drop it.
    _self_prefix = {
        "EngineType.PE": "PE",
        "EngineType.Activation": "Activation",
        "EngineType.DVE": "DVE",
        "EngineType.Pool": "Pool",
        "EngineType.SP": "SP",
    }
    for f in nc.m.functions:
        for bb in f.blocks:
            for inst in bb.instructions:
                si = inst.sync_info
                if si is None or not si.on_wait or len(si.on_wait) < 2:
                    continue
                pref = _self_prefix.get(str(getattr(inst, "engine", "")), None)
                if pref is None:
                    continue
                kept = [w for w in si.on_wait
                        if not str(w.ant_name).startswith(pref)]
                if not kept or len(kept) == len(si.on_wait):
                    continue
                si.on_wait = kept

    # Safety net: any instruction still carrying >1 wait gets all but its
    # last wait hoisted into preceding same-engine NoOps (1 wait each).
    uid = [0]
    for f in nc.m.functions:
        for bb in f.blocks:
            new_insts = []
            for inst in bb.instructions:
                si = inst.sync_info
                if si is not None and si.on_wait and len(si.on_wait) > 1:
                    for w in si.on_wait[:-1]:
                        uid[0] += 1
                        nop = mybir.InstNoOp(
                            name=f"I-waitsplit-{uid[0]}", ins=[], outs=[])
                        nop.engine = inst.engine
                        nop.sync_info = mybir.SyncInfo(
                            on_wait=[w], on_update=[])
                        new_insts.append(nop)
                    si.on_wait = [si.on_wait[-1]]
                new_insts.append(inst)
            bb.instructions = new_insts
    return nc


_CACHE = {}


def _prep_host(inputs):
    """Fold BN into weights; build head-split layouts shared by all cores."""
    import ml_dtypes
    f = np.float32
    Wq = (inputs["Wq"] * inputs["sq"][:, None]).astype(f)
    Wk = (inputs["Wk"] * inputs["sk"][:, None]).astype(f)
    Wv = (inputs["Wv"] * inputs["sv"][:, None]).astype(f)
    Wp = (inputs["Wp"] * inputs["sp"][:, None]).astype(f)

    def split(Wt, b):
        o = []
        for g in range(2):
            Wx = np.zeros((C, 128), f)
            bx = np.zeros((128, 1), f)
            for j in range(4):
                h = 4 * g + j
                Wx[:, 32 * j:32 * j + KD] = Wt[:, KD * h:KD * (h + 1)]
                bx[32 * j:32 * j + KD, 0] = b[KD * h:KD * (h + 1)]
            o.append((np.ascontiguousarray(Wx), bx))
        return o

    (wqA, bqA), (wqB, bqB) = split(Wq.T.astype(f), inputs["bq"])
    (wkA, bkA), (wkB, bkB) = split(Wk.T.astype(f), inputs["bk"])
    WvT = Wv.T.astype(f)                      # [C, 256] cols (h, d)
    wv_ext = np.zeros((C, 264), f)            # col 33h+32 stays 0
    bv_ext = np.zeros((264,), f)
    for h in range(NUM_HEADS):
        wv_ext[:, HB * h:HB * h + 32] = WvT[:, 32 * h:32 * (h + 1)]
        bv_ext[HB * h:HB * h + 32] = inputs["bv"][32 * h:32 * (h + 1)]
        bv_ext[HB * h + 32] = 1.0             # softmax denominator column
    ident_bf16 = np.eye(128, dtype=ml_dtypes.bfloat16)
    return dict(
        w_qA=wqA, w_qB=wqB, w_kA=wkA, w_kB=wkB,
        w_v=wv_ext, w_p=np.ascontiguousarray(Wp.T).astype(ml_dtypes.bfloat16),
        b_qA=bqA, b_qB=bqB, b_kA=bkA, b_kB=bkB,
        b_v=np.ascontiguousarray(np.broadcast_to(bv_ext, (128, 264))),
        b_p=inputs["bp"].astype(f).reshape(C, 1),
        ident=ident_bf16,
    )


def kernel(**inputs) -> np.ndarray:
    inputs = {k: np.asarray(v) for k, v in inputs.items()}
    if "nc" not in _CACHE:
        _CACHE["nc"] = build_nc()
    nc = _CACHE["nc"]

    shared = _prep_host(inputs)
    rgb = np.ascontiguousarray(inputs["rgb"].astype(np.float32).reshape(B, C, N))
    edge = np.ascontiguousarray(inputs["edge"].astype(np.float32).reshape(B, C, N))

    in_maps = []
    for core in range(8):
        b, qs = core // 4, core % 4
        m = dict(shared)
        m["rgb_s"] = np.ascontiguousarray(rgb[b][:, QCH * qs:QCH * (qs + 1)])
        m["edge"] = edge[b]
        in_maps.append(m)

    res = run_bass_kernel_spmd(nc, in_maps, core_ids=list(range(8)))
    full = np.zeros((B, C, N), np.float32)
    for core in range(8):
        b, qs = core // 4, core % 4
        full[b][:, QCH * qs:QCH * (qs + 1)] = res.results[core]["out"]
    return full.reshape(B, C, H, W)


# revision 13
# speedup vs baseline: 1.4363x; 1.0996x over previous
"""Cross-modal attention Trainium2 kernel.

Reference computation (all 1x1 convs + folded eval-mode BN):
  q = BN(Wq @ rgb), k = BN(Wk @ edge), v = BN(Wv @ edge)
  attn = softmax(q^T k) per head; xx = relu(attn @ v); out = BN(Wp @ xx)

Shapes: B=2, C=256, H=W=64 (N=4096), heads=8, key_dim=16, d=32.

Sharding: 8 cores = (batch b in {0,1}) x (query-slice qs in {0..3}, 1024
queries each). Each core computes K/V projections for the full N of its
batch (cheap) and attention + output projection for its query slice; the
host concatenates slices. No collectives.

Per-core dataflow (scores kept transposed so softmax-sum and the AV
contraction both run on the m axis without transposing the big matrix):
  sT[m, q] = sum_kd kk[kd, m] qq[kd, q]     (PE, fp32r, 32x128 row-tiled)
  e = exp(sT)        split across two engines, balanced ~53/47:
      ACT:  exp activation (PSUM->SBUF bf16)
      DVE:  Schraudolph bitwise exp: bf16_bits(int16(s*184.665 + 16248.6))
            = 2^(s*log2 e) with ~+-3% multiplicative error that cancels in
            softmax normalization (max |s| ~= 45, safe window (-88, +89)).
  av[q, (h: d|den)] += e[mtile]^T @ [v|1]   (PE bf16, exp as stationary)
  xx = relu(av), xnm = xx * recip(den)      (ACT relu; Pool per-head scale)
  out = Wp^T @ xxt + bp                     (PE bf16 transpose, DMA PSUM->SBUF
                                             move, matmul, ACT bias add)

Engine budget per core (TimelineSim): PE ~163us (QK^T 109 + AV 30 + proj/
misc 24), ACT ~158us (exp share + bias adds + relu), DVE ~158us (exp share
+ v moves), Pool/SP/DMA light. Baseline (all exp on ACT) was 315us.
"""

import sys

for p in ("/opt/trn_rl_repo", "/opt/trn_rl_repo/concourse"):
    if p not in sys.path:
        sys.path.insert(0, p)

import numpy as np

import concourse.bass as bass
import concourse.mybir as mybir
import concourse.tile as tile
from concourse.bass_utils import run_bass_kernel_spmd

F32 = mybir.dt.float32
F32R = mybir.dt.float32r
BF16 = mybir.dt.bfloat16
I16 = mybir.dt.int16
AF = mybir.ActivationFunctionType
ALU = mybir.AluOpType

NUM_HEADS, KD, D = 8, 16, 32
B, C, H, W = 2, 256, 64, 64
N = H * W            # 4096 keys per batch
QCH = 1024           # queries per core
NMT = N // 128       # 32 m-tiles
HB = 33              # per-head AV block: 32 v-cols + 1 denominator col

# Schraudolph exp: bf16_bits(int16(s*SCH_C1 + SCH_C2)) ~= e^s.
SCH_C1 = 184.66496580927726     # 128 * log2(e)
SCH_C2 = 16248.6                # 127*128 minus mean-zeroing interp bias
ACT_FRAC = 0.5266               # share of exp tiles on ACT (rest on DVE)


def build_nc(act_frac=ACT_FRAC):
    nc = bass.Bass()

    rgb_s = nc.dram_tensor("rgb_s", [C, QCH], F32R, kind="ExternalInput")
    edge = nc.dram_tensor("edge", [C, N], F32R, kind="ExternalInput")
    w_qA = nc.dram_tensor("w_qA", [C, 128], F32R, kind="ExternalInput")
    w_qB = nc.dram_tensor("w_qB", [C, 128], F32R, kind="ExternalInput")
    w_kA = nc.dram_tensor("w_kA", [C, 128], F32R, kind="ExternalInput")
    w_kB = nc.dram_tensor("w_kB", [C, 128], F32R, kind="ExternalInput")
    w_v = nc.dram_tensor("w_v", [C, 264], F32R, kind="ExternalInput")
    w_p = nc.dram_tensor("w_p", [256, C], BF16, kind="ExternalInput")
    b_qA = nc.dram_tensor("b_qA", [128, 1], F32, kind="ExternalInput")
    b_qB = nc.dram_tensor("b_qB", [128, 1], F32, kind="ExternalInput")
    b_kA = nc.dram_tensor("b_kA", [128, 1], F32, kind="ExternalInput")
    b_kB = nc.dram_tensor("b_kB", [128, 1], F32, kind="ExternalInput")
    b_v = nc.dram_tensor("b_v", [128, 264], F32, kind="ExternalInput")
    b_p = nc.dram_tensor("b_p", [C, 1], F32, kind="ExternalInput")
    ident = nc.dram_tensor("ident", [128, 128], BF16, kind="ExternalInput")
    out = nc.dram_tensor("out", [C, QCH], F32, kind="ExternalOutput")

    with tile.TileContext(nc) as tc:
        with tc.tile_pool(name="const", bufs=1) as cp, \
             tc.tile_pool(name="data", bufs=1) as dp:
            wq = [cp.tile([128, 256], F32R, name=f"wq{x}", tag=f"wq{x}") for x in "AB"]
            wk = [cp.tile([128, 256], F32R, name=f"wk{x}", tag=f"wk{x}") for x in "AB"]
            wv = cp.tile([128, 528], F32R, name="wv", tag="wv")
            wp = cp.tile([128, 512], BF16, name="wp", tag="wp")
            bq = [cp.tile([128, 1], F32, name=f"bq{x}", tag=f"bq{x}") for x in "AB"]
            bk = [cp.tile([128, 1], F32, name=f"bk{x}", tag=f"bk{x}") for x in "AB"]
            bv = cp.tile([128, 264], F32, name="bv", tag="bv")
            bp = cp.tile([128, 2], F32, name="bp", tag="bp")
            idn = cp.tile([128, 128], BF16, name="idn", tag="idn")
            zrow = cp.tile([1, 128], BF16, name="zrow", tag="zrow")

            nc.sync.dma_start(bk[0][:], b_kA[:])
            nc.sync.dma_start(bq[0][:], b_qA[:])
            nc.vector.memset(zrow[:], 0.0)

            rgb_sb = dp.tile([128, 2 * QCH], F32R, name="rgb_sb", tag="rgb")
            edge_sb = [dp.tile([128, N], F32R, name=f"edge{k}", tag=f"edge{k}") for k in range(2)]

            # Direct DMA into the f32r tiles (f32r bits == f32; the HW
            # matmul rounds on read). Inputs are chunked and ordered so the
            # first QKT can start ~6us in: kk[0]-chunk-0 + qq[0]-qc0 deps
            # first, then pieces in first-use order. The projections
            # themselves are interleaved into the first attention groups.
            for k in range(2):
                nc.sync.dma_start(wk[0][:, 128 * k:128 * (k + 1)], w_kA[128 * k:128 * (k + 1), :])
                nc.sync.dma_start(wq[0][:, 128 * k:128 * (k + 1)], w_qA[128 * k:128 * (k + 1), :])
            for k in range(2):      # edge piece 0 (cols 0..1024) both halves
                nc.sync.dma_start(edge_sb[k][:, 0:1024], edge[128 * k:128 * (k + 1), 0:1024])
            for k in range(2):      # rgb first halves (qc=0 queries)
                nc.sync.dma_start(rgb_sb[:, QCH * k:QCH * k + 512], rgb_s[128 * k:128 * (k + 1), 0:512])
            for k in range(2):
                nc.sync.dma_start(wv[:, 264 * k:264 * (k + 1)], w_v[128 * k:128 * (k + 1), :])
            nc.sync.dma_start(bv[:], b_v[:])
            nc.sync.dma_start(bq[1][:], b_qB[:])
            nc.sync.dma_start(bk[1][:], b_kB[:])
            nc.sync.dma_start(bp[:, 0:1], b_p[0:128, :])
            nc.sync.dma_start(bp[:, 1:2], b_p[128:256, :])
            for k in range(2):
                nc.sync.dma_start(edge_sb[k][:, 1024:2048], edge[128 * k:128 * (k + 1), 1024:2048])
            for k in range(2):
                nc.sync.dma_start(rgb_sb[:, QCH * k + 512:QCH * (k + 1)], rgb_s[128 * k:128 * (k + 1), 512:1024])
            for k in range(2):
                nc.sync.dma_start(wk[1][:, 128 * k:128 * (k + 1)], w_kB[128 * k:128 * (k + 1), :])
                nc.sync.dma_start(wq[1][:, 128 * k:128 * (k + 1)], w_qB[128 * k:128 * (k + 1), :])
            for p in range(2, 4):
                for k in range(2):
                    nc.sync.dma_start(edge_sb[k][:, 1024 * p:1024 * (p + 1)],
                                      edge[128 * k:128 * (k + 1), 1024 * p:1024 * (p + 1)])
            for k in range(2):
                nc.sync.dma_start(wp[:, 256 * k:256 * (k + 1)], w_p[128 * k:128 * (k + 1), :])
            nc.sync.dma_start(idn[:], ident[:])

            qq = [dp.tile([128, QCH], F32R, name=f"qq{x}", tag=f"qq{x}") for x in "AB"]
            kk = [dp.tile([128, N], F32R, name=f"kk{x}", tag=f"kk{x}") for x in "AB"]
            vto = dp.tile([128, NMT * 8 * HB], BF16, name="vto", tag="vto")
            outb = [dp.tile([128, QCH], F32, name=f"outb{k}", tag=f"outb{k}") for k in range(2)]
            xxt = [dp.tile([128, 512], BF16, name=f"xxt{k}", tag=f"xxt{k}") for k in range(2)]

            # ---- pools (PSUM pools span the whole kernel: releasing a
            # PSUM pool makes successor tiles inherit released-zone deps
            # as multi-wait instructions, which walrus rejects) ----
            # PSUM budget (8 banks): scp 3 x [128,1024]f32 (2 banks each)
            # for a 3-deep QKT->exp score pipeline; avp 2 x [128,264]f32
            # (1 bank) for the per-(x,pr) AV accumulator, s-chunks packed
            # along columns. Projection / transpose / out-proj psums
            # borrow scp zones transiently.
            scp = tc.alloc_tile_pool(name="scp", bufs=3, space="PSUM")
            avp = tc.alloc_tile_pool(name="avp", bufs=2, space="PSUM")
            ep = tc.alloc_tile_pool(name="exp", bufs=10)
            sp = tc.alloc_tile_pool(name="stg", bufs=6)

            # ---- fused projection + attention stream ----
            # All work is emitted as ONE software-pipelined mtile stream
            # across the 8 (qc, x, pr) head-pair groups. Projections ride
            # as per-mtile hooks inside the first two groups (hidden under
            # the attention steady state); AV consumption lags QKT/exp by
            # LAG mtiles (across group boundaries too) so PE never waits
            # on the exp engines; per-group epilogues (relu/recip/scale)
            # fire when the group's last AV flushes, and their PE parts
            # (transposes, output projection) are deferred hooks inside
            # later groups' streams.

            # ACT warmup: absorb the DVE tick (activation-bias const
            # tiles are DVE-written) so the first exp carries only the PE
            # wait — walrus allows a single wait per Activation.
            actw = dp.tile([1, 1], BF16, name="actw", tag="actw")
            nc.scalar.activation(actw[:], zrow[0:1, 0:1], AF.Exp)

            def proj_qq(x, j):
                ps = scp.tile([128, 512], F32, name="ps_q", tag="w")
                for k in range(2):
                    nc.tensor.matmul(
                        ps[:], wq[x][:, 128 * k:128 * (k + 1)],
                        rgb_sb[:, QCH * k + 512 * j:QCH * k + 512 * (j + 1)],
                        start=(k == 0), stop=(k == 1))
                nc.scalar.activation(
                    qq[x][:, 512 * j:512 * (j + 1)], ps[:],
                    AF.Identity, bias=bq[x][:])

            def proj_kk(x, j):
                ps = scp.tile([128, 512], F32, name="ps_k", tag="w")
                for k in range(2):
                    nc.tensor.matmul(
                        ps[:], wk[x][:, 128 * k:128 * (k + 1)],
                        edge_sb[k][:, 512 * j:512 * (j + 1)],
                        start=(k == 0), stop=(k == 1))
                nc.scalar.activation(
                    kk[x][:, 512 * j:512 * (j + 1)], ps[:],
                    AF.Identity, bias=bk[x][:])

            def proj_vto(mt):
                # v^T: vto[m, (h: 32 v | den)]. wv has zero columns at the
                # denominator positions and bv carries the 1.0s there.
                ps = scp.tile([128, 264], F32, name="ps_v", tag="w")
                for k in range(2):
                    nc.tensor.matmul(
                        ps[:], edge_sb[k][:, 128 * mt:128 * (mt + 1)],
                        wv[:, 264 * k:264 * (k + 1)],
                        start=(k == 0), stop=(k == 1))
                nc.vector.tensor_add(
                    vto[:, 8 * HB * mt:8 * HB * (mt + 1)], ps[:], bv[:])

            def transposes(x):
                xnm_x = xnm_tiles[x]
                for s in range(4):
                    tp = scp.tile([128, 128], BF16, name="tp", tag="w")
                    nc.tensor.transpose(tp[:], xnm_x[:, 128 * s:128 * (s + 1)], idn[:])
                    nc.vector.tensor_copy(xxt[x][:, 128 * s:128 * (s + 1)], tp[:])

            def outproj(qc):
                q0 = 512 * qc
                for ct in range(2):
                    ps = scp.tile([128, 512], F32, name="ps_p", tag="w")
                    for k in range(2):
                        nc.tensor.matmul(
                            ps[:], wp[:, 256 * k + 128 * ct:256 * k + 128 * (ct + 1)],
                            xxt[k][:], start=(k == 0), stop=(k == 1))
                    nc.scalar.activation(
                        outb[ct][:, q0:q0 + 512], ps[:],
                        AF.Identity, bias=bp[:, ct:ct + 1])

            # per-mtile PE-stream hooks: {(gi, mt): [fn, ...]}
            hooks = {}

            def add_hook(gi, mt, fn):
                hooks.setdefault((gi, mt), []).append(fn)

            add_hook(0, 0, lambda: proj_kk(0, 0))
            add_hook(0, 0, lambda: proj_qq(0, 0))
            for mt in range(NMT):           # vto(mt) feeds AV(mt), LAG behind
                add_hook(0, mt, lambda mt=mt: proj_vto(mt))
            for c in range(1, 8):           # kk[0] chunk c feeds QKT(mt>=4c)
                add_hook(0, 4 * c - 2, lambda c=c: proj_kk(0, c))
            add_hook(0, 18, lambda: proj_qq(1, 0))
            add_hook(0, 22, lambda: proj_qq(0, 1))
            add_hook(0, 26, lambda: proj_qq(1, 1))
            for c in range(8):              # kk[1] feeds the x=1 groups
                add_hook(1, 3 * c, lambda c=c: proj_kk(1, c))

            # Bresenham dispatcher: which engine computes exp of each
            # score tile (ACT exact exp vs DVE bitwise 2^x). The first two
            # groups push extra share to ACT since DVE carries the vto
            # bias-adds there.
            exp_acc = [0.0]
            cur_frac = [act_frac]

            def emit_exp(et, sc):
                exp_acc[0] += cur_frac[0]
                if exp_acc[0] >= 1.0:
                    exp_acc[0] -= 1.0
                    nc.scalar.activation(et[:], sc[:], AF.Exp)
                else:
                    nc.vector.tensor_scalar(
                        et[:].bitcast(I16), sc[:], SCH_C1, SCH_C2,
                        ALU.mult, ALU.add)

            GROUPS = [(qc, x, pr) for qc in (0, 1) for x in (0, 1) for pr in (0, 1)]
            # deferred PE-side epilogue hooks: group gi's xnm tile (x done
            # at odd gi) is transposed inside group gi+1's stream; the
            # output projection of qc=0 rides in group 4 (qc=1 x=0 pr=0).
            add_hook(2, 10, lambda: transposes(0))
            add_hook(4, 10, lambda: transposes(1))
            add_hook(4, 16, lambda: outproj(0))
            add_hook(6, 10, lambda: transposes(0))
            GROUP_FRACS = [0.62, 0.55] + [act_frac] * 6
            LAG = 2
            xnm_tiles = [None, None]
            pend = []       # (emit_av_fn, post_fn_or_None)

            def flush_one():
                fn, post = pend.pop(0)
                fn()
                if post is not None:
                    post()

            for gi, (qc, x, pr) in enumerate(GROUPS):
                q0 = 512 * qc
                cur_frac[0] = GROUP_FRACS[gi]
                avt = avp.tile([128, 264], F32, name="av", tag="av")
                # one whole-bank start=True clear; the AV matmuls use
                # start=False (first per-element write overwrites, later
                # ones accumulate)
                nc.tensor.matmul(avt[:], zrow[:], vto[0:1, 0:264],
                                 start=True, stop=False)

                def make_av(avt, x, pr, mt, et):
                    def emit_av():
                        for j2 in range(2):
                            h = 4 * x + 2 * pr + j2
                            for s in range(4):
                                nc.tensor.matmul(
                                    avt[:, 66 * s + 33 * j2:66 * s + 33 * j2 + 33],
                                    et[:, 512 * j2 + 128 * s:512 * j2 + 128 * (s + 1)],
                                    vto[:, 8 * HB * mt + HB * h:8 * HB * mt + HB * (h + 1)],
                                    start=False, stop=(mt == NMT - 1))
                    return emit_av

                def make_epilogue(avt, gi, qc, x, pr):
                    def epilogue():
                        # relu (ACT, PSUM->SBUF), reciprocal of the
                        # denominators (DVE), per-block scale into the
                        # shared pre-transpose tile (Pool).
                        xxm = sp.tile([128, 264], F32, name="xxm", tag="xxm")
                        nc.scalar.activation(xxm[:], avt[:], AF.Relu)
                        rden = sp.tile([128, 8], F32, name="rden", tag="rden")
                        nc.vector.reciprocal(
                            rden[:],
                            xxm[:].rearrange("p (g c) -> p g c", c=33)[:, :, 32])
                        if pr == 0:
                            xnm_tiles[x] = sp.tile([128, 512], BF16,
                                                   name="xnm", tag="xnm")
                        xnm_x = xnm_tiles[x]
                        for g in range(8):      # g = 2*s + j2
                            s, j2 = g // 2, g % 2
                            c0 = 128 * s + 64 * pr + 32 * j2
                            nc.gpsimd.tensor_scalar_mul(
                                xnm_x[:, c0:c0 + 32],
                                xxm[:, 33 * g:33 * g + 32],
                                rden[:, g:g + 1])
                    return epilogue

                for mt in range(NMT):
                    for fn in hooks.get((gi, mt), []):
                        fn()
                    while len(pend) > LAG:
                        flush_one()
                    sc = scp.tile([128, 1024], F32, name="sc", tag="w")
                    for j2 in range(2):
                        j = 2 * pr + j2
                        nc.tensor.matmul(
                            sc[:, 512 * j2:512 * (j2 + 1)],
                            kk[x][32 * j:32 * j + KD, 128 * mt:128 * (mt + 1)],
                            qq[x][32 * j:32 * j + KD, q0:q0 + 512],
                            start=True, stop=True,
                            tile_position=(32 * j, 0))
                    et = ep.tile([128, 1024], BF16, name="et", tag="et")
                    emit_exp(et, sc)
                    post = make_epilogue(avt, gi, qc, x, pr) if mt == NMT - 1 else None
                    pend.append((make_av(avt, x, pr, mt, et), post))

            # tail: flush remaining AVs (fires the last epilogue), then
            # the final transposes + output projection
            while pend:
                flush_one()
            transposes(1)
            outproj(1)

            for ct in range(2):
                nc.gpsimd.dma_start(out[128 * ct:128 * (ct + 1), :], outb[ct][:])
            for _p in (sp, ep, avp, scp):
                _p.release()

    # walrus codegen accepts only ONE sync wait on compute instructions
    # (Matmult / Activation / TensorTensor / ...). The multi-wait cases
    # Tile emits here are all {self-engine, other}: a self-engine wait
    # orders an instruction against an earlier instruction on the SAME
    # in-order engine (WAW through PE's single PSUM write port, ACT/DVE
    # pipeline order), which the hardware already guarantees — drop it.
    _self_prefix = {
        "EngineType.PE": "PE",
        "EngineType.Activation": "Activation",
        "EngineType.DVE": "DVE",
        "EngineType.Pool": "Pool",
        "EngineType.SP": "SP",
    }
    for f in nc.m.functions:
        for bb in f.blocks:
            for inst in bb.instructions:
                si = inst.sync_info
                if si is None or not si.on_wait or len(si.on_wait) < 2:
                    continue
                pref = _self_prefix.get(str(getattr(inst, "engine", "")), None)
                if pref is None:
                    continue
                kept = [w for w in si.on_wait
                        if not str(w.ant_name).startswith(pref)]
                if not kept or len(kept) == len(si.on_wait):
                    continue
                si.on_wait = kept

    # Safety net: any instruction still carrying >1 wait gets all but its
    # last wait hoisted into preceding same-engine NoOps (1 wait each).
    uid = [0]
    for f in nc.m.functions:
        for bb in f.blocks:
            new_insts = []
            for inst in bb.instructions:
                si = inst.sync_info
                if si is not None and si.on_wait and len(si.on_wait) > 1:
                    for w in si.on_wait[:-1]:
                        uid[0] += 1
                        nop = mybir.InstNoOp(
                            name=f"I-waitsplit-{uid[0]}", ins=[], outs=[])
                        nop.engine = inst.engine
                        nop.sync_info = mybir.SyncInfo(
                            on_wait=[w], on_update=[])
                        new_insts.append(nop)
                    si.on_wait = [si.on_wait[-1]]
                new_insts.append(inst)
            bb.instructions = new_insts
    return nc


_CACHE = {}


def _prep_host(inputs):
    """Fold BN into weights; build head-split layouts shared by all cores."""
    import ml_dtypes
    f = np.float32
    Wq = (inputs["Wq"] * inputs["sq"][:, None]).astype(f)
    Wk = (inputs["Wk"] * inputs["sk"][:, None]).astype(f)
    Wv = (inputs["Wv"] * inputs["sv"][:, None]).astype(f)
    Wp = (inputs["Wp"] * inputs["sp"][:, None]).astype(f)

    def split(Wt, b):
        o = []
        for g in range(2):
            Wx = np.zeros((C, 128), f)
            bx = np.zeros((128, 1), f)
            for j in range(4):
                h = 4 * g + j
                Wx[:, 32 * j:32 * j + KD] = Wt[:, KD * h:KD * (h + 1)]
                bx[32 * j:32 * j + KD, 0] = b[KD * h:KD * (h + 1)]
            o.append((np.ascontiguousarray(Wx), bx))
        return o

    (wqA, bqA), (wqB, bqB) = split(Wq.T.astype(f), inputs["bq"])
    (wkA, bkA), (wkB, bkB) = split(Wk.T.astype(f), inputs["bk"])
    WvT = Wv.T.astype(f)                      # [C, 256] cols (h, d)
    wv_ext = np.zeros((C, 264), f)            # col 33h+32 stays 0
    bv_ext = np.zeros((264,), f)
    for h in range(NUM_HEADS):
        wv_ext[:, HB * h:HB * h + 32] = WvT[:, 32 * h:32 * (h + 1)]
        bv_ext[HB * h:HB * h + 32] = inputs["bv"][32 * h:32 * (h + 1)]
        bv_ext[HB * h + 32] = 1.0             # softmax denominator column
    ident_bf16 = np.eye(128, dtype=ml_dtypes.bfloat16)
    return dict(
        w_qA=wqA, w_qB=wqB, w_kA=wkA, w_kB=wkB,
        w_v=wv_ext, w_p=np.ascontiguousarray(Wp.T).astype(ml_dtypes.bfloat16),
        b_qA=bqA, b_qB=bqB, b_kA=bkA, b_kB=bkB,
        b_v=np.ascontiguousarray(np.broadcast_to(bv_ext, (128, 264))),
        b_p=inputs["bp"].astype(f).reshape(C, 1),
        ident=ident_bf16,
    )


def kernel(**inputs) -> np.ndarray:
    inputs = {k: np.asarray(v) for k, v in inputs.items()}
    if "nc" not in _CACHE:
        _CACHE["nc"] = build_nc()
    nc = _CACHE["nc"]

    shared = _prep_host(inputs)
    rgb = np.ascontiguousarray(inputs["rgb"].astype(np.float32).reshape(B, C, N))
    edge = np.ascontiguousarray(inputs["edge"].astype(np.float32).reshape(B, C, N))

    in_maps = []
    for core in range(8):
        b, qs = core // 4, core % 4
        m = dict(shared)
        m["rgb_s"] = np.ascontiguousarray(rgb[b][:, QCH * qs:QCH * (qs + 1)])
        m["edge"] = edge[b]
        in_maps.append(m)

    res = run_bass_kernel_spmd(nc, in_maps, core_ids=list(range(8)))
    full = np.zeros((B, C, N), np.float32)
    for core in range(8):
        b, qs = core // 4, core % 4
        full[b][:, QCH * qs:QCH * (qs + 1)] = res.results[core]["out"]
    return full.reshape(B, C, H, W)


# revision 14
# speedup vs baseline: 1.4486x; 1.0086x over previous
"""Cross-modal attention Trainium2 kernel.

Reference computation (all 1x1 convs + folded eval-mode BN):
  q = BN(Wq @ rgb), k = BN(Wk @ edge), v = BN(Wv @ edge)
  attn = softmax(q^T k) per head; xx = relu(attn @ v); out = BN(Wp @ xx)

Shapes: B=2, C=256, H=W=64 (N=4096), heads=8, key_dim=16, d=32.

Sharding: 8 cores = (batch b in {0,1}) x (query-slice qs in {0..3}, 1024
queries each). Each core computes K/V projections for the full N of its
batch (cheap) and attention + output projection for its query slice; the
host concatenates slices. No collectives.

Per-core dataflow (scores kept transposed so softmax-sum and the AV
contraction both run on the m axis without transposing the big matrix):
  sT[m, q] = sum_kd kk[kd, m] qq[kd, q]     (PE, fp32r, 32x128 row-tiled)
  e = exp(sT)        split across two engines, balanced ~53/47:
      ACT:  exp activation (PSUM->SBUF bf16)
      DVE:  Schraudolph bitwise exp: bf16_bits(int16(s*184.665 + 16248.6))
            = 2^(s*log2 e) with ~+-3% multiplicative error that cancels in
            softmax normalization (max |s| ~= 45, safe window (-88, +89)).
  av[q, (h: d|den)] += e[mtile]^T @ [v|1]   (PE bf16, exp as stationary)
  xx = relu(av), xnm = xx * recip(den)      (ACT relu; Pool per-head scale)
  out = Wp^T @ xxt + bp                     (PE bf16 transpose, DMA PSUM->SBUF
                                             move, matmul, ACT bias add)

Engine budget per core (TimelineSim): PE ~163us (QK^T 109 + AV 30 + proj/
misc 24), ACT ~158us (exp share + bias adds + relu), DVE ~158us (exp share
+ v moves), Pool/SP/DMA light. Baseline (all exp on ACT) was 315us.
"""

import sys

for p in ("/opt/trn_rl_repo", "/opt/trn_rl_repo/concourse"):
    if p not in sys.path:
        sys.path.insert(0, p)

import numpy as np

import concourse.bass as bass
import concourse.mybir as mybir
import concourse.tile as tile
from concourse.bass_utils import run_bass_kernel_spmd

F32 = mybir.dt.float32
F32R = mybir.dt.float32r
BF16 = mybir.dt.bfloat16
I16 = mybir.dt.int16
AF = mybir.ActivationFunctionType
ALU = mybir.AluOpType

NUM_HEADS, KD, D = 8, 16, 32
B, C, H, W = 2, 256, 64, 64
N = H * W            # 4096 keys per batch
QCH = 1024           # queries per core
NMT = N // 128       # 32 m-tiles
HB = 33              # per-head AV block: 32 v-cols + 1 denominator col

# Schraudolph exp: bf16_bits(int16(s*SCH_C1 + SCH_C2)) ~= e^s.
SCH_C1 = 184.66496580927726     # 128 * log2(e)
SCH_C2 = 16248.6                # 127*128 minus mean-zeroing interp bias
ACT_FRAC = 0.5266               # share of exp tiles on ACT (rest on DVE)


def build_nc(act_frac=ACT_FRAC):
    nc = bass.Bass()

    rgb_s = nc.dram_tensor("rgb_s", [C, QCH], F32R, kind="ExternalInput")
    edge = nc.dram_tensor("edge", [C, N], F32R, kind="ExternalInput")
    w_qA = nc.dram_tensor("w_qA", [C, 128], F32R, kind="ExternalInput")
    w_qB = nc.dram_tensor("w_qB", [C, 128], F32R, kind="ExternalInput")
    w_kA = nc.dram_tensor("w_kA", [C, 128], F32R, kind="ExternalInput")
    w_kB = nc.dram_tensor("w_kB", [C, 128], F32R, kind="ExternalInput")
    w_v = nc.dram_tensor("w_v", [C, 264], F32R, kind="ExternalInput")
    w_p = nc.dram_tensor("w_p", [256, C], BF16, kind="ExternalInput")
    b_qA = nc.dram_tensor("b_qA", [128, 1], F32, kind="ExternalInput")
    b_qB = nc.dram_tensor("b_qB", [128, 1], F32, kind="ExternalInput")
    b_kA = nc.dram_tensor("b_kA", [128, 1], F32, kind="ExternalInput")
    b_kB = nc.dram_tensor("b_kB", [128, 1], F32, kind="ExternalInput")
    b_v = nc.dram_tensor("b_v", [128, 264], F32, kind="ExternalInput")
    b_p = nc.dram_tensor("b_p", [C, 1], F32, kind="ExternalInput")
    ident = nc.dram_tensor("ident", [128, 128], BF16, kind="ExternalInput")
    out = nc.dram_tensor("out", [C, QCH], F32, kind="ExternalOutput")

    with tile.TileContext(nc) as tc:
        with tc.tile_pool(name="const", bufs=1) as cp, \
             tc.tile_pool(name="data", bufs=1) as dp:
            wq = [cp.tile([128, 256], F32R, name=f"wq{x}", tag=f"wq{x}") for x in "AB"]
            wk = [cp.tile([128, 256], F32R, name=f"wk{x}", tag=f"wk{x}") for x in "AB"]
            wv = cp.tile([128, 528], F32R, name="wv", tag="wv")
            wp = cp.tile([128, 512], BF16, name="wp", tag="wp")
            bq = [cp.tile([128, 1], F32, name=f"bq{x}", tag=f"bq{x}") for x in "AB"]
            bk = [cp.tile([128, 1], F32, name=f"bk{x}", tag=f"bk{x}") for x in "AB"]
            bv = cp.tile([128, 264], F32, name="bv", tag="bv")
            bp = cp.tile([128, 2], F32, name="bp", tag="bp")
            idn = cp.tile([128, 128], BF16, name="idn", tag="idn")
            zrow = cp.tile([1, 128], BF16, name="zrow", tag="zrow")

            nc.sync.dma_start(bk[0][:], b_kA[:])
            nc.sync.dma_start(bq[0][:], b_qA[:])
            nc.vector.memset(zrow[:], 0.0)

            rgb_sb = dp.tile([128, 2 * QCH], F32R, name="rgb_sb", tag="rgb")
            edge_sb = [dp.tile([128, N], F32R, name=f"edge{k}", tag=f"edge{k}") for k in range(2)]

            # Direct DMA into the f32r tiles (f32r bits == f32; the HW
            # matmul rounds on read). Inputs are chunked and ordered so the
            # first QKT can start ~6us in: kk[0]-chunk-0 + qq[0]-qc0 deps
            # first, then pieces in first-use order. The projections
            # themselves are interleaved into the first attention groups.
            for k in range(2):
                nc.sync.dma_start(wk[0][:, 128 * k:128 * (k + 1)], w_kA[128 * k:128 * (k + 1), :])
                nc.sync.dma_start(wq[0][:, 128 * k:128 * (k + 1)], w_qA[128 * k:128 * (k + 1), :])
            for k in range(2):      # first edge columns (kk chunk 0 dep)
                nc.sync.dma_start(edge_sb[k][:, 0:512], edge[128 * k:128 * (k + 1), 0:512])
            for k in range(2):      # rgb first halves (qc=0 queries)
                nc.sync.dma_start(rgb_sb[:, QCH * k:QCH * k + 512], rgb_s[128 * k:128 * (k + 1), 0:512])
            for k in range(2):
                nc.sync.dma_start(edge_sb[k][:, 512:1024], edge[128 * k:128 * (k + 1), 512:1024])
            for k in range(2):
                nc.sync.dma_start(wv[:, 264 * k:264 * (k + 1)], w_v[128 * k:128 * (k + 1), :])
            nc.sync.dma_start(bv[:], b_v[:])
            nc.sync.dma_start(bq[1][:], b_qB[:])
            nc.sync.dma_start(bk[1][:], b_kB[:])
            nc.sync.dma_start(bp[:, 0:1], b_p[0:128, :])
            nc.sync.dma_start(bp[:, 1:2], b_p[128:256, :])
            for k in range(2):
                nc.sync.dma_start(edge_sb[k][:, 1024:2048], edge[128 * k:128 * (k + 1), 1024:2048])
            for k in range(2):
                nc.sync.dma_start(rgb_sb[:, QCH * k + 512:QCH * (k + 1)], rgb_s[128 * k:128 * (k + 1), 512:1024])
            for k in range(2):
                nc.sync.dma_start(wk[1][:, 128 * k:128 * (k + 1)], w_kB[128 * k:128 * (k + 1), :])
                nc.sync.dma_start(wq[1][:, 128 * k:128 * (k + 1)], w_qB[128 * k:128 * (k + 1), :])
            for p in range(2, 4):
                for k in range(2):
                    nc.sync.dma_start(edge_sb[k][:, 1024 * p:1024 * (p + 1)],
                                      edge[128 * k:128 * (k + 1), 1024 * p:1024 * (p + 1)])
            for k in range(2):
                nc.sync.dma_start(wp[:, 256 * k:256 * (k + 1)], w_p[128 * k:128 * (k + 1), :])
            nc.sync.dma_start(idn[:], ident[:])

            qq = [dp.tile([128, QCH], F32R, name=f"qq{x}", tag=f"qq{x}") for x in "AB"]
            kk = [dp.tile([128, N], F32R, name=f"kk{x}", tag=f"kk{x}") for x in "AB"]
            vto = dp.tile([128, NMT * 8 * HB], BF16, name="vto", tag="vto")
            outb = [dp.tile([128, QCH], F32, name=f"outb{k}", tag=f"outb{k}") for k in range(2)]
            xxt = [dp.tile([128, 512], BF16, name=f"xxt{k}", tag=f"xxt{k}") for k in range(2)]

            # ---- pools (PSUM pools span the whole kernel: releasing a
            # PSUM pool makes successor tiles inherit released-zone deps
            # as multi-wait instructions, which walrus rejects) ----
            # PSUM budget (8 banks): scp 3 x [128,1024]f32 (2 banks each)
            # for a 3-deep QKT->exp score pipeline; avp 2 x [128,264]f32
            # (1 bank) for the per-(x,pr) AV accumulator, s-chunks packed
            # along columns. Projection / transpose / out-proj psums
            # borrow scp zones transiently.
            scp = tc.alloc_tile_pool(name="scp", bufs=3, space="PSUM")
            avp = tc.alloc_tile_pool(name="avp", bufs=2, space="PSUM")
            ep = tc.alloc_tile_pool(name="exp", bufs=10)
            sp = tc.alloc_tile_pool(name="stg", bufs=6)

            # ---- fused projection + attention stream ----
            # All work is emitted as ONE software-pipelined mtile stream
            # across the 8 (qc, x, pr) head-pair groups. Projections ride
            # as per-mtile hooks inside the first two groups (hidden under
            # the attention steady state); AV consumption lags QKT/exp by
            # LAG mtiles (across group boundaries too) so PE never waits
            # on the exp engines; per-group epilogues (relu/recip/scale)
            # fire when the group's last AV flushes, and their PE parts
            # (transposes, output projection) are deferred hooks inside
            # later groups' streams.

            # ACT warmup: absorb the DVE tick (activation-bias const
            # tiles are DVE-written) so the first exp carries only the PE
            # wait — walrus allows a single wait per Activation.
            actw = dp.tile([1, 1], BF16, name="actw", tag="actw")
            nc.scalar.activation(actw[:], zrow[0:1, 0:1], AF.Exp)

            def proj_qq(x, j):
                ps = scp.tile([128, 512], F32, name="ps_q", tag="w")
                for k in range(2):
                    nc.tensor.matmul(
                        ps[:], wq[x][:, 128 * k:128 * (k + 1)],
                        rgb_sb[:, QCH * k + 512 * j:QCH * k + 512 * (j + 1)],
                        start=(k == 0), stop=(k == 1))
                nc.scalar.activation(
                    qq[x][:, 512 * j:512 * (j + 1)], ps[:],
                    AF.Identity, bias=bq[x][:])

            def proj_kk(x, j):
                ps = scp.tile([128, 512], F32, name="ps_k", tag="w")
                for k in range(2):
                    nc.tensor.matmul(
                        ps[:], wk[x][:, 128 * k:128 * (k + 1)],
                        edge_sb[k][:, 512 * j:512 * (j + 1)],
                        start=(k == 0), stop=(k == 1))
                nc.scalar.activation(
                    kk[x][:, 512 * j:512 * (j + 1)], ps[:],
                    AF.Identity, bias=bk[x][:])

            def proj_vto(mt):
                # v^T: vto[m, (h: 32 v | den)]. wv has zero columns at the
                # denominator positions and bv carries the 1.0s there.
                ps = scp.tile([128, 264], F32, name="ps_v", tag="w")
                for k in range(2):
                    nc.tensor.matmul(
                        ps[:], edge_sb[k][:, 128 * mt:128 * (mt + 1)],
                        wv[:, 264 * k:264 * (k + 1)],
                        start=(k == 0), stop=(k == 1))
                nc.vector.tensor_add(
                    vto[:, 8 * HB * mt:8 * HB * (mt + 1)], ps[:], bv[:])

            def transposes(x):
                xnm_x = xnm_tiles[x]
                for s in range(4):
                    tp = scp.tile([128, 128], BF16, name="tp", tag="w")
                    nc.tensor.transpose(tp[:], xnm_x[:, 128 * s:128 * (s + 1)], idn[:])
                    nc.vector.tensor_copy(xxt[x][:, 128 * s:128 * (s + 1)], tp[:])

            def outproj(qc):
                q0 = 512 * qc
                for ct in range(2):
                    ps = scp.tile([128, 512], F32, name="ps_p", tag="w")
                    for k in range(2):
                        nc.tensor.matmul(
                            ps[:], wp[:, 256 * k + 128 * ct:256 * k + 128 * (ct + 1)],
                            xxt[k][:], start=(k == 0), stop=(k == 1))
                    nc.scalar.activation(
                        outb[ct][:, q0:q0 + 512], ps[:],
                        AF.Identity, bias=bp[:, ct:ct + 1])

            # per-mtile PE-stream hooks: {(gi, mt): [fn, ...]}
            hooks = {}

            def add_hook(gi, mt, fn):
                hooks.setdefault((gi, mt), []).append(fn)

            add_hook(0, 0, lambda: proj_kk(0, 0))
            add_hook(0, 0, lambda: proj_qq(0, 0))
            for mt in range(NMT):           # vto(mt) feeds AV(mt), LAG behind
                add_hook(0, mt, lambda mt=mt: proj_vto(mt))
            for c in range(1, 8):           # kk[0] chunk c feeds QKT(mt>=4c)
                add_hook(0, 4 * c - 2, lambda c=c: proj_kk(0, c))
            add_hook(0, 18, lambda: proj_qq(1, 0))
            add_hook(0, 22, lambda: proj_qq(0, 1))
            add_hook(0, 26, lambda: proj_qq(1, 1))
            for c in range(8):              # kk[1] feeds the x=1 groups
                add_hook(1, 3 * c, lambda c=c: proj_kk(1, c))

            # Exp dispatcher: fixed per-group A/D pattern (ACT exact exp
            # vs DVE bitwise 2^x). Strict alternation avoids queue jitter;
            # the extra ACT tiles (DVE is ~15% slower per tile and carries
            # the vto adds early on) are pinned at mt 0/16 where the PE
            # stream has hook/flush slack.
            PAT_STEADY = ["A" if (mt % 2 == 0 or mt in (1, 17)) else "D"
                          for mt in range(NMT)]          # 18A / 14D
            PAT_FIRST = ["A" if (mt % 2 == 0 or mt % 4 == 1) else "D"
                         for mt in range(NMT)]           # 24A / 8D
            GROUP_PATS = [PAT_FIRST, PAT_STEADY] + [PAT_STEADY] * 6
            cur_pat = [PAT_STEADY]

            def emit_exp(et, sc, mt):
                if cur_pat[0][mt] == "A":
                    nc.scalar.activation(et[:], sc[:], AF.Exp)
                else:
                    nc.vector.tensor_scalar(
                        et[:].bitcast(I16), sc[:], SCH_C1, SCH_C2,
                        ALU.mult, ALU.add)

            GROUPS = [(qc, x, pr) for qc in (0, 1) for x in (0, 1) for pr in (0, 1)]
            # deferred PE-side epilogue hooks: group gi's xnm tile (x done
            # at odd gi) is transposed inside group gi+1's stream; the
            # output projection of qc=0 rides in group 4 (qc=1 x=0 pr=0).
            add_hook(2, 10, lambda: transposes(0))
            add_hook(4, 10, lambda: transposes(1))
            add_hook(4, 16, lambda: outproj(0))
            add_hook(6, 10, lambda: transposes(0))

            def out_dma_qc0():
                for ct in range(2):
                    nc.sync.dma_start(out[128 * ct:128 * (ct + 1), 0:512],
                                      outb[ct][:, 0:512])
            add_hook(5, 8, out_dma_qc0)
            LAG = 3
            xnm_tiles = [None, None]
            pend = []       # (emit_av_fn, post_fn_or_None)

            def flush_one():
                fn, post = pend.pop(0)
                fn()
                if post is not None:
                    post()

            for gi, (qc, x, pr) in enumerate(GROUPS):
                q0 = 512 * qc
                cur_pat[0] = GROUP_PATS[gi]
                avt = avp.tile([128, 264], F32, name="av", tag="av")
                # one whole-bank start=True clear; the AV matmuls use
                # start=False (first per-element write overwrites, later
                # ones accumulate)
                nc.tensor.matmul(avt[:], zrow[:], vto[0:1, 0:264],
                                 start=True, stop=False)

                def make_av(avt, x, pr, mt, et):
                    def emit_av():
                        for j2 in range(2):
                            h = 4 * x + 2 * pr + j2
                            for s in range(4):
                                nc.tensor.matmul(
                                    avt[:, 66 * s + 33 * j2:66 * s + 33 * j2 + 33],
                                    et[:, 512 * j2 + 128 * s:512 * j2 + 128 * (s + 1)],
                                    vto[:, 8 * HB * mt + HB * h:8 * HB * mt + HB * (h + 1)],
                                    start=False, stop=(mt == NMT - 1))
                    return emit_av

                def make_epilogue(avt, gi, qc, x, pr):
                    def epilogue():
                        # relu (ACT, PSUM->SBUF), reciprocal of the
                        # denominators (DVE), per-block scale into the
                        # shared pre-transpose tile (Pool).
                        xxm = sp.tile([128, 264], F32, name="xxm", tag="xxm")
                        nc.scalar.activation(xxm[:], avt[:], AF.Relu)
                        rden = sp.tile([128, 8], F32, name="rden", tag="rden")
                        nc.vector.reciprocal(
                            rden[:],
                            xxm[:].rearrange("p (g c) -> p g c", c=33)[:, :, 32])
                        if pr == 0:
                            xnm_tiles[x] = sp.tile([128, 512], BF16,
                                                   name="xnm", tag="xnm")
                        xnm_x = xnm_tiles[x]
                        for g in range(8):      # g = 2*s + j2
                            s, j2 = g // 2, g % 2
                            c0 = 128 * s + 64 * pr + 32 * j2
                            nc.gpsimd.tensor_scalar_mul(
                                xnm_x[:, c0:c0 + 32],
                                xxm[:, 33 * g:33 * g + 32],
                                rden[:, g:g + 1])
                    return epilogue

                for mt in range(NMT):
                    for fn in hooks.get((gi, mt), []):
                        fn()
                    while len(pend) > LAG:
                        flush_one()
                    sc = scp.tile([128, 1024], F32, name="sc", tag="w")
                    for j2 in range(2):
                        j = 2 * pr + j2
                        nc.tensor.matmul(
                            sc[:, 512 * j2:512 * (j2 + 1)],
                            kk[x][32 * j:32 * j + KD, 128 * mt:128 * (mt + 1)],
                            qq[x][32 * j:32 * j + KD, q0:q0 + 512],
                            start=True, stop=True,
                            tile_position=(32 * j, 0))
                    et = ep.tile([128, 1024], BF16, name="et", tag="et")
                    emit_exp(et, sc, mt)
                    post = make_epilogue(avt, gi, qc, x, pr) if mt == NMT - 1 else None
                    pend.append((make_av(avt, x, pr, mt, et), post))

            # tail: flush remaining AVs (fires the last epilogue), then
            # the final transposes + output projection
            while pend:
                flush_one()
            transposes(1)
            outproj(1)

            for ct in range(2):
                nc.sync.dma_start(out[128 * ct:128 * (ct + 1), 512:1024],
                                  outb[ct][:, 512:1024])
            for _p in (sp, ep, avp, scp):
                _p.release()

    # walrus codegen accepts only ONE sync wait on compute instructions
    # (Matmult / Activation / TensorTensor / ...). The multi-wait cases
    # Tile emits here are all {self-engine, other}: a self-engine wait
    # orders an instruction against an earlier instruction on the SAME
    # in-order engine (WAW through PE's single PSUM write port, ACT/DVE
    # pipeline order), which the hardware already guarantees — drop it.
    _self_prefix = {
        "EngineType.PE": "PE",
        "EngineType.Activation": "Activation",
        "EngineType.DVE": "DVE",
        "EngineType.Pool": "Pool",
        "EngineType.SP": "SP",
    }
    for f in nc.m.functions:
        for bb in f.blocks:
            for inst in bb.instructions:
                si = inst.sync_info
                if si is None or not si.on_wait or len(si.on_wait) < 2:
                    continue
                pref = _self_prefix.get(str(getattr(inst, "engine", "")), None)
                if pref is None:
                    continue
                kept = [w for w in si.on_wait
                        if not str(w.ant_name).startswith(pref)]
                if not kept or len(kept) == len(si.on_wait):
                    continue
                si.on_wait = kept

    # Safety net: any instruction still carrying >1 wait gets all but its
    # last wait hoisted into preceding same-engine NoOps (1 wait each).
    uid = [0]
    for f in nc.m.functions:
        for bb in f.blocks:
            new_insts = []
            for inst in bb.instructions:
                si = inst.sync_info
                if si is not None and si.on_wait and len(si.on_wait) > 1:
                    for w in si.on_wait[:-1]:
                        uid[0] += 1
                        nop = mybir.InstNoOp(
                            name=f"I-waitsplit-{uid[0]}", ins=[], outs=[])
                        nop.engine = inst.engine
                        nop.sync_info = mybir.SyncInfo(
                            on_wait=[w], on_update=[])
                        new_insts.append(nop)
                    si.on_wait = [si.on_wait[-1]]
                new_insts.append(inst)
            bb.instructions = new_insts
    return nc


_CACHE = {}


def _prep_host(inputs):
    """Fold BN into weights; build head-split layouts shared by all cores."""
    import ml_dtypes
    f = np.float32
    Wq = (inputs["Wq"] * inputs["sq"][:, None]).astype(f)
    Wk = (inputs["Wk"] * inputs["sk"][:, None]).astype(f)
    Wv = (inputs["Wv"] * inputs["sv"][:, None]).astype(f)
    Wp = (inputs["Wp"] * inputs["sp"][:, None]).astype(f)

    def split(Wt, b):
        o = []
        for g in range(2):
            Wx = np.zeros((C, 128), f)
            bx = np.zeros((128, 1), f)
            for j in range(4):
                h = 4 * g + j
                Wx[:, 32 * j:32 * j + KD] = Wt[:, KD * h:KD * (h + 1)]
                bx[32 * j:32 * j + KD, 0] = b[KD * h:KD * (h + 1)]
            o.append((np.ascontiguousarray(Wx), bx))
        return o

    (wqA, bqA), (wqB, bqB) = split(Wq.T.astype(f), inputs["bq"])
    (wkA, bkA), (wkB, bkB) = split(Wk.T.astype(f), inputs["bk"])
    WvT = Wv.T.astype(f)                      # [C, 256] cols (h, d)
    wv_ext = np.zeros((C, 264), f)            # col 33h+32 stays 0
    bv_ext = np.zeros((264,), f)
    for h in range(NUM_HEADS):
        wv_ext[:, HB * h:HB * h + 32] = WvT[:, 32 * h:32 * (h + 1)]
        bv_ext[HB * h:HB * h + 32] = inputs["bv"][32 * h:32 * (h + 1)]
        bv_ext[HB * h + 32] = 1.0             # softmax denominator column
    ident_bf16 = np.eye(128, dtype=ml_dtypes.bfloat16)
    return dict(
        w_qA=wqA, w_qB=wqB, w_kA=wkA, w_kB=wkB,
        w_v=wv_ext, w_p=np.ascontiguousarray(Wp.T).astype(ml_dtypes.bfloat16),
        b_qA=bqA, b_qB=bqB, b_kA=bkA, b_kB=bkB,
        b_v=np.ascontiguousarray(np.broadcast_to(bv_ext, (128, 264))),
        b_p=inputs["bp"].astype(f).reshape(C, 1),
        ident=ident_bf16,
    )


def kernel(**inputs) -> np.ndarray:
    inputs = {k: np.asarray(v) for k, v in inputs.items()}
    if "nc" not in _CACHE:
        _CACHE["nc"] = build_nc()
    nc = _CACHE["nc"]

    shared = _prep_host(inputs)
    rgb = np.ascontiguousarray(inputs["rgb"].astype(np.float32).reshape(B, C, N))
    edge = np.ascontiguousarray(inputs["edge"].astype(np.float32).reshape(B, C, N))

    in_maps = []
    for core in range(8):
        b, qs = core // 4, core % 4
        m = dict(shared)
        m["rgb_s"] = np.ascontiguousarray(rgb[b][:, QCH * qs:QCH * (qs + 1)])
        m["edge"] = edge[b]
        in_maps.append(m)

    res = run_bass_kernel_spmd(nc, in_maps, core_ids=list(range(8)))
    full = np.zeros((B, C, N), np.float32)
    for core in range(8):
        b, qs = core // 4, core % 4
        full[b][:, QCH * qs:QCH * (qs + 1)] = res.results[core]["out"]
    return full.reshape(B, C, H, W)


# revision 16
# speedup vs baseline: 1.4736x; 1.0173x over previous
"""Cross-modal attention Trainium2 kernel.

Reference computation (all 1x1 convs + folded eval-mode BN):
  q = BN(Wq @ rgb), k = BN(Wk @ edge), v = BN(Wv @ edge)
  attn = softmax(q^T k) per head; xx = relu(attn @ v); out = BN(Wp @ xx)

Shapes: B=2, C=256, H=W=64 (N=4096), heads=8, key_dim=16, d=32.

Sharding: 8 cores = (batch b in {0,1}) x (query-slice qs in {0..3}, 1024
queries each). Each core computes K/V projections for the full N of its
batch (cheap) and attention + output projection for its query slice; the
host concatenates slices. No collectives.

Per-core dataflow (scores kept transposed so softmax-sum and the AV
contraction both run on the m axis without transposing the big matrix):
  sT[m, q] = sum_kd kk[kd, m] qq[kd, q]     (PE, fp32r, 32x128 row-tiled)
  e = exp(sT)        split across two engines, balanced ~53/47:
      ACT:  exp activation (PSUM->SBUF bf16)
      DVE:  Schraudolph bitwise exp: bf16_bits(int16(s*184.665 + 16248.6))
            = 2^(s*log2 e) with ~+-3% multiplicative error that cancels in
            softmax normalization (max |s| ~= 45, safe window (-88, +89)).
  av[q, (h: d|den)] += e[mtile]^T @ [v|1]   (PE bf16, exp as stationary)
  xx = relu(av), xnm = xx * recip(den)      (ACT relu; Pool per-head scale)
  out = Wp^T @ xxt + bp                     (PE bf16 transpose, DMA PSUM->SBUF
                                             move, matmul, ACT bias add)

Engine budget per core (TimelineSim): PE ~163us (QK^T 109 + AV 30 + proj/
misc 24), ACT ~158us (exp share + bias adds + relu), DVE ~158us (exp share
+ v moves), Pool/SP/DMA light. Baseline (all exp on ACT) was 315us.
"""

import sys

for p in ("/opt/trn_rl_repo", "/opt/trn_rl_repo/concourse"):
    if p not in sys.path:
        sys.path.insert(0, p)

import numpy as np

import concourse.bass as bass
import concourse.mybir as mybir
import concourse.tile as tile
from concourse.bass_utils import run_bass_kernel_spmd

F32 = mybir.dt.float32
F32R = mybir.dt.float32r
BF16 = mybir.dt.bfloat16
I16 = mybir.dt.int16
AF = mybir.ActivationFunctionType
ALU = mybir.AluOpType

NUM_HEADS, KD, D = 8, 16, 32
B, C, H, W = 2, 256, 64, 64
N = H * W            # 4096 keys per batch
QCH = 1024           # queries per core
NMT = N // 128       # 32 m-tiles
HB = 33              # per-head AV block: 32 v-cols + 1 denominator col

# Schraudolph exp: bf16_bits(int16(s*SCH_C1 + SCH_C2)) ~= e^s.
SCH_C1 = 184.66496580927726     # 128 * log2(e)
SCH_C2 = 16248.6                # 127*128 minus mean-zeroing interp bias
ACT_FRAC = 0.5266               # share of exp tiles on ACT (rest on DVE)


def build_nc(act_frac=ACT_FRAC):
    nc = bass.Bass()

    rgb_s = nc.dram_tensor("rgb_s", [C, QCH], F32R, kind="ExternalInput")
    edge = nc.dram_tensor("edge", [C, N], F32R, kind="ExternalInput")
    w_qA = nc.dram_tensor("w_qA", [C, 128], F32R, kind="ExternalInput")
    w_qB = nc.dram_tensor("w_qB", [C, 128], F32R, kind="ExternalInput")
    w_kA = nc.dram_tensor("w_kA", [C, 128], F32R, kind="ExternalInput")
    w_kB = nc.dram_tensor("w_kB", [C, 128], F32R, kind="ExternalInput")
    w_v = nc.dram_tensor("w_v", [C, 264], F32R, kind="ExternalInput")
    w_p = nc.dram_tensor("w_p", [256, C], BF16, kind="ExternalInput")
    b_qA = nc.dram_tensor("b_qA", [128, 1], F32, kind="ExternalInput")
    b_qB = nc.dram_tensor("b_qB", [128, 1], F32, kind="ExternalInput")
    b_kA = nc.dram_tensor("b_kA", [128, 1], F32, kind="ExternalInput")
    b_kB = nc.dram_tensor("b_kB", [128, 1], F32, kind="ExternalInput")
    b_v = nc.dram_tensor("b_v", [128, 264], F32, kind="ExternalInput")
    b_p = nc.dram_tensor("b_p", [C, 1], F32, kind="ExternalInput")
    ident = nc.dram_tensor("ident", [128, 128], BF16, kind="ExternalInput")
    out = nc.dram_tensor("out", [C, QCH], F32, kind="ExternalOutput")

    with tile.TileContext(nc) as tc:
        with tc.tile_pool(name="const", bufs=1) as cp, \
             tc.tile_pool(name="data", bufs=1) as dp:
            wq = [cp.tile([128, 256], F32R, name=f"wq{x}", tag=f"wq{x}") for x in "AB"]
            wk = [cp.tile([128, 256], F32R, name=f"wk{x}", tag=f"wk{x}") for x in "AB"]
            wv = cp.tile([128, 528], F32R, name="wv", tag="wv")
            wp = cp.tile([128, 512], BF16, name="wp", tag="wp")
            bq = [cp.tile([128, 1], F32, name=f"bq{x}", tag=f"bq{x}") for x in "AB"]
            bk = [cp.tile([128, 1], F32, name=f"bk{x}", tag=f"bk{x}") for x in "AB"]
            bv = cp.tile([128, 264], F32, name="bv", tag="bv")
            bp = cp.tile([128, 2], F32, name="bp", tag="bp")
            idn = cp.tile([128, 128], BF16, name="idn", tag="idn")
            zrow = cp.tile([1, 128], BF16, name="zrow", tag="zrow")

            nc.sync.dma_start(bk[0][:], b_kA[:])
            nc.sync.dma_start(bq[0][:], b_qA[:])
            nc.vector.memset(zrow[:], 0.0)

            rgb_sb = dp.tile([128, 2 * QCH], F32R, name="rgb_sb", tag="rgb")
            edge_sb = [dp.tile([128, N], F32R, name=f"edge{k}", tag=f"edge{k}") for k in range(2)]

            # Direct DMA into the f32r tiles (f32r bits == f32; the HW
            # matmul rounds on read). Inputs are chunked and ordered so the
            # first QKT can start ~6us in: kk[0]-chunk-0 + qq[0]-qc0 deps
            # first, then pieces in first-use order. The projections
            # themselves are interleaved into the first attention groups.
            for k in range(2):
                nc.sync.dma_start(wk[0][:, 128 * k:128 * (k + 1)], w_kA[128 * k:128 * (k + 1), :])
                nc.sync.dma_start(wq[0][:, 128 * k:128 * (k + 1)], w_qA[128 * k:128 * (k + 1), :])
            for k in range(2):      # first edge columns (kk chunk 0 dep)
                nc.sync.dma_start(edge_sb[k][:, 0:512], edge[128 * k:128 * (k + 1), 0:512])
            for k in range(2):      # rgb first halves (qc=0 queries)
                nc.sync.dma_start(rgb_sb[:, QCH * k:QCH * k + 512], rgb_s[128 * k:128 * (k + 1), 0:512])
            for k in range(2):
                nc.sync.dma_start(edge_sb[k][:, 512:1024], edge[128 * k:128 * (k + 1), 512:1024])
            for k in range(2):
                nc.sync.dma_start(wv[:, 264 * k:264 * (k + 1)], w_v[128 * k:128 * (k + 1), :])
            nc.sync.dma_start(bv[:], b_v[:])
            nc.sync.dma_start(bq[1][:], b_qB[:])
            nc.sync.dma_start(bk[1][:], b_kB[:])
            nc.sync.dma_start(bp[:, 0:1], b_p[0:128, :])
            nc.sync.dma_start(bp[:, 1:2], b_p[128:256, :])
            for k in range(2):
                nc.sync.dma_start(edge_sb[k][:, 1024:2048], edge[128 * k:128 * (k + 1), 1024:2048])
            for k in range(2):
                nc.sync.dma_start(rgb_sb[:, QCH * k + 512:QCH * (k + 1)], rgb_s[128 * k:128 * (k + 1), 512:1024])
            for k in range(2):
                nc.sync.dma_start(wk[1][:, 128 * k:128 * (k + 1)], w_kB[128 * k:128 * (k + 1), :])
                nc.sync.dma_start(wq[1][:, 128 * k:128 * (k + 1)], w_qB[128 * k:128 * (k + 1), :])
            for p in range(2, 4):
                for k in range(2):
                    nc.sync.dma_start(edge_sb[k][:, 1024 * p:1024 * (p + 1)],
                                      edge[128 * k:128 * (k + 1), 1024 * p:1024 * (p + 1)])
            for k in range(2):
                nc.sync.dma_start(wp[:, 256 * k:256 * (k + 1)], w_p[128 * k:128 * (k + 1), :])
            nc.sync.dma_start(idn[:], ident[:])

            qq = [dp.tile([128, QCH], F32R, name=f"qq{x}", tag=f"qq{x}") for x in "AB"]
            kk = [dp.tile([128, N], F32R, name=f"kk{x}", tag=f"kk{x}") for x in "AB"]
            vto = dp.tile([128, NMT * 8 * HB], BF16, name="vto", tag="vto")
            outb = [dp.tile([128, QCH], F32, name=f"outb{k}", tag=f"outb{k}") for k in range(2)]
            xxt = [dp.tile([128, 512], BF16, name=f"xxt{k}", tag=f"xxt{k}") for k in range(2)]

            # ---- pools (PSUM pools span the whole kernel: releasing a
            # PSUM pool makes successor tiles inherit released-zone deps
            # as multi-wait instructions, which walrus rejects) ----
            # PSUM budget (8 banks): scp 3 x [128,1024]f32 (2 banks each)
            # for a 3-deep QKT->exp score pipeline; avp 2 x [128,264]f32
            # (1 bank) for the per-(x,pr) AV accumulator, s-chunks packed
            # along columns. Projection / transpose / out-proj psums
            # borrow scp zones transiently.
            scp = tc.alloc_tile_pool(name="scp", bufs=3, space="PSUM")
            avp = tc.alloc_tile_pool(name="avp", bufs=1, space="PSUM")
            psv = tc.alloc_tile_pool(name="psv", bufs=1, space="PSUM")
            ep = tc.alloc_tile_pool(name="exp", bufs=12)
            sp = tc.alloc_tile_pool(name="stg", bufs=6)

            # ---- fused projection + attention stream ----
            # All work is emitted as ONE software-pipelined mtile stream
            # across the 8 (qc, x, pr) head-pair groups. Projections ride
            # as per-mtile hooks inside the first two groups (hidden under
            # the attention steady state); AV consumption lags QKT/exp by
            # LAG mtiles (across group boundaries too) so PE never waits
            # on the exp engines; per-group epilogues (relu/recip/scale)
            # fire when the group's last AV flushes, and their PE parts
            # (transposes, output projection) are deferred hooks inside
            # later groups' streams.

            # ACT warmup: absorb the DVE tick (activation-bias const
            # tiles are DVE-written) so the first exp carries only the PE
            # wait — walrus allows a single wait per Activation.
            actw = dp.tile([1, 1], BF16, name="actw", tag="actw")
            nc.scalar.activation(actw[:], zrow[0:1, 0:1], AF.Exp)

            def proj_qq(x, j):
                ps = scp.tile([128, 512], F32, name="ps_q", tag="w")
                for k in range(2):
                    nc.tensor.matmul(
                        ps[:], wq[x][:, 128 * k:128 * (k + 1)],
                        rgb_sb[:, QCH * k + 512 * j:QCH * k + 512 * (j + 1)],
                        start=(k == 0), stop=(k == 1))
                nc.scalar.activation(
                    qq[x][:, 512 * j:512 * (j + 1)], ps[:],
                    AF.Identity, bias=bq[x][:])

            def proj_kk(x, j):
                ps = scp.tile([128, 512], F32, name="ps_k", tag="w")
                for k in range(2):
                    nc.tensor.matmul(
                        ps[:], wk[x][:, 128 * k:128 * (k + 1)],
                        edge_sb[k][:, 512 * j:512 * (j + 1)],
                        start=(k == 0), stop=(k == 1))
                nc.scalar.activation(
                    kk[x][:, 512 * j:512 * (j + 1)], ps[:],
                    AF.Identity, bias=bk[x][:])

            def proj_vto(mt):
                # v^T: vto[m, (h: 32 v | den)]. wv has zero columns at the
                # denominator positions and bv carries the 1.0s there.
                ps = psv.tile([128, 264], F32, name="ps_v", tag="v")
                for k in range(2):
                    nc.tensor.matmul(
                        ps[:], edge_sb[k][:, 128 * mt:128 * (mt + 1)],
                        wv[:, 264 * k:264 * (k + 1)],
                        start=(k == 0), stop=(k == 1))
                nc.vector.tensor_add(
                    vto[:, 8 * HB * mt:8 * HB * (mt + 1)], ps[:], bv[:])

            def transposes(x):
                xnm_x = xnm_tiles[x]
                for s in range(4):
                    tp = scp.tile([128, 128], BF16, name="tp", tag="w")
                    nc.tensor.transpose(tp[:], xnm_x[:, 128 * s:128 * (s + 1)], idn[:])
                    nc.vector.tensor_copy(xxt[x][:, 128 * s:128 * (s + 1)], tp[:])

            def outproj(qc):
                q0 = 512 * qc
                for ct in range(2):
                    ps = scp.tile([128, 512], F32, name="ps_p", tag="w")
                    for k in range(2):
                        nc.tensor.matmul(
                            ps[:], wp[:, 256 * k + 128 * ct:256 * k + 128 * (ct + 1)],
                            xxt[k][:], start=(k == 0), stop=(k == 1))
                    nc.scalar.activation(
                        outb[ct][:, q0:q0 + 512], ps[:],
                        AF.Identity, bias=bp[:, ct:ct + 1])

            # per-mtile PE-stream hooks: {(gi, mt): [fn, ...]}
            hooks = {}

            def add_hook(gi, mt, fn):
                hooks.setdefault((gi, mt), []).append(fn)

            add_hook(0, 0, lambda: proj_kk(0, 0))
            add_hook(0, 0, lambda: proj_qq(0, 0))
            for mt in range(NMT):           # vto(mt) feeds AV(mt), LAG behind
                add_hook(0, mt, lambda mt=mt: proj_vto(mt))
            for c in range(1, 8):           # kk[0] chunk c feeds QKT(mt>=4c)
                add_hook(0, 4 * c - 2, lambda c=c: proj_kk(0, c))
            add_hook(1, 24, lambda: proj_qq(1, 0))
            add_hook(3, 10, lambda: proj_qq(0, 1))
            add_hook(5, 14, lambda: proj_qq(1, 1))
            # kk[1] feeds the x=1 groups: chunk 0 late in gi1, the rest
            # just-in-time inside gi2 (chunk c consumed from mt=4c)
            add_hook(1, 28, lambda: proj_kk(1, 0))
            for c in range(1, 8):
                add_hook(2, 4 * c - 2, lambda c=c: proj_kk(1, c))

            # Exp dispatcher: fixed per-group A/D pattern (ACT exact exp
            # vs DVE bitwise 2^x). Strict alternation avoids queue jitter;
            # the extra ACT tiles (DVE is ~15% slower per tile and carries
            # the vto adds early on) are pinned at mt 0/16 where the PE
            # stream has hook/flush slack.
            PAT_STEADY = ["A" if (mt % 2 == 0 or mt in (9, 21)) else "D"
                          for mt in range(NMT)]          # 18A / 14D
            PAT_FIRST = ["A" if (mt % 2 == 0 or mt in (1, 9, 17, 25)) else "D"
                         for mt in range(NMT)]           # 20A / 12D
            GROUP_PATS = [PAT_FIRST, PAT_STEADY] + [PAT_STEADY] * 6
            cur_pat = [PAT_STEADY]

            def emit_exp(et, sc, mt):
                if cur_pat[0][mt] == "A":
                    nc.scalar.activation(et[:], sc[:], AF.Exp)
                else:
                    nc.vector.tensor_scalar(
                        et[:].bitcast(I16), sc[:], SCH_C1, SCH_C2,
                        ALU.mult, ALU.add)

            GROUPS = [(qc, x, pr) for qc in (0, 1) for x in (0, 1) for pr in (0, 1)]
            # deferred PE-side epilogue hooks: group gi's xnm tile (x done
            # at odd gi) is transposed inside group gi+1's stream; the
            # output projection of qc=0 rides in group 4 (qc=1 x=0 pr=0).
            add_hook(2, 10, lambda: transposes(0))
            add_hook(4, 10, lambda: transposes(1))
            add_hook(4, 16, lambda: outproj(0))
            add_hook(6, 10, lambda: transposes(0))

            def out_dma_qc0():
                for ct in range(2):
                    nc.sync.dma_start(out[128 * ct:128 * (ct + 1), 0:512],
                                      outb[ct][:, 0:512])
            add_hook(5, 8, out_dma_qc0)
            LAG = 3
            xnm_tiles = [None, None]
            pend = []       # (emit_av_fn, post_fn_or_None)

            def flush_one():
                fn, post = pend.pop(0)
                fn()
                if post is not None:
                    post()

            for gi, (qc, x, pr) in enumerate(GROUPS):
                q0 = 512 * qc
                cur_pat[0] = GROUP_PATS[gi]
                avt = avp.tile([128, 264], F32, name="av", tag="av")

                def make_av(avt, x, pr, mt, et):
                    def emit_av():
                        for j2 in range(2):
                            h = 4 * x + 2 * pr + j2
                            for s in range(4):
                                nc.tensor.matmul(
                                    avt[:, 66 * s + 33 * j2:66 * s + 33 * j2 + 33],
                                    et[:, 512 * j2 + 128 * s:512 * j2 + 128 * (s + 1)],
                                    vto[:, 8 * HB * mt + HB * h:8 * HB * mt + HB * (h + 1)],
                                    start=False, stop=(mt == NMT - 1))
                    return emit_av

                def make_epilogue(avt, gi, qc, x, pr):
                    def epilogue():
                        # relu (ACT, PSUM->SBUF), reciprocal of the
                        # denominators (DVE), per-block scale into the
                        # shared pre-transpose tile (Pool).
                        xxm = sp.tile([128, 264], F32, name="xxm", tag="xxm")
                        nc.scalar.activation(xxm[:], avt[:], AF.Relu)
                        rden = sp.tile([128, 8], F32, name="rden", tag="rden")
                        nc.vector.reciprocal(
                            rden[:],
                            xxm[:].rearrange("p (g c) -> p g c", c=33)[:, :, 32])
                        if pr == 0:
                            xnm_tiles[x] = sp.tile([128, 512], BF16,
                                                   name="xnm", tag="xnm")
                        xnm_x = xnm_tiles[x]
                        for g in range(8):      # g = 2*s + j2
                            s, j2 = g // 2, g % 2
                            c0 = 128 * s + 64 * pr + 32 * j2
                            nc.gpsimd.tensor_scalar_mul(
                                xnm_x[:, c0:c0 + 32],
                                xxm[:, 33 * g:33 * g + 32],
                                rden[:, g:g + 1])
                    return epilogue

                for mt in range(NMT):
                    for fn in hooks.get((gi, mt), []):
                        fn()
                    if mt == 6:
                        # whole-bank start=True clear of this group's AV
                        # accumulator; deferred here so it lands after the
                        # previous group's epilogue (relu) was emitted —
                        # avp has a single zone. The AV matmuls use
                        # start=False (first per-element write overwrites,
                        # later ones accumulate).
                        nc.tensor.matmul(avt[:], zrow[:], vto[0:1, 0:264],
                                         start=True, stop=False)
                    # deeper flush threshold early in the group: the first
                    # own-AV flush (needs the cleared accumulator) waits
                    # until mt=8, well past the clear
                    limit = 7 if mt < 8 else LAG
                    while len(pend) > limit:
                        flush_one()
                    sc = scp.tile([128, 1024], F32, name="sc", tag="w")
                    for j2 in range(2):
                        j = 2 * pr + j2
                        nc.tensor.matmul(
                            sc[:, 512 * j2:512 * (j2 + 1)],
                            kk[x][32 * j:32 * j + KD, 128 * mt:128 * (mt + 1)],
                            qq[x][32 * j:32 * j + KD, q0:q0 + 512],
                            start=True, stop=True,
                            tile_position=(32 * j, 0))
                    et = ep.tile([128, 1024], BF16, name="et", tag="et")
                    emit_exp(et, sc, mt)
                    post = make_epilogue(avt, gi, qc, x, pr) if mt == NMT - 1 else None
                    pend.append((make_av(avt, x, pr, mt, et), post))

            # tail: flush remaining AVs (fires the last epilogue), then
            # the final transposes + output projection
            while pend:
                flush_one()
            transposes(1)
            outproj(1)

            for ct in range(2):
                nc.sync.dma_start(out[128 * ct:128 * (ct + 1), 512:1024],
                                  outb[ct][:, 512:1024])
            for _p in (sp, ep, psv, avp, scp):
                _p.release()

    # walrus codegen accepts only ONE sync wait on compute instructions
    # (Matmult / Activation / TensorTensor / ...). The multi-wait cases
    # Tile emits here are all {self-engine, other}: a self-engine wait
    # orders an instruction against an earlier instruction on the SAME
    # in-order engine (WAW through PE's single PSUM write port, ACT/DVE
    # pipeline order), which the hardware already guarantees — drop it.
    _self_prefix = {
        "EngineType.PE": "PE",
        "EngineType.Activation": "Activation",
        "EngineType.DVE": "DVE",
        "EngineType.Pool": "Pool",
        "EngineType.SP": "SP",
    }
    for f in nc.m.functions:
        for bb in f.blocks:
            for inst in bb.instructions:
                si = inst.sync_info
                if si is None or not si.on_wait or len(si.on_wait) < 2:
                    continue
                pref = _self_prefix.get(str(getattr(inst, "engine", "")), None)
                if pref is None:
                    continue
                kept = [w for w in si.on_wait
                        if not str(w.ant_name).startswith(pref)]
                if not kept or len(kept) == len(si.on_wait):
                    continue
                si.on_wait = kept

    # Safety net: any instruction still carrying >1 wait gets all but its
    # last wait hoisted into preceding same-engine NoOps (1 wait each).
    uid = [0]
    for f in nc.m.functions:
        for bb in f.blocks:
            new_insts = []
            for inst in bb.instructions:
                si = inst.sync_info
                if si is not None and si.on_wait and len(si.on_wait) > 1:
                    for w in si.on_wait[:-1]:
                        uid[0] += 1
                        nop = mybir.InstNoOp(
                            name=f"I-waitsplit-{uid[0]}", ins=[], outs=[])
                        nop.engine = inst.engine
                        nop.sync_info = mybir.SyncInfo(
                            on_wait=[w], on_update=[])
                        new_insts.append(nop)
                    si.on_wait = [si.on_wait[-1]]
                new_insts.append(inst)
            bb.instructions = new_insts
    return nc


_CACHE = {}


def _prep_host(inputs):
    """Fold BN into weights; build head-split layouts shared by all cores."""
    import ml_dtypes
    f = np.float32
    Wq = (inputs["Wq"] * inputs["sq"][:, None]).astype(f)
    Wk = (inputs["Wk"] * inputs["sk"][:, None]).astype(f)
    Wv = (inputs["Wv"] * inputs["sv"][:, None]).astype(f)
    Wp = (inputs["Wp"] * inputs["sp"][:, None]).astype(f)

    def split(Wt, b):
        o = []
        for g in range(2):
            Wx = np.zeros((C, 128), f)
            bx = np.zeros((128, 1), f)
            for j in range(4):
                h = 4 * g + j
                Wx[:, 32 * j:32 * j + KD] = Wt[:, KD * h:KD * (h + 1)]
                bx[32 * j:32 * j + KD, 0] = b[KD * h:KD * (h + 1)]
            o.append((np.ascontiguousarray(Wx), bx))
        return o

    (wqA, bqA), (wqB, bqB) = split(Wq.T.astype(f), inputs["bq"])
    (wkA, bkA), (wkB, bkB) = split(Wk.T.astype(f), inputs["bk"])
    WvT = Wv.T.astype(f)                      # [C, 256] cols (h, d)
    wv_ext = np.zeros((C, 264), f)            # col 33h+32 stays 0
    bv_ext = np.zeros((264,), f)
    for h in range(NUM_HEADS):
        wv_ext[:, HB * h:HB * h + 32] = WvT[:, 32 * h:32 * (h + 1)]
        bv_ext[HB * h:HB * h + 32] = inputs["bv"][32 * h:32 * (h + 1)]
        bv_ext[HB * h + 32] = 1.0             # softmax denominator column
    ident_bf16 = np.eye(128, dtype=ml_dtypes.bfloat16)
    return dict(
        w_qA=wqA, w_qB=wqB, w_kA=wkA, w_kB=wkB,
        w_v=wv_ext, w_p=np.ascontiguousarray(Wp.T).astype(ml_dtypes.bfloat16),
        b_qA=bqA, b_qB=bqB, b_kA=bkA, b_kB=bkB,
        b_v=np.ascontiguousarray(np.broadcast_to(bv_ext, (128, 264))),
        b_p=inputs["bp"].astype(f).reshape(C, 1),
        ident=ident_bf16,
    )


def kernel(**inputs) -> np.ndarray:
    inputs = {k: np.asarray(v) for k, v in inputs.items()}
    if "nc" not in _CACHE:
        _CACHE["nc"] = build_nc()
    nc = _CACHE["nc"]

    shared = _prep_host(inputs)
    rgb = np.ascontiguousarray(inputs["rgb"].astype(np.float32).reshape(B, C, N))
    edge = np.ascontiguousarray(inputs["edge"].astype(np.float32).reshape(B, C, N))

    in_maps = []
    for core in range(8):
        b, qs = core // 4, core % 4
        m = dict(shared)
        m["rgb_s"] = np.ascontiguousarray(rgb[b][:, QCH * qs:QCH * (qs + 1)])
        m["edge"] = edge[b]
        in_maps.append(m)

    res = run_bass_kernel_spmd(nc, in_maps, core_ids=list(range(8)))
    full = np.zeros((B, C, N), np.float32)
    for core in range(8):
        b, qs = core // 4, core % 4
        full[b][:, QCH * qs:QCH * (qs + 1)] = res.results[core]["out"]
    return full.reshape(B, C, H, W)


# revision 18
# speedup vs baseline: 1.5208x; 1.0320x over previous
"""Cross-modal attention Trainium2 kernel.

Reference computation (all 1x1 convs + folded eval-mode BN):
  q = BN(Wq @ rgb), k = BN(Wk @ edge), v = BN(Wv @ edge)
  attn = softmax(q^T k) per head; xx = relu(attn @ v); out = BN(Wp @ xx)

Shapes: B=2, C=256, H=W=64 (N=4096), heads=8, key_dim=16, d=32.

Sharding: 8 cores = (batch b in {0,1}) x (query-slice qs in {0..3}, 1024
queries each). Each core computes K/V projections for the full N of its
batch (cheap) and attention + output projection for its query slice; the
host concatenates slices. No collectives.

Per-core dataflow (scores kept transposed so softmax-sum and the AV
contraction both run on the m axis without transposing the big matrix):
  sT[m, q] = sum_kd kk[kd, m] qq[kd, q]     (PE, fp32r, 32x128 row-tiled)
  e = exp(sT)        split across two engines, balanced ~53/47:
      ACT:  exp activation (PSUM->SBUF bf16)
      DVE:  Schraudolph bitwise exp: bf16_bits(int16(s*184.665 + 16248.6))
            = 2^(s*log2 e) with ~+-3% multiplicative error that cancels in
            softmax normalization (max |s| ~= 45, safe window (-88, +89)).
  av[q, (h: d|den)] += e[mtile]^T @ [v|1]   (PE bf16, exp as stationary)
  xx = relu(av), xnm = xx * recip(den)      (ACT relu; Pool per-head scale)
  out = Wp^T @ xxt + bp                     (PE bf16 transpose, DMA PSUM->SBUF
                                             move, matmul, ACT bias add)

Engine budget per core (TimelineSim): PE ~163us (QK^T 109 + AV 30 + proj/
misc 24), ACT ~158us (exp share + bias adds + relu), DVE ~158us (exp share
+ v moves), Pool/SP/DMA light. Baseline (all exp on ACT) was 315us.
"""

import sys

for p in ("/opt/trn_rl_repo", "/opt/trn_rl_repo/concourse"):
    if p not in sys.path:
        sys.path.insert(0, p)

import numpy as np

import concourse.bass as bass
import concourse.mybir as mybir
import concourse.tile as tile
from concourse.bass_utils import run_bass_kernel_spmd

F32 = mybir.dt.float32
F32R = mybir.dt.float32r
BF16 = mybir.dt.bfloat16
I16 = mybir.dt.int16
AF = mybir.ActivationFunctionType
ALU = mybir.AluOpType

NUM_HEADS, KD, D = 8, 16, 32
B, C, H, W = 2, 256, 64, 64
N = H * W            # 4096 keys per batch
QCH = 1024           # queries per core
NMT = N // 128       # 32 m-tiles
HB = 33              # per-head AV block: 32 v-cols + 1 denominator col

# Schraudolph exp: bf16_bits(int16(s*SCH_C1 + SCH_C2)) ~= e^s.
SCH_C1 = 184.66496580927726     # 128 * log2(e)
SCH_C2 = 16248.6                # 127*128 minus mean-zeroing interp bias
ACT_FRAC = 0.5266               # share of exp tiles on ACT (rest on DVE)


def build_nc(act_frac=ACT_FRAC):
    nc = bass.Bass()

    rgb_s = nc.dram_tensor("rgb_s", [C, QCH], F32R, kind="ExternalInput")
    edge = nc.dram_tensor("edge", [C, N], F32R, kind="ExternalInput")
    w_qA = nc.dram_tensor("w_qA", [C, 128], F32R, kind="ExternalInput")
    w_qB = nc.dram_tensor("w_qB", [C, 128], F32R, kind="ExternalInput")
    w_kA = nc.dram_tensor("w_kA", [C, 128], F32R, kind="ExternalInput")
    w_kB = nc.dram_tensor("w_kB", [C, 128], F32R, kind="ExternalInput")
    w_v = nc.dram_tensor("w_v", [C, 264], F32R, kind="ExternalInput")
    w_p = nc.dram_tensor("w_p", [256, C], BF16, kind="ExternalInput")
    b_qA = nc.dram_tensor("b_qA", [128, 1], F32, kind="ExternalInput")
    b_qB = nc.dram_tensor("b_qB", [128, 1], F32, kind="ExternalInput")
    b_kA = nc.dram_tensor("b_kA", [128, 1], F32, kind="ExternalInput")
    b_kB = nc.dram_tensor("b_kB", [128, 1], F32, kind="ExternalInput")
    b_v = nc.dram_tensor("b_v", [128, 264], F32, kind="ExternalInput")
    b_p = nc.dram_tensor("b_p", [C, 1], F32, kind="ExternalInput")
    ident = nc.dram_tensor("ident", [128, 128], BF16, kind="ExternalInput")
    out = nc.dram_tensor("out", [C, QCH], F32, kind="ExternalOutput")

    with tile.TileContext(nc) as tc:
        with tc.tile_pool(name="const", bufs=1) as cp, \
             tc.tile_pool(name="data", bufs=1) as dp:
            wq = [cp.tile([128, 256], F32R, name=f"wq{x}", tag=f"wq{x}") for x in "AB"]
            wk = [cp.tile([128, 256], F32R, name=f"wk{x}", tag=f"wk{x}") for x in "AB"]
            wv = cp.tile([128, 528], F32R, name="wv", tag="wv")
            wp = cp.tile([128, 512], BF16, name="wp", tag="wp")
            bq = [cp.tile([128, 1], F32, name=f"bq{x}", tag=f"bq{x}") for x in "AB"]
            bk = [cp.tile([128, 1], F32, name=f"bk{x}", tag=f"bk{x}") for x in "AB"]
            bv = cp.tile([128, 264], F32, name="bv", tag="bv")
            bp = cp.tile([128, 2], F32, name="bp", tag="bp")
            idn = cp.tile([128, 128], BF16, name="idn", tag="idn")
            zrow = cp.tile([1, 128], BF16, name="zrow", tag="zrow")

            nc.sync.dma_start(bk[0][:], b_kA[:])
            nc.sync.dma_start(bq[0][:], b_qA[:])
            nc.vector.memset(zrow[:], 0.0)

            rgb_sb = dp.tile([128, 2 * QCH], F32R, name="rgb_sb", tag="rgb")
            edge_sb = [dp.tile([128, N], F32R, name=f"edge{k}", tag=f"edge{k}") for k in range(2)]

            # Direct DMA into the f32r tiles (f32r bits == f32; the HW
            # matmul rounds on read). Inputs are chunked and ordered so the
            # first QKT can start ~6us in: kk[0]-chunk-0 + qq[0]-qc0 deps
            # first, then pieces in first-use order. The projections
            # themselves are interleaved into the first attention groups.
            for k in range(2):
                nc.sync.dma_start(wk[0][:, 128 * k:128 * (k + 1)], w_kA[128 * k:128 * (k + 1), :])
                nc.sync.dma_start(wq[0][:, 128 * k:128 * (k + 1)], w_qA[128 * k:128 * (k + 1), :])
            for k in range(2):      # first edge columns (kk chunk 0 dep)
                nc.sync.dma_start(edge_sb[k][:, 0:512], edge[128 * k:128 * (k + 1), 0:512])
            for k in range(2):      # rgb first halves (qc=0 queries)
                nc.sync.dma_start(rgb_sb[:, QCH * k:QCH * k + 512], rgb_s[128 * k:128 * (k + 1), 0:512])
            for k in range(2):
                nc.sync.dma_start(edge_sb[k][:, 512:1024], edge[128 * k:128 * (k + 1), 512:1024])
            for k in range(2):
                nc.sync.dma_start(wv[:, 264 * k:264 * (k + 1)], w_v[128 * k:128 * (k + 1), :])
            nc.sync.dma_start(bv[:], b_v[:])
            nc.sync.dma_start(bq[1][:], b_qB[:])
            nc.sync.dma_start(bk[1][:], b_kB[:])
            nc.sync.dma_start(bp[:, 0:1], b_p[0:128, :])
            nc.sync.dma_start(bp[:, 1:2], b_p[128:256, :])
            for k in range(2):
                nc.sync.dma_start(edge_sb[k][:, 1024:2048], edge[128 * k:128 * (k + 1), 1024:2048])
            for k in range(2):
                nc.sync.dma_start(rgb_sb[:, QCH * k + 512:QCH * (k + 1)], rgb_s[128 * k:128 * (k + 1), 512:1024])
            for k in range(2):
                nc.sync.dma_start(wk[1][:, 128 * k:128 * (k + 1)], w_kB[128 * k:128 * (k + 1), :])
                nc.sync.dma_start(wq[1][:, 128 * k:128 * (k + 1)], w_qB[128 * k:128 * (k + 1), :])
            for p in range(2, 4):
                for k in range(2):
                    nc.sync.dma_start(edge_sb[k][:, 1024 * p:1024 * (p + 1)],
                                      edge[128 * k:128 * (k + 1), 1024 * p:1024 * (p + 1)])
            for k in range(2):
                nc.sync.dma_start(wp[:, 256 * k:256 * (k + 1)], w_p[128 * k:128 * (k + 1), :])
            nc.sync.dma_start(idn[:], ident[:])

            qq = [dp.tile([128, QCH], F32R, name=f"qq{x}", tag=f"qq{x}") for x in "AB"]
            kk = [dp.tile([128, N], F32R, name=f"kk{x}", tag=f"kk{x}") for x in "AB"]
            vto = dp.tile([128, NMT * 8 * HB], BF16, name="vto", tag="vto")
            outb = [dp.tile([128, QCH], F32, name=f"outb{k}", tag=f"outb{k}") for k in range(2)]
            xxt = [dp.tile([128, 512], BF16, name=f"xxt{k}", tag=f"xxt{k}") for k in range(2)]

            # ---- pools (PSUM pools span the whole kernel: releasing a
            # PSUM pool makes successor tiles inherit released-zone deps
            # as multi-wait instructions, which walrus rejects) ----
            # PSUM budget (8 banks): scp 3 x [128,1024]f32 (2 banks each)
            # for a 3-deep QKT->exp score pipeline; avp 2 x [128,264]f32
            # (1 bank) for the per-(x,pr) AV accumulator, s-chunks packed
            # along columns. Projection / transpose / out-proj psums
            # borrow scp zones transiently.
            scp = tc.alloc_tile_pool(name="scp", bufs=3, space="PSUM")
            avp = tc.alloc_tile_pool(name="avp", bufs=1, space="PSUM")
            psv = tc.alloc_tile_pool(name="psv", bufs=1, space="PSUM")
            ep = tc.alloc_tile_pool(name="exp", bufs=12)
            sp = tc.alloc_tile_pool(name="stg", bufs=6)

            # ---- fused projection + attention stream ----
            # All work is emitted as ONE software-pipelined mtile stream
            # across the 8 (qc, x, pr) head-pair groups. Projections ride
            # as per-mtile hooks inside the first two groups (hidden under
            # the attention steady state); AV consumption lags QKT/exp by
            # LAG mtiles (across group boundaries too) so PE never waits
            # on the exp engines; per-group epilogues (relu/recip/scale)
            # fire when the group's last AV flushes, and their PE parts
            # (transposes, output projection) are deferred hooks inside
            # later groups' streams.

            # ACT warmup: absorb the DVE tick (activation-bias const
            # tiles are DVE-written) so the first exp carries only the PE
            # wait — walrus allows a single wait per Activation.
            actw = dp.tile([1, 1], BF16, name="actw", tag="actw")
            nc.scalar.activation(actw[:], zrow[0:1, 0:1], AF.Exp)

            def proj_qq(x, j):
                ps = scp.tile([128, 512], F32, name="ps_q", tag="w")
                for k in range(2):
                    nc.tensor.matmul(
                        ps[:], wq[x][:, 128 * k:128 * (k + 1)],
                        rgb_sb[:, QCH * k + 512 * j:QCH * k + 512 * (j + 1)],
                        start=(k == 0), stop=(k == 1))
                nc.scalar.activation(
                    qq[x][:, 512 * j:512 * (j + 1)], ps[:],
                    AF.Identity, bias=bq[x][:])

            def proj_kk(x, j):
                ps = scp.tile([128, 512], F32, name="ps_k", tag="w")
                for k in range(2):
                    nc.tensor.matmul(
                        ps[:], wk[x][:, 128 * k:128 * (k + 1)],
                        edge_sb[k][:, 512 * j:512 * (j + 1)],
                        start=(k == 0), stop=(k == 1))
                nc.scalar.activation(
                    kk[x][:, 512 * j:512 * (j + 1)], ps[:],
                    AF.Identity, bias=bk[x][:])

            def proj_vto(mt):
                # v^T: vto[m, (h: 32 v | den)]. wv has zero columns at the
                # denominator positions and bv carries the 1.0s there.
                ps = psv.tile([128, 264], F32, name="ps_v", tag="v")
                for k in range(2):
                    nc.tensor.matmul(
                        ps[:], edge_sb[k][:, 128 * mt:128 * (mt + 1)],
                        wv[:, 264 * k:264 * (k + 1)],
                        start=(k == 0), stop=(k == 1))
                nc.vector.tensor_add(
                    vto[:, 8 * HB * mt:8 * HB * (mt + 1)], ps[:], bv[:])

            def transposes(x):
                xnm_x = xnm_tiles[x]
                for s in range(4):
                    tp = scp.tile([128, 128], BF16, name="tp", tag="w")
                    nc.tensor.transpose(tp[:], xnm_x[:, 128 * s:128 * (s + 1)], idn[:])
                    nc.vector.tensor_copy(xxt[x][:, 128 * s:128 * (s + 1)], tp[:])

            def outproj(qc):
                q0 = 512 * qc
                for ct in range(2):
                    ps = scp.tile([128, 512], F32, name="ps_p", tag="w")
                    for k in range(2):
                        nc.tensor.matmul(
                            ps[:], wp[:, 256 * k + 128 * ct:256 * k + 128 * (ct + 1)],
                            xxt[k][:], start=(k == 0), stop=(k == 1))
                    nc.scalar.activation(
                        outb[ct][:, q0:q0 + 512], ps[:],
                        AF.Identity, bias=bp[:, ct:ct + 1])

            # per-mtile PE-stream hooks: {(gi, mt): [fn, ...]}
            hooks = {}

            def add_hook(gi, mt, fn):
                hooks.setdefault((gi, mt), []).append(fn)

            add_hook(0, 0, lambda: proj_kk(0, 0))
            add_hook(0, 0, lambda: proj_qq(0, 0))
            for mt in range(NMT):           # vto(mt) feeds AV(mt), LAG behind
                add_hook(0, mt, lambda mt=mt: proj_vto(mt))
            for c in range(1, 8):           # kk[0] chunk c feeds QKT(mt>=4c)
                add_hook(0, 4 * c - 2, lambda c=c: proj_kk(0, c))
            add_hook(1, 24, lambda: proj_qq(1, 0))
            add_hook(3, 10, lambda: proj_qq(0, 1))
            add_hook(5, 14, lambda: proj_qq(1, 1))
            # kk[1] feeds the x=1 groups: chunk 0 late in gi1, the rest
            # just-in-time inside gi2 (chunk c consumed from mt=4c)
            add_hook(1, 28, lambda: proj_kk(1, 0))
            for c in range(1, 8):
                add_hook(2, 4 * c - 2, lambda c=c: proj_kk(1, c))

            # Exp dispatcher: fixed per-group A/D pattern (ACT exact exp
            # vs DVE bitwise 2^x). Strict alternation avoids queue jitter;
            # the extra ACT tiles (DVE is ~15% slower per tile and carries
            # the vto adds early on) are pinned at mt 0/16 where the PE
            # stream has hook/flush slack.
            PAT_STEADY = ["A" if (mt % 2 == 0 or mt == 9) else "D"
                          for mt in range(NMT)]          # 17A / 15D
            PAT_FIRST = ["A" if (mt % 2 == 0 or mt in (3, 9, 17, 25, 29)) else "D"
                         for mt in range(NMT)]           # 21A / 11D
            GROUP_PATS = [PAT_FIRST, PAT_STEADY] + [PAT_STEADY] * 6
            cur_pat = [PAT_STEADY]

            def emit_exp(et, sc, mt):
                if cur_pat[0][mt] == "A":
                    nc.scalar.activation(et[:], sc[:], AF.Exp)
                else:
                    nc.vector.tensor_scalar(
                        et[:].bitcast(I16), sc[:], SCH_C1, SCH_C2,
                        ALU.mult, ALU.add)

            GROUPS = [(qc, x, pr) for qc in (0, 1) for x in (0, 1) for pr in (0, 1)]
            # deferred PE-side epilogue hooks: group gi's xnm tile (x done
            # at odd gi) is transposed inside group gi+1's stream; the
            # output projection of qc=0 rides in group 4 (qc=1 x=0 pr=0).
            add_hook(2, 10, lambda: transposes(0))
            add_hook(4, 10, lambda: transposes(1))
            add_hook(4, 16, lambda: outproj(0))
            add_hook(6, 10, lambda: transposes(0))

            def out_dma_qc0():
                for ct in range(2):
                    nc.sync.dma_start(out[128 * ct:128 * (ct + 1), 0:512],
                                      outb[ct][:, 0:512])
            add_hook(5, 8, out_dma_qc0)
            LAG = 3
            xnm_tiles = [None, None]
            pend = []       # (emit_av_fn, post_fn_or_None)

            def flush_one():
                fn, post = pend.pop(0)
                fn()
                if post is not None:
                    post()

            for gi, (qc, x, pr) in enumerate(GROUPS):
                q0 = 512 * qc
                cur_pat[0] = GROUP_PATS[gi]
                avt = avp.tile([128, 264], F32, name="av", tag="av")

                def make_av(avt, x, pr, mt, et):
                    def emit_av():
                        for j2 in range(2):
                            h = 4 * x + 2 * pr + j2
                            for s in range(4):
                                nc.tensor.matmul(
                                    avt[:, 66 * s + 33 * j2:66 * s + 33 * j2 + 33],
                                    et[:, 512 * j2 + 128 * s:512 * j2 + 128 * (s + 1)],
                                    vto[:, 8 * HB * mt + HB * h:8 * HB * mt + HB * (h + 1)],
                                    start=False, stop=(mt == NMT - 1))
                    return emit_av

                def make_epilogue(avt, gi, qc, x, pr):
                    def epilogue():
                        # relu (ACT, PSUM->SBUF), reciprocal of the
                        # denominators (DVE), per-block scale into the
                        # shared pre-transpose tile (Pool).
                        xxm = sp.tile([128, 264], F32, name="xxm", tag="xxm")
                        nc.scalar.activation(xxm[:], avt[:], AF.Relu)
                        rden = sp.tile([128, 8], F32, name="rden", tag="rden")
                        nc.vector.reciprocal(
                            rden[:],
                            xxm[:].rearrange("p (g c) -> p g c", c=33)[:, :, 32])
                        if pr == 0:
                            xnm_tiles[x] = sp.tile([128, 512], BF16,
                                                   name="xnm", tag="xnm")
                        xnm_x = xnm_tiles[x]
                        if gi == len(GROUPS) - 1:
                            # tail: DVE is idle; one strided op beats the
                            # serial Pool-launch chain
                            nc.vector.scalar_tensor_tensor(
                                xnm_x[:].rearrange(
                                    "p (s v u c) -> p s v u c",
                                    v=2, u=2, c=32)[:, :, pr, :, :],
                                xxm[:].rearrange(
                                    "p (s u c) -> p s u c",
                                    u=2, c=33)[:, :, :, 0:32],
                                1.0,
                                rden[:].rearrange("p (s u) -> p s u", u=2)
                                    .unsqueeze(3).broadcast_to([128, 4, 2, 32]),
                                ALU.mult, ALU.mult)
                        else:
                            for g in range(8):      # g = 2*s + j2
                                s, j2 = g // 2, g % 2
                                c0 = 128 * s + 64 * pr + 32 * j2
                                nc.gpsimd.tensor_scalar_mul(
                                    xnm_x[:, c0:c0 + 32],
                                    xxm[:, 33 * g:33 * g + 32],
                                    rden[:, g:g + 1])
                    return epilogue

                for mt in range(NMT):
                    for fn in hooks.get((gi, mt), []):
                        fn()
                    if mt == 6:
                        # whole-bank start=True clear of this group's AV
                        # accumulator; deferred here so it lands after the
                        # previous group's epilogue (relu) was emitted —
                        # avp has a single zone. The AV matmuls use
                        # start=False (first per-element write overwrites,
                        # later ones accumulate).
                        nc.tensor.matmul(avt[:], zrow[:], vto[0:1, 0:264],
                                         start=True, stop=False)
                    # deeper flush threshold early in the group: the first
                    # own-AV flush (needs the cleared accumulator) waits
                    # until mt=8, well past the clear
                    limit = 7 if mt < 8 else LAG
                    while len(pend) > limit:
                        flush_one()
                    sc = scp.tile([128, 1024], F32, name="sc", tag="w")
                    for j2 in range(2):
                        j = 2 * pr + j2
                        nc.tensor.matmul(
                            sc[:, 512 * j2:512 * (j2 + 1)],
                            kk[x][32 * j:32 * j + KD, 128 * mt:128 * (mt + 1)],
                            qq[x][32 * j:32 * j + KD, q0:q0 + 512],
                            start=True, stop=True,
                            tile_position=(32 * j, 0))
                    et = ep.tile([128, 1024], BF16, name="et", tag="et")
                    emit_exp(et, sc, mt)
                    post = make_epilogue(avt, gi, qc, x, pr) if mt == NMT - 1 else None
                    pend.append((make_av(avt, x, pr, mt, et), post))

            # tail: flush remaining AVs (fires the last epilogue), then
            # the final transposes + output projection
            while pend:
                flush_one()
            transposes(1)
            outproj(1)

            for ct in range(2):
                nc.sync.dma_start(out[128 * ct:128 * (ct + 1), 512:1024],
                                  outb[ct][:, 512:1024])
            for _p in (sp, ep, psv, avp, scp):
                _p.release()

    # walrus codegen accepts only ONE sync wait on compute instructions
    # (Matmult / Activation / TensorTensor / ...). The multi-wait cases
    # Tile emits here are all {self-engine, other}: a self-engine wait
    # orders an instruction against an earlier instruction on the SAME
    # in-order engine (WAW through PE's single PSUM write port, ACT/DVE
    # pipeline order), which the hardware already guarantees — drop it.
    _self_prefix = {
        "EngineType.PE": "PE",
        "EngineType.Activation": "Activation",
        "EngineType.DVE": "DVE",
        "EngineType.Pool": "Pool",
        "EngineType.SP": "SP",
    }
    for f in nc.m.functions:
        for bb in f.blocks:
            for inst in bb.instructions:
                si = inst.sync_info
                if si is None or not si.on_wait or len(si.on_wait) < 2:
                    continue
                pref = _self_prefix.get(str(getattr(inst, "engine", "")), None)
                if pref is None:
                    continue
                kept = [w for w in si.on_wait
                        if not str(w.ant_name).startswith(pref)]
                if not kept or len(kept) == len(si.on_wait):
                    continue
                si.on_wait = kept

    # Safety net: any instruction still carrying >1 wait gets all but its
    # last wait hoisted into preceding same-engine NoOps (1 wait each).
    uid = [0]
    for f in nc.m.functions:
        for bb in f.blocks:
            new_insts = []
            for inst in bb.instructions:
                si = inst.sync_info
                if si is not None and si.on_wait and len(si.on_wait) > 1:
                    for w in si.on_wait[:-1]:
                        uid[0] += 1
                        nop = mybir.InstNoOp(
                            name=f"I-waitsplit-{uid[0]}", ins=[], outs=[])
                        nop.engine = inst.engine
                        nop.sync_info = mybir.SyncInfo(
                            on_wait=[w], on_update=[])
                        new_insts.append(nop)
                    si.on_wait = [si.on_wait[-1]]
                new_insts.append(inst)
            bb.instructions = new_insts
    return nc


_CACHE = {}


def _prep_host(inputs):
    """Fold BN into weights; build head-split layouts shared by all cores."""
    import ml_dtypes
    f = np.float32
    Wq = (inputs["Wq"] * inputs["sq"][:, None]).astype(f)
    Wk = (inputs["Wk"] * inputs["sk"][:, None]).astype(f)
    Wv = (inputs["Wv"] * inputs["sv"][:, None]).astype(f)
    Wp = (inputs["Wp"] * inputs["sp"][:, None]).astype(f)

    def split(Wt, b):
        o = []
        for g in range(2):
            Wx = np.zeros((C, 128), f)
            bx = np.zeros((128, 1), f)
            for j in range(4):
                h = 4 * g + j
                Wx[:, 32 * j:32 * j + KD] = Wt[:, KD * h:KD * (h + 1)]
                bx[32 * j:32 * j + KD, 0] = b[KD * h:KD * (h + 1)]
            o.append((np.ascontiguousarray(Wx), bx))
        return o

    (wqA, bqA), (wqB, bqB) = split(Wq.T.astype(f), inputs["bq"])
    (wkA, bkA), (wkB, bkB) = split(Wk.T.astype(f), inputs["bk"])
    WvT = Wv.T.astype(f)                      # [C, 256] cols (h, d)
    wv_ext = np.zeros((C, 264), f)            # col 33h+32 stays 0
    bv_ext = np.zeros((264,), f)
    for h in range(NUM_HEADS):
        wv_ext[:, HB * h:HB * h + 32] = WvT[:, 32 * h:32 * (h + 1)]
        bv_ext[HB * h:HB * h + 32] = inputs["bv"][32 * h:32 * (h + 1)]
        bv_ext[HB * h + 32] = 1.0             # softmax denominator column
    ident_bf16 = np.eye(128, dtype=ml_dtypes.bfloat16)
    return dict(
        w_qA=wqA, w_qB=wqB, w_kA=wkA, w_kB=wkB,
        w_v=wv_ext, w_p=np.ascontiguousarray(Wp.T).astype(ml_dtypes.bfloat16),
        b_qA=bqA, b_qB=bqB, b_kA=bkA, b_kB=bkB,
        b_v=np.ascontiguousarray(np.broadcast_to(bv_ext, (128, 264))),
        b_p=inputs["bp"].astype(f).reshape(C, 1),
        ident=ident_bf16,
    )


def kernel(**inputs) -> np.ndarray:
    inputs = {k: np.asarray(v) for k, v in inputs.items()}
    if "nc" not in _CACHE:
        _CACHE["nc"] = build_nc()
    nc = _CACHE["nc"]

    shared = _prep_host(inputs)
    rgb = np.ascontiguousarray(inputs["rgb"].astype(np.float32).reshape(B, C, N))
    edge = np.ascontiguousarray(inputs["edge"].astype(np.float32).reshape(B, C, N))

    in_maps = []
    for core in range(8):
        b, qs = core // 4, core % 4
        m = dict(shared)
        m["rgb_s"] = np.ascontiguousarray(rgb[b][:, QCH * qs:QCH * (qs + 1)])
        m["edge"] = edge[b]
        in_maps.append(m)

    res = run_bass_kernel_spmd(nc, in_maps, core_ids=list(range(8)))
    full = np.zeros((B, C, N), np.float32)
    for core in range(8):
        b, qs = core // 4, core % 4
        full[b][:, QCH * qs:QCH * (qs + 1)] = res.results[core]["out"]
    return full.reshape(B, C, H, W)


# revision 20
# speedup vs baseline: 1.5553x; 1.0227x over previous
"""Cross-modal attention Trainium2 kernel.

Reference computation (all 1x1 convs + folded eval-mode BN):
  q = BN(Wq @ rgb), k = BN(Wk @ edge), v = BN(Wv @ edge)
  attn = softmax(q^T k) per head; xx = relu(attn @ v); out = BN(Wp @ xx)

Shapes: B=2, C=256, H=W=64 (N=4096), heads=8, key_dim=16, d=32.

Sharding: 8 cores = (batch b in {0,1}) x (query-slice qs in {0..3}, 1024
queries each). Each core computes K/V projections for the full N of its
batch (cheap) and attention + output projection for its query slice; the
host concatenates slices. No collectives.

Per-core dataflow (scores kept transposed so softmax-sum and the AV
contraction both run on the m axis without transposing the big matrix):
  sT[m, q] = sum_kd kk[kd, m] qq[kd, q]     (PE, fp32r, 32x128 row-tiled)
  e = exp(sT)        split across two engines, balanced ~53/47:
      ACT:  exp activation (PSUM->SBUF bf16)
      DVE:  Schraudolph bitwise exp: bf16_bits(int16(s*184.665 + 16248.6))
            = 2^(s*log2 e) with ~+-3% multiplicative error that cancels in
            softmax normalization (max |s| ~= 45, safe window (-88, +89)).
  av[q, (h: d|den)] += e[mtile]^T @ [v|1]   (PE bf16, exp as stationary)
  xx = relu(av), xnm = xx * recip(den)      (ACT relu; Pool per-head scale)
  out = Wp^T @ xxt + bp                     (PE bf16 transpose, DMA PSUM->SBUF
                                             move, matmul, ACT bias add)

Engine budget per core (TimelineSim): PE ~163us (QK^T 109 + AV 30 + proj/
misc 24), ACT ~158us (exp share + bias adds + relu), DVE ~158us (exp share
+ v moves), Pool/SP/DMA light. Baseline (all exp on ACT) was 315us.
"""

import sys

for p in ("/opt/trn_rl_repo", "/opt/trn_rl_repo/concourse"):
    if p not in sys.path:
        sys.path.insert(0, p)

import numpy as np

import concourse.bass as bass
import concourse.mybir as mybir
import concourse.tile as tile
from concourse.bass_utils import run_bass_kernel_spmd

F32 = mybir.dt.float32
F32R = mybir.dt.float32r
BF16 = mybir.dt.bfloat16
I16 = mybir.dt.int16
AF = mybir.ActivationFunctionType
ALU = mybir.AluOpType

NUM_HEADS, KD, D = 8, 16, 32
B, C, H, W = 2, 256, 64, 64
N = H * W            # 4096 keys per batch
QCH = 1024           # queries per core
NMT = N // 128       # 32 m-tiles
HB = 33              # per-head AV block: 32 v-cols + 1 denominator col

# Schraudolph exp: bf16_bits(int16(s*SCH_C1 + SCH_C2)) ~= e^s.
SCH_C1 = 184.66496580927726     # 128 * log2(e)
SCH_C2 = 16248.6                # 127*128 minus mean-zeroing interp bias
ACT_FRAC = 0.5266               # share of exp tiles on ACT (rest on DVE)


def build_nc(act_frac=ACT_FRAC):
    nc = bass.Bass()

    rgb_s = nc.dram_tensor("rgb_s", [C, QCH], F32R, kind="ExternalInput")
    edge = nc.dram_tensor("edge", [C, N], F32R, kind="ExternalInput")
    w_qA = nc.dram_tensor("w_qA", [C, 128], F32R, kind="ExternalInput")
    w_qB = nc.dram_tensor("w_qB", [C, 128], F32R, kind="ExternalInput")
    w_kA = nc.dram_tensor("w_kA", [C, 128], F32R, kind="ExternalInput")
    w_kB = nc.dram_tensor("w_kB", [C, 128], F32R, kind="ExternalInput")
    w_v = nc.dram_tensor("w_v", [C, 256], F32R, kind="ExternalInput")
    w_p = nc.dram_tensor("w_p", [256, C], BF16, kind="ExternalInput")
    b_qA = nc.dram_tensor("b_qA", [128, 1], F32, kind="ExternalInput")
    b_qB = nc.dram_tensor("b_qB", [128, 1], F32, kind="ExternalInput")
    b_kA = nc.dram_tensor("b_kA", [128, 1], F32, kind="ExternalInput")
    b_kB = nc.dram_tensor("b_kB", [128, 1], F32, kind="ExternalInput")
    b_v = nc.dram_tensor("b_v", [128, 256], F32, kind="ExternalInput")
    b_p = nc.dram_tensor("b_p", [C, 1], F32, kind="ExternalInput")
    ident = nc.dram_tensor("ident", [128, 128], BF16, kind="ExternalInput")
    out = nc.dram_tensor("out", [C, QCH], F32, kind="ExternalOutput")

    with tile.TileContext(nc) as tc:
        with tc.tile_pool(name="const", bufs=1) as cp, \
             tc.tile_pool(name="data", bufs=1) as dp:
            wq = [cp.tile([128, 256], F32R, name=f"wq{x}", tag=f"wq{x}") for x in "AB"]
            wk = [cp.tile([128, 256], F32R, name=f"wk{x}", tag=f"wk{x}") for x in "AB"]
            wv = cp.tile([128, 512], F32R, name="wv", tag="wv")
            wp = cp.tile([128, 512], BF16, name="wp", tag="wp")
            bq = [cp.tile([128, 1], F32, name=f"bq{x}", tag=f"bq{x}") for x in "AB"]
            bk = [cp.tile([128, 1], F32, name=f"bk{x}", tag=f"bk{x}") for x in "AB"]
            bv = cp.tile([128, 256], F32, name="bv", tag="bv")
            bp = cp.tile([128, 2], F32, name="bp", tag="bp")
            idn = cp.tile([128, 128], BF16, name="idn", tag="idn")
            zrow = cp.tile([1, 128], BF16, name="zrow", tag="zrow")

            nc.sync.dma_start(bk[0][:], b_kA[:])
            nc.sync.dma_start(bq[0][:], b_qA[:])
            nc.vector.memset(zrow[:], 0.0)

            rgb_sb = dp.tile([128, 2 * QCH], F32R, name="rgb_sb", tag="rgb")
            edge_sb = [dp.tile([128, N], F32R, name=f"edge{k}", tag=f"edge{k}") for k in range(2)]

            # Direct DMA into the f32r tiles (f32r bits == f32; the HW
            # matmul rounds on read). Inputs are chunked and ordered so the
            # first QKT can start ~6us in: kk[0]-chunk-0 + qq[0]-qc0 deps
            # first, then pieces in first-use order. The projections
            # themselves are interleaved into the first attention groups.
            for k in range(2):
                nc.sync.dma_start(wk[0][:, 128 * k:128 * (k + 1)], w_kA[128 * k:128 * (k + 1), :])
                nc.sync.dma_start(wq[0][:, 128 * k:128 * (k + 1)], w_qA[128 * k:128 * (k + 1), :])
            for k in range(2):      # first edge columns (kk chunk 0 dep)
                nc.sync.dma_start(edge_sb[k][:, 0:512], edge[128 * k:128 * (k + 1), 0:512])
            for k in range(2):      # rgb first halves (qc=0 queries)
                nc.sync.dma_start(rgb_sb[:, QCH * k:QCH * k + 512], rgb_s[128 * k:128 * (k + 1), 0:512])
            for k in range(2):
                nc.sync.dma_start(edge_sb[k][:, 512:1024], edge[128 * k:128 * (k + 1), 512:1024])
            for k in range(2):
                nc.sync.dma_start(wv[:, 256 * k:256 * (k + 1)], w_v[128 * k:128 * (k + 1), :])
            nc.sync.dma_start(bv[:], b_v[:])
            nc.sync.dma_start(bq[1][:], b_qB[:])
            nc.sync.dma_start(bk[1][:], b_kB[:])
            nc.sync.dma_start(bp[:, 0:1], b_p[0:128, :])
            nc.sync.dma_start(bp[:, 1:2], b_p[128:256, :])
            for k in range(2):
                nc.sync.dma_start(edge_sb[k][:, 1024:2048], edge[128 * k:128 * (k + 1), 1024:2048])
            for k in range(2):
                nc.sync.dma_start(rgb_sb[:, QCH * k + 512:QCH * (k + 1)], rgb_s[128 * k:128 * (k + 1), 512:1024])
            for k in range(2):
                nc.sync.dma_start(wk[1][:, 128 * k:128 * (k + 1)], w_kB[128 * k:128 * (k + 1), :])
                nc.sync.dma_start(wq[1][:, 128 * k:128 * (k + 1)], w_qB[128 * k:128 * (k + 1), :])
            for p in range(2, 4):
                for k in range(2):
                    nc.sync.dma_start(edge_sb[k][:, 1024 * p:1024 * (p + 1)],
                                      edge[128 * k:128 * (k + 1), 1024 * p:1024 * (p + 1)])
            for k in range(2):
                nc.sync.dma_start(wp[:, 256 * k:256 * (k + 1)], w_p[128 * k:128 * (k + 1), :])
            nc.sync.dma_start(idn[:], ident[:])

            qq = [dp.tile([128, QCH], F32R, name=f"qq{x}", tag=f"qq{x}") for x in "AB"]
            kk = [dp.tile([128, N], F32R, name=f"kk{x}", tag=f"kk{x}") for x in "AB"]
            vto = dp.tile([128, NMT * 8 * HB], BF16, name="vto", tag="vto")
            # denominator columns (33rd of each head block) are constant 1.0
            nc.vector.memset(
                vto[:].rearrange("p (m c) -> p m c", c=HB)[:, :, 32], 1.0)
            outb = [dp.tile([128, QCH], F32, name=f"outb{k}", tag=f"outb{k}") for k in range(2)]
            xxt = [dp.tile([128, 512], BF16, name=f"xxt{k}", tag=f"xxt{k}") for k in range(2)]

            # ---- pools (PSUM pools span the whole kernel: releasing a
            # PSUM pool makes successor tiles inherit released-zone deps
            # as multi-wait instructions, which walrus rejects) ----
            # PSUM budget (8 banks): scp 3 x [128,1024]f32 (2 banks each)
            # for a 3-deep QKT->exp score pipeline; avp 2 x [128,264]f32
            # (1 bank) for the per-(x,pr) AV accumulator, s-chunks packed
            # along columns. Projection / transpose / out-proj psums
            # borrow scp zones transiently.
            scp = tc.alloc_tile_pool(name="scp", bufs=3, space="PSUM")
            avp = tc.alloc_tile_pool(name="avp", bufs=1, space="PSUM")
            psv = tc.alloc_tile_pool(name="psv", bufs=1, space="PSUM")
            # single fixed PSUM bank shared (via disjoint regions /
            # lifetimes) by: double-buffered vto projection slots (gi0),
            # later the kk/qq projection psums, out-proj psums and
            # transpose outputs — keeping the 3 sc zones churn-free.
            fixt = psv.tile([128, 512], F32, name="fixt", tag="v")
            ep = tc.alloc_tile_pool(name="exp", bufs=12)
            sp = tc.alloc_tile_pool(name="stg", bufs=6)

            # ---- fused projection + attention stream ----
            # All work is emitted as ONE software-pipelined mtile stream
            # across the 8 (qc, x, pr) head-pair groups. Projections ride
            # as per-mtile hooks inside the first two groups (hidden under
            # the attention steady state); AV consumption lags QKT/exp by
            # LAG mtiles (across group boundaries too) so PE never waits
            # on the exp engines; per-group epilogues (relu/recip/scale)
            # fire when the group's last AV flushes, and their PE parts
            # (transposes, output projection) are deferred hooks inside
            # later groups' streams.

            # ACT warmup: absorb the DVE tick (activation-bias const
            # tiles are DVE-written) so the first exp carries only the PE
            # wait — walrus allows a single wait per Activation.
            actw = dp.tile([1, 1], BF16, name="actw", tag="actw")
            nc.scalar.activation(actw[:], zrow[0:1, 0:1], AF.Exp)

            def proj_qq(x, j, ps=None):
                ps = ps if ps is not None else scp.tile([128, 512], F32, name="ps_q", tag="w")
                for k in range(2):
                    nc.tensor.matmul(
                        ps[:], wq[x][:, 128 * k:128 * (k + 1)],
                        rgb_sb[:, QCH * k + 512 * j:QCH * k + 512 * (j + 1)],
                        start=(k == 0), stop=(k == 1))
                nc.scalar.activation(
                    qq[x][:, 512 * j:512 * (j + 1)], ps[:],
                    AF.Identity, bias=bq[x][:])

            def proj_kk(x, j, ps=None):
                ps = ps if ps is not None else scp.tile([128, 512], F32, name="ps_k", tag="w")
                for k in range(2):
                    nc.tensor.matmul(
                        ps[:], wk[x][:, 128 * k:128 * (k + 1)],
                        edge_sb[k][:, 512 * j:512 * (j + 1)],
                        start=(k == 0), stop=(k == 1))
                nc.scalar.activation(
                    kk[x][:, 512 * j:512 * (j + 1)], ps[:],
                    AF.Identity, bias=bk[x][:])

            def proj_vto(mt):
                # v^T without denominator columns: ps slot [128, 256],
                # double-buffered inside the fixed bank; the DVE bias-add
                # scatters into vto's 33-stride head blocks (den columns
                # are memset to 1.0 once at start).
                ps = fixt[:, 256 * (mt % 2):256 * (mt % 2) + 256]
                for k in range(2):
                    nc.tensor.matmul(
                        ps, edge_sb[k][:, 128 * mt:128 * (mt + 1)],
                        wv[:, 256 * k:256 * (k + 1)],
                        start=(k == 0), stop=(k == 1))
                nc.vector.tensor_add(
                    vto[:, 8 * HB * mt:8 * HB * (mt + 1)].rearrange(
                        "p (h c) -> p h c", c=HB)[:, :, 0:32],
                    ps.rearrange("p (h c) -> p h c", c=32), bv[:])

            def transposes(x):
                xnm_x = xnm_tiles[x]
                for s in range(4):
                    tp = fixt[:, 64 * s:64 * (s + 1)].bitcast(BF16)
                    nc.tensor.transpose(tp, xnm_x[:, 128 * s:128 * (s + 1)], idn[:])
                    nc.vector.tensor_copy(xxt[x][:, 128 * s:128 * (s + 1)], tp)

            def outproj(qc):
                q0 = 512 * qc
                for ct in range(2):
                    ps = fixt[:]
                    for k in range(2):
                        nc.tensor.matmul(
                            ps, wp[:, 256 * k + 128 * ct:256 * k + 128 * (ct + 1)],
                            xxt[k][:], start=(k == 0), stop=(k == 1))
                    nc.scalar.activation(
                        outb[ct][:, q0:q0 + 512], ps,
                        AF.Identity, bias=bp[:, ct:ct + 1])

            # per-mtile PE-stream hooks: {(gi, mt): [fn, ...]}
            hooks = {}

            def add_hook(gi, mt, fn):
                hooks.setdefault((gi, mt), []).append(fn)

            add_hook(0, 0, lambda: proj_kk(0, 0))
            add_hook(0, 0, lambda: proj_qq(0, 0))
            for mt in range(NMT):           # vto(mt) feeds AV(mt), LAG behind
                add_hook(0, mt, lambda mt=mt: proj_vto(mt))
            for c in range(1, 8):           # kk[0] chunk c feeds QKT(mt>=4c)
                add_hook(0, 4 * c - 2, lambda c=c: proj_kk(0, c))
            add_hook(1, 24, lambda: proj_qq(1, 0, fixt[:]))
            add_hook(3, 10, lambda: proj_qq(0, 1, fixt[:]))
            add_hook(5, 14, lambda: proj_qq(1, 1, fixt[:]))
            # kk[1] feeds the x=1 groups: chunk 0 late in gi1, the rest
            # just-in-time inside gi2 (chunk c consumed from mt=4c); all
            # use the fixed scratch bank (vto is finished by then)
            add_hook(1, 28, lambda: proj_kk(1, 0, fixt[:]))
            for c in range(1, 8):
                add_hook(2, 4 * c - 2, lambda c=c: proj_kk(1, c, fixt[:]))

            # Exp dispatcher: fixed per-group A/D pattern (ACT exact exp
            # vs DVE bitwise 2^x). Strict alternation avoids queue jitter;
            # the extra ACT tiles (DVE is ~15% slower per tile and carries
            # the vto adds early on) are pinned at mt 0/16 where the PE
            # stream has hook/flush slack.
            PAT_STEADY = ["A" if (mt % 2 == 0 or mt == 9) else "D"
                          for mt in range(NMT)]          # 17A / 15D
            PAT_FIRST = ["A" if (mt % 2 == 0 or mt in (3, 9, 17, 25, 29)) else "D"
                         for mt in range(NMT)]           # 21A / 11D
            GROUP_PATS = [PAT_FIRST, PAT_STEADY] + [PAT_STEADY] * 6
            cur_pat = [PAT_STEADY]

            def emit_exp(et, sc, mt):
                if cur_pat[0][mt] == "A":
                    nc.scalar.activation(et[:], sc[:], AF.Exp)
                else:
                    nc.vector.tensor_scalar(
                        et[:].bitcast(I16), sc[:], SCH_C1, SCH_C2,
                        ALU.mult, ALU.add)

            GROUPS = [(qc, x, pr) for qc in (0, 1) for x in (0, 1) for pr in (0, 1)]
            # deferred PE-side epilogue hooks: group gi's xnm tile (x done
            # at odd gi) is transposed inside group gi+1's stream; the
            # output projection of qc=0 rides in group 4 (qc=1 x=0 pr=0).
            add_hook(2, 10, lambda: transposes(0))
            add_hook(4, 10, lambda: transposes(1))
            add_hook(4, 16, lambda: outproj(0))
            add_hook(6, 10, lambda: transposes(0))

            def out_dma_qc0():
                for ct in range(2):
                    nc.sync.dma_start(out[128 * ct:128 * (ct + 1), 0:512],
                                      outb[ct][:, 0:512])
            add_hook(5, 8, out_dma_qc0)
            LAG = 3
            xnm_tiles = [None, None]
            pend = []       # (emit_av_fn, post_fn_or_None)

            def flush_one():
                fn, post = pend.pop(0)
                fn()
                if post is not None:
                    post()

            for gi, (qc, x, pr) in enumerate(GROUPS):
                q0 = 512 * qc
                cur_pat[0] = GROUP_PATS[gi]
                avt = avp.tile([128, 264], F32, name="av", tag="av")

                def make_av(avt, x, pr, mt, et):
                    def emit_av():
                        for j2 in range(2):
                            h = 4 * x + 2 * pr + j2
                            for s in range(4):
                                nc.tensor.matmul(
                                    avt[:, 66 * s + 33 * j2:66 * s + 33 * j2 + 33],
                                    et[:, 512 * j2 + 128 * s:512 * j2 + 128 * (s + 1)],
                                    vto[:, 8 * HB * mt + HB * h:8 * HB * mt + HB * (h + 1)],
                                    start=False, stop=(mt == NMT - 1))
                    return emit_av

                def make_epilogue(avt, gi, qc, x, pr):
                    def epilogue():
                        # relu (ACT, PSUM->SBUF), reciprocal of the
                        # denominators (DVE), per-block scale into the
                        # shared pre-transpose tile (Pool).
                        xxm = sp.tile([128, 264], F32, name="xxm", tag="xxm")
                        nc.scalar.activation(xxm[:], avt[:], AF.Relu)
                        rden = sp.tile([128, 8], F32, name="rden", tag="rden")
                        nc.vector.reciprocal(
                            rden[:],
                            xxm[:].rearrange("p (g c) -> p g c", c=33)[:, :, 32])
                        if pr == 0:
                            xnm_tiles[x] = sp.tile([128, 512], BF16,
                                                   name="xnm", tag="xnm")
                        xnm_x = xnm_tiles[x]
                        if gi == len(GROUPS) - 1:
                            # tail: DVE is idle; one strided op beats the
                            # serial Pool-launch chain
                            nc.vector.scalar_tensor_tensor(
                                xnm_x[:].rearrange(
                                    "p (s v u c) -> p s v u c",
                                    v=2, u=2, c=32)[:, :, pr, :, :],
                                xxm[:].rearrange(
                                    "p (s u c) -> p s u c",
                                    u=2, c=33)[:, :, :, 0:32],
                                1.0,
                                rden[:].rearrange("p (s u) -> p s u", u=2)
                                    .unsqueeze(3).broadcast_to([128, 4, 2, 32]),
                                ALU.mult, ALU.mult)
                        else:
                            for g in range(8):      # g = 2*s + j2
                                s, j2 = g // 2, g % 2
                                c0 = 128 * s + 64 * pr + 32 * j2
                                nc.gpsimd.tensor_scalar_mul(
                                    xnm_x[:, c0:c0 + 32],
                                    xxm[:, 33 * g:33 * g + 32],
                                    rden[:, g:g + 1])
                    return epilogue

                for mt in range(NMT):
                    for fn in hooks.get((gi, mt), []):
                        fn()
                    if mt == 6:
                        # whole-bank start=True clear of this group's AV
                        # accumulator; deferred here so it lands after the
                        # previous group's epilogue (relu) was emitted —
                        # avp has a single zone. The AV matmuls use
                        # start=False (first per-element write overwrites,
                        # later ones accumulate).
                        nc.tensor.matmul(avt[:], zrow[:], vto[0:1, 0:264],
                                         start=True, stop=False)
                    # deeper flush threshold early in the group: the first
                    # own-AV flush (needs the cleared accumulator) waits
                    # until mt=8, well past the clear
                    limit = 7 if mt < 8 else LAG
                    while len(pend) > limit:
                        flush_one()
                    sc = scp.tile([128, 1024], F32, name="sc", tag="w")
                    for j2 in range(2):
                        j = 2 * pr + j2
                        nc.tensor.matmul(
                            sc[:, 512 * j2:512 * (j2 + 1)],
                            kk[x][32 * j:32 * j + KD, 128 * mt:128 * (mt + 1)],
                            qq[x][32 * j:32 * j + KD, q0:q0 + 512],
                            start=True, stop=True,
                            tile_position=(32 * j, 0))
                    et = ep.tile([128, 1024], BF16, name="et", tag="et")
                    emit_exp(et, sc, mt)
                    post = make_epilogue(avt, gi, qc, x, pr) if mt == NMT - 1 else None
                    pend.append((make_av(avt, x, pr, mt, et), post))

            # tail: flush remaining AVs (fires the last epilogue), then
            # the final transposes + output projection
            while pend:
                flush_one()
            transposes(1)
            outproj(1)

            for ct in range(2):
                nc.sync.dma_start(out[128 * ct:128 * (ct + 1), 512:1024],
                                  outb[ct][:, 512:1024])
            for _p in (sp, ep, psv, avp, scp):
                _p.release()

    # walrus codegen accepts only ONE sync wait on compute instructions
    # (Matmult / Activation / TensorTensor / ...). The multi-wait cases
    # Tile emits here are all {self-engine, other}: a self-engine wait
    # orders an instruction against an earlier instruction on the SAME
    # in-order engine (WAW through PE's single PSUM write port, ACT/DVE
    # pipeline order), which the hardware already guarantees — drop it.
    _self_prefix = {
        "EngineType.PE": "PE",
        "EngineType.Activation": "Activation",
        "EngineType.DVE": "DVE",
        "EngineType.Pool": "Pool",
        "EngineType.SP": "SP",
    }
    for f in nc.m.functions:
        for bb in f.blocks:
            for inst in bb.instructions:
                si = inst.sync_info
                if si is None or not si.on_wait or len(si.on_wait) < 2:
                    continue
                pref = _self_prefix.get(str(getattr(inst, "engine", "")), None)
                if pref is None:
                    continue
                kept = [w for w in si.on_wait
                        if not str(w.ant_name).startswith(pref)]
                if not kept or len(kept) == len(si.on_wait):
                    continue
                si.on_wait = kept

    # Safety net: any instruction still carrying >1 wait gets all but its
    # last wait hoisted into preceding same-engine NoOps (1 wait each).
    uid = [0]
    for f in nc.m.functions:
        for bb in f.blocks:
            new_insts = []
            for inst in bb.instructions:
                si = inst.sync_info
                if si is not None and si.on_wait and len(si.on_wait) > 1:
                    for w in si.on_wait[:-1]:
                        uid[0] += 1
                        nop = mybir.InstNoOp(
                            name=f"I-waitsplit-{uid[0]}", ins=[], outs=[])
                        nop.engine = inst.engine
                        nop.sync_info = mybir.SyncInfo(
                            on_wait=[w], on_update=[])
                        new_insts.append(nop)
                    si.on_wait = [si.on_wait[-1]]
                new_insts.append(inst)
            bb.instructions = new_insts
    return nc


_CACHE = {}


def _prep_host(inputs):
    """Fold BN into weights; build head-split layouts shared by all cores."""
    import ml_dtypes
    f = np.float32
    Wq = (inputs["Wq"] * inputs["sq"][:, None]).astype(f)
    Wk = (inputs["Wk"] * inputs["sk"][:, None]).astype(f)
    Wv = (inputs["Wv"] * inputs["sv"][:, None]).astype(f)
    Wp = (inputs["Wp"] * inputs["sp"][:, None]).astype(f)

    def split(Wt, b):
        o = []
        for g in range(2):
            Wx = np.zeros((C, 128), f)
            bx = np.zeros((128, 1), f)
            for j in range(4):
                h = 4 * g + j
                Wx[:, 32 * j:32 * j + KD] = Wt[:, KD * h:KD * (h + 1)]
                bx[32 * j:32 * j + KD, 0] = b[KD * h:KD * (h + 1)]
            o.append((np.ascontiguousarray(Wx), bx))
        return o

    (wqA, bqA), (wqB, bqB) = split(Wq.T.astype(f), inputs["bq"])
    (wkA, bkA), (wkB, bkB) = split(Wk.T.astype(f), inputs["bk"])
    WvT = Wv.T.astype(f)                      # [C, 256] cols (h, d)
    ident_bf16 = np.eye(128, dtype=ml_dtypes.bfloat16)
    return dict(
        w_qA=wqA, w_qB=wqB, w_kA=wkA, w_kB=wkB,
        w_v=np.ascontiguousarray(WvT),
        w_p=np.ascontiguousarray(Wp.T).astype(ml_dtypes.bfloat16),
        b_qA=bqA, b_qB=bqB, b_kA=bkA, b_kB=bkB,
        b_v=np.ascontiguousarray(np.broadcast_to(inputs["bv"].astype(f), (128, 256))),
        b_p=inputs["bp"].astype(f).reshape(C, 1),
        ident=ident_bf16,
    )


def kernel(**inputs) -> np.ndarray:
    inputs = {k: np.asarray(v) for k, v in inputs.items()}
    if "nc" not in _CACHE:
        _CACHE["nc"] = build_nc()
    nc = _CACHE["nc"]

    shared = _prep_host(inputs)
    rgb = np.ascontiguousarray(inputs["rgb"].astype(np.float32).reshape(B, C, N))
    edge = np.ascontiguousarray(inputs["edge"].astype(np.float32).reshape(B, C, N))

    in_maps = []
    for core in range(8):
        b, qs = core // 4, core % 4
        m = dict(shared)
        m["rgb_s"] = np.ascontiguousarray(rgb[b][:, QCH * qs:QCH * (qs + 1)])
        m["edge"] = edge[b]
        in_maps.append(m)

    res = run_bass_kernel_spmd(nc, in_maps, core_ids=list(range(8)))
    full = np.zeros((B, C, N), np.float32)
    for core in range(8):
        b, qs = core // 4, core % 4
        full[b][:, QCH * qs:QCH * (qs + 1)] = res.results[core]["out"]
    return full.reshape(B, C, H, W)


# revision 22
# speedup vs baseline: 1.5774x; 1.0142x over previous
"""Cross-modal attention Trainium2 kernel.

Reference computation (all 1x1 convs + folded eval-mode BN):
  q = BN(Wq @ rgb), k = BN(Wk @ edge), v = BN(Wv @ edge)
  attn = softmax(q^T k) per head; xx = relu(attn @ v); out = BN(Wp @ xx)

Shapes: B=2, C=256, H=W=64 (N=4096), heads=8, key_dim=16, d=32.

Sharding: 8 cores = (batch b in {0,1}) x (query-slice qs in {0..3}, 1024
queries each). Each core computes K/V projections for the full N of its
batch (cheap) and attention + output projection for its query slice; the
host concatenates slices. No collectives.

Per-core dataflow (scores kept transposed so softmax-sum and the AV
contraction both run on the m axis without transposing the big matrix):
  sT[m, q] = sum_kd kk[kd, m] qq[kd, q]     (PE, fp32r, 32x128 row-tiled)
  e = exp(sT)        split across two engines, balanced ~53/47:
      ACT:  exp activation (PSUM->SBUF bf16)
      DVE:  Schraudolph bitwise exp: bf16_bits(int16(s*184.665 + 16248.6))
            = 2^(s*log2 e) with ~+-3% multiplicative error that cancels in
            softmax normalization (max |s| ~= 45, safe window (-88, +89)).
  av[q, (h: d|den)] += e[mtile]^T @ [v|1]   (PE bf16, exp as stationary)
  xx = relu(av), xnm = xx * recip(den)      (ACT relu; Pool per-head scale)
  out = Wp^T @ xxt + bp                     (PE bf16 transpose, DMA PSUM->SBUF
                                             move, matmul, ACT bias add)

Engine budget per core (TimelineSim): PE ~163us (QK^T 109 + AV 30 + proj/
misc 24), ACT ~158us (exp share + bias adds + relu), DVE ~158us (exp share
+ v moves), Pool/SP/DMA light. Baseline (all exp on ACT) was 315us.
"""

import sys

for p in ("/opt/trn_rl_repo", "/opt/trn_rl_repo/concourse"):
    if p not in sys.path:
        sys.path.insert(0, p)

import numpy as np

import concourse.bass as bass
import concourse.mybir as mybir
import concourse.tile as tile
from concourse.bass_utils import run_bass_kernel_spmd

F32 = mybir.dt.float32
F32R = mybir.dt.float32r
BF16 = mybir.dt.bfloat16
I16 = mybir.dt.int16
AF = mybir.ActivationFunctionType
ALU = mybir.AluOpType

NUM_HEADS, KD, D = 8, 16, 32
B, C, H, W = 2, 256, 64, 64
N = H * W            # 4096 keys per batch
QCH = 1024           # queries per core
NMT = N // 128       # 32 m-tiles
HB = 33              # per-head AV block: 32 v-cols + 1 denominator col

# Schraudolph exp: bf16_bits(int16(s*SCH_C1 + SCH_C2)) ~= e^s.
SCH_C1 = 184.66496580927726     # 128 * log2(e)
SCH_C2 = 16248.6                # 127*128 minus mean-zeroing interp bias
ACT_FRAC = 0.5266               # share of exp tiles on ACT (rest on DVE)


def build_nc(act_frac=ACT_FRAC):
    nc = bass.Bass()

    rgb_s = nc.dram_tensor("rgb_s", [C, QCH], F32R, kind="ExternalInput")
    edge = nc.dram_tensor("edge", [C, N], F32R, kind="ExternalInput")
    w_qA = nc.dram_tensor("w_qA", [C, 128], F32R, kind="ExternalInput")
    w_qB = nc.dram_tensor("w_qB", [C, 128], F32R, kind="ExternalInput")
    w_kA = nc.dram_tensor("w_kA", [C, 128], F32R, kind="ExternalInput")
    w_kB = nc.dram_tensor("w_kB", [C, 128], F32R, kind="ExternalInput")
    w_v = nc.dram_tensor("w_v", [C, 256], F32R, kind="ExternalInput")
    w_p = nc.dram_tensor("w_p", [256, C], BF16, kind="ExternalInput")
    b_qA = nc.dram_tensor("b_qA", [128, 1], F32, kind="ExternalInput")
    b_qB = nc.dram_tensor("b_qB", [128, 1], F32, kind="ExternalInput")
    b_kA = nc.dram_tensor("b_kA", [128, 1], F32, kind="ExternalInput")
    b_kB = nc.dram_tensor("b_kB", [128, 1], F32, kind="ExternalInput")
    b_v = nc.dram_tensor("b_v", [128, 256], F32, kind="ExternalInput")
    b_p = nc.dram_tensor("b_p", [C, 1], F32, kind="ExternalInput")
    ident = nc.dram_tensor("ident", [128, 128], BF16, kind="ExternalInput")
    out = nc.dram_tensor("out", [C, QCH], F32, kind="ExternalOutput")

    with tile.TileContext(nc) as tc:
        with tc.tile_pool(name="const", bufs=1) as cp, \
             tc.tile_pool(name="data", bufs=1) as dp:
            wq = [cp.tile([128, 256], F32R, name=f"wq{x}", tag=f"wq{x}") for x in "AB"]
            wk = [cp.tile([128, 256], F32R, name=f"wk{x}", tag=f"wk{x}") for x in "AB"]
            wv = cp.tile([128, 512], F32R, name="wv", tag="wv")
            wp = cp.tile([128, 512], BF16, name="wp", tag="wp")
            bq = [cp.tile([128, 1], F32, name=f"bq{x}", tag=f"bq{x}") for x in "AB"]
            bk = [cp.tile([128, 1], F32, name=f"bk{x}", tag=f"bk{x}") for x in "AB"]
            bv = cp.tile([128, 256], F32, name="bv", tag="bv")
            bp = cp.tile([128, 2], F32, name="bp", tag="bp")
            idn = cp.tile([128, 128], BF16, name="idn", tag="idn")
            zrow = cp.tile([1, 128], BF16, name="zrow", tag="zrow")

            nc.sync.dma_start(bk[0][:], b_kA[:])
            nc.sync.dma_start(bq[0][:], b_qA[:])
            nc.vector.memset(zrow[:], 0.0)

            rgb_sb = dp.tile([128, 2 * QCH], F32R, name="rgb_sb", tag="rgb")
            edge_sb = [dp.tile([128, N], F32R, name=f"edge{k}", tag=f"edge{k}") for k in range(2)]

            # Direct DMA into the f32r tiles (f32r bits == f32; the HW
            # matmul rounds on read). Inputs are chunked and ordered so the
            # first QKT can start ~6us in: kk[0]-chunk-0 + qq[0]-qc0 deps
            # first, then pieces in first-use order. The projections
            # themselves are interleaved into the first attention groups.
            # critical first transfers fan out over three DGE queues so
            # the serial per-queue issue time (~0.6us each) overlaps
            for k in range(2):
                nc.sync.dma_start(wk[0][:, 128 * k:128 * (k + 1)], w_kA[128 * k:128 * (k + 1), :])
                nc.scalar.dma_start(wq[0][:, 128 * k:128 * (k + 1)], w_qA[128 * k:128 * (k + 1), :])
            for k in range(2):      # first edge columns (kk chunk 0 dep)
                nc.gpsimd.dma_start(edge_sb[k][:, 0:512], edge[128 * k:128 * (k + 1), 0:512])
            for k in range(2):      # rgb first halves (qc=0 queries)
                nc.scalar.dma_start(rgb_sb[:, QCH * k:QCH * k + 512], rgb_s[128 * k:128 * (k + 1), 0:512])
            for k in range(2):
                nc.sync.dma_start(edge_sb[k][:, 512:1024], edge[128 * k:128 * (k + 1), 512:1024])
            for k in range(2):
                nc.sync.dma_start(wv[:, 256 * k:256 * (k + 1)], w_v[128 * k:128 * (k + 1), :])
            nc.sync.dma_start(bv[:], b_v[:])
            nc.sync.dma_start(bq[1][:], b_qB[:])
            nc.sync.dma_start(bk[1][:], b_kB[:])
            nc.sync.dma_start(bp[:, 0:1], b_p[0:128, :])
            nc.sync.dma_start(bp[:, 1:2], b_p[128:256, :])
            for k in range(2):
                nc.sync.dma_start(edge_sb[k][:, 1024:2048], edge[128 * k:128 * (k + 1), 1024:2048])
            for k in range(2):
                nc.sync.dma_start(rgb_sb[:, QCH * k + 512:QCH * (k + 1)], rgb_s[128 * k:128 * (k + 1), 512:1024])
            for k in range(2):
                nc.sync.dma_start(wk[1][:, 128 * k:128 * (k + 1)], w_kB[128 * k:128 * (k + 1), :])
                nc.sync.dma_start(wq[1][:, 128 * k:128 * (k + 1)], w_qB[128 * k:128 * (k + 1), :])
            for p in range(2, 4):
                for k in range(2):
                    nc.sync.dma_start(edge_sb[k][:, 1024 * p:1024 * (p + 1)],
                                      edge[128 * k:128 * (k + 1), 1024 * p:1024 * (p + 1)])
            for k in range(2):
                nc.sync.dma_start(wp[:, 256 * k:256 * (k + 1)], w_p[128 * k:128 * (k + 1), :])
            nc.sync.dma_start(idn[:], ident[:])

            qq = [dp.tile([128, QCH], F32R, name=f"qq{x}", tag=f"qq{x}") for x in "AB"]
            kk = [dp.tile([128, N], F32R, name=f"kk{x}", tag=f"kk{x}") for x in "AB"]
            vto = dp.tile([128, NMT * 8 * HB], BF16, name="vto", tag="vto")
            # denominator columns (33rd of each head block) are constant 1.0
            nc.vector.memset(
                vto[:].rearrange("p (m c) -> p m c", c=HB)[:, :, 32], 1.0)
            outb = [dp.tile([128, QCH], F32, name=f"outb{k}", tag=f"outb{k}") for k in range(2)]
            xxt = [dp.tile([128, 512], BF16, name=f"xxt{k}", tag=f"xxt{k}") for k in range(2)]

            # ---- pools (PSUM pools span the whole kernel: releasing a
            # PSUM pool makes successor tiles inherit released-zone deps
            # as multi-wait instructions, which walrus rejects) ----
            # PSUM budget (8 banks): scp 3 x [128,1024]f32 (2 banks each)
            # for a 3-deep QKT->exp score pipeline; avp 2 x [128,264]f32
            # (1 bank) for the per-(x,pr) AV accumulator, s-chunks packed
            # along columns. Projection / transpose / out-proj psums
            # borrow scp zones transiently.
            scp = tc.alloc_tile_pool(name="scp", bufs=3, space="PSUM")
            avp = tc.alloc_tile_pool(name="avp", bufs=1, space="PSUM")
            psv = tc.alloc_tile_pool(name="psv", bufs=1, space="PSUM")
            # single fixed PSUM bank shared (via disjoint regions /
            # lifetimes) by: double-buffered vto projection slots (gi0),
            # later the kk/qq projection psums, out-proj psums and
            # transpose outputs — keeping the 3 sc zones churn-free.
            fixt = psv.tile([128, 512], F32, name="fixt", tag="v")
            ep = tc.alloc_tile_pool(name="exp", bufs=12)
            sp = tc.alloc_tile_pool(name="stg", bufs=6)

            # ---- fused projection + attention stream ----
            # All work is emitted as ONE software-pipelined mtile stream
            # across the 8 (qc, x, pr) head-pair groups. Projections ride
            # as per-mtile hooks inside the first two groups (hidden under
            # the attention steady state); AV consumption lags QKT/exp by
            # LAG mtiles (across group boundaries too) so PE never waits
            # on the exp engines; per-group epilogues (relu/recip/scale)
            # fire when the group's last AV flushes, and their PE parts
            # (transposes, output projection) are deferred hooks inside
            # later groups' streams.

            # ACT warmup: absorb the DVE tick (activation-bias const
            # tiles are DVE-written) so the first exp carries only the PE
            # wait — walrus allows a single wait per Activation.
            actw = dp.tile([1, 1], BF16, name="actw", tag="actw")
            nc.scalar.activation(actw[:], zrow[0:1, 0:1], AF.Exp)

            def proj_qq(x, j, ps=None):
                ps = ps if ps is not None else scp.tile([128, 512], F32, name="ps_q", tag="w")
                for k in range(2):
                    nc.tensor.matmul(
                        ps[:], wq[x][:, 128 * k:128 * (k + 1)],
                        rgb_sb[:, QCH * k + 512 * j:QCH * k + 512 * (j + 1)],
                        start=(k == 0), stop=(k == 1))
                nc.scalar.activation(
                    qq[x][:, 512 * j:512 * (j + 1)], ps[:],
                    AF.Identity, bias=bq[x][:])

            def proj_kk(x, j, ps=None):
                ps = ps if ps is not None else scp.tile([128, 512], F32, name="ps_k", tag="w")
                for k in range(2):
                    nc.tensor.matmul(
                        ps[:], wk[x][:, 128 * k:128 * (k + 1)],
                        edge_sb[k][:, 512 * j:512 * (j + 1)],
                        start=(k == 0), stop=(k == 1))
                nc.scalar.activation(
                    kk[x][:, 512 * j:512 * (j + 1)], ps[:],
                    AF.Identity, bias=bk[x][:])

            def proj_vto(mt):
                # v^T without denominator columns: ps slot [128, 256],
                # double-buffered inside the fixed bank; the DVE bias-add
                # scatters into vto's 33-stride head blocks (den columns
                # are memset to 1.0 once at start).
                ps = fixt[:, 256 * (mt % 2):256 * (mt % 2) + 256]
                for k in range(2):
                    nc.tensor.matmul(
                        ps, edge_sb[k][:, 128 * mt:128 * (mt + 1)],
                        wv[:, 256 * k:256 * (k + 1)],
                        start=(k == 0), stop=(k == 1))
                nc.vector.tensor_add(
                    vto[:, 8 * HB * mt:8 * HB * (mt + 1)].rearrange(
                        "p (h c) -> p h c", c=HB)[:, :, 0:32],
                    ps.rearrange("p (h c) -> p h c", c=32), bv[:])

            def transposes(x):
                xnm_x = xnm_tiles[x]
                for s in range(4):
                    tp = fixt[:, 64 * s:64 * (s + 1)].bitcast(BF16)
                    nc.tensor.transpose(tp, xnm_x[:, 128 * s:128 * (s + 1)], idn[:])
                    nc.vector.tensor_copy(xxt[x][:, 128 * s:128 * (s + 1)], tp)

            def outproj(qc):
                q0 = 512 * qc
                for ct in range(2):
                    ps = fixt[:]
                    for k in range(2):
                        nc.tensor.matmul(
                            ps, wp[:, 256 * k + 128 * ct:256 * k + 128 * (ct + 1)],
                            xxt[k][:], start=(k == 0), stop=(k == 1))
                    nc.scalar.activation(
                        outb[ct][:, q0:q0 + 512], ps,
                        AF.Identity, bias=bp[:, ct:ct + 1])

            # per-mtile PE-stream hooks: {(gi, mt): [fn, ...]}
            hooks = {}

            def add_hook(gi, mt, fn):
                hooks.setdefault((gi, mt), []).append(fn)

            add_hook(0, 0, lambda: proj_kk(0, 0))
            add_hook(0, 0, lambda: proj_qq(0, 0))
            for mt in range(NMT):           # vto(mt) feeds AV(mt), LAG behind
                add_hook(0, mt, lambda mt=mt: proj_vto(mt))
            for c in range(1, 8):           # kk[0] chunk c feeds QKT(mt>=4c)
                add_hook(0, 4 * c - 2, lambda c=c: proj_kk(0, c))
            add_hook(1, 24, lambda: proj_qq(1, 0, fixt[:]))
            add_hook(3, 10, lambda: proj_qq(0, 1, fixt[:]))
            add_hook(5, 14, lambda: proj_qq(1, 1, fixt[:]))
            # kk[1] feeds the x=1 groups: chunk 0 late in gi1, the rest
            # just-in-time inside gi2 (chunk c consumed from mt=4c); all
            # use the fixed scratch bank (vto is finished by then)
            add_hook(1, 28, lambda: proj_kk(1, 0, fixt[:]))
            for c in range(1, 8):
                add_hook(2, 4 * c - 2, lambda c=c: proj_kk(1, c, fixt[:]))

            # Exp dispatcher: fixed per-group A/D pattern (ACT exact exp
            # vs DVE bitwise 2^x). Strict alternation avoids queue jitter;
            # the extra ACT tiles (DVE is ~15% slower per tile and carries
            # the vto adds early on) are pinned at mt 0/16 where the PE
            # stream has hook/flush slack.
            PAT_STEADY = ["A" if (mt % 2 == 0 or mt == 9) else "D"
                          for mt in range(NMT)]          # 17A / 15D
            PAT_FIRST = ["A" if mt % 4 != 3 else "D"
                         for mt in range(NMT)]           # 24A / 8D
            GROUP_PATS = [PAT_FIRST, PAT_STEADY] + [PAT_STEADY] * 6
            cur_pat = [PAT_STEADY]

            def emit_exp(et, sc, mt):
                if cur_pat[0][mt] == "A":
                    nc.scalar.activation(et[:], sc[:], AF.Exp)
                else:
                    nc.vector.tensor_scalar(
                        et[:].bitcast(I16), sc[:], SCH_C1, SCH_C2,
                        ALU.mult, ALU.add)

            GROUPS = [(qc, x, pr) for qc in (0, 1) for x in (0, 1) for pr in (0, 1)]
            # deferred PE-side epilogue hooks: group gi's xnm tile (x done
            # at odd gi) is transposed inside group gi+1's stream; the
            # output projection of qc=0 rides in group 4 (qc=1 x=0 pr=0).
            add_hook(2, 10, lambda: transposes(0))
            add_hook(4, 10, lambda: transposes(1))
            add_hook(4, 16, lambda: outproj(0))
            add_hook(6, 10, lambda: transposes(0))

            def out_dma_qc0():
                for ct in range(2):
                    nc.sync.dma_start(out[128 * ct:128 * (ct + 1), 0:512],
                                      outb[ct][:, 0:512])
            add_hook(5, 8, out_dma_qc0)
            LAG = 3
            xnm_tiles = [None, None]
            pend = []       # (emit_av_fn, post_fn_or_None)

            def flush_one():
                fn, post = pend.pop(0)
                fn()
                if post is not None:
                    post()

            for gi, (qc, x, pr) in enumerate(GROUPS):
                q0 = 512 * qc
                cur_pat[0] = GROUP_PATS[gi]
                avt = avp.tile([128, 264], F32, name="av", tag="av")

                def make_av(avt, x, pr, mt, et):
                    def emit_av():
                        for j2 in range(2):
                            h = 4 * x + 2 * pr + j2
                            for s in range(4):
                                nc.tensor.matmul(
                                    avt[:, 66 * s + 33 * j2:66 * s + 33 * j2 + 33],
                                    et[:, 512 * j2 + 128 * s:512 * j2 + 128 * (s + 1)],
                                    vto[:, 8 * HB * mt + HB * h:8 * HB * mt + HB * (h + 1)],
                                    start=False, stop=(mt == NMT - 1))
                    return emit_av

                def make_epilogue(avt, gi, qc, x, pr):
                    def epilogue():
                        # relu (ACT, PSUM->SBUF), reciprocal of the
                        # denominators (DVE), per-block scale into the
                        # shared pre-transpose tile (Pool).
                        xxm = sp.tile([128, 264], F32, name="xxm", tag="xxm")
                        nc.scalar.activation(xxm[:], avt[:], AF.Relu)
                        rden = sp.tile([128, 8], F32, name="rden", tag="rden")
                        nc.vector.reciprocal(
                            rden[:],
                            xxm[:].rearrange("p (g c) -> p g c", c=33)[:, :, 32])
                        if pr == 0:
                            xnm_tiles[x] = sp.tile([128, 512], BF16,
                                                   name="xnm", tag="xnm")
                        xnm_x = xnm_tiles[x]
                        if gi == len(GROUPS) - 1:
                            # tail: DVE is idle; one strided op beats the
                            # serial Pool-launch chain
                            nc.vector.scalar_tensor_tensor(
                                xnm_x[:].rearrange(
                                    "p (s v u c) -> p s v u c",
                                    v=2, u=2, c=32)[:, :, pr, :, :],
                                xxm[:].rearrange(
                                    "p (s u c) -> p s u c",
                                    u=2, c=33)[:, :, :, 0:32],
                                1.0,
                                rden[:].rearrange("p (s u) -> p s u", u=2)
                                    .unsqueeze(3).broadcast_to([128, 4, 2, 32]),
                                ALU.mult, ALU.mult)
                        else:
                            for g in range(8):      # g = 2*s + j2
                                s, j2 = g // 2, g % 2
                                c0 = 128 * s + 64 * pr + 32 * j2
                                nc.gpsimd.tensor_scalar_mul(
                                    xnm_x[:, c0:c0 + 32],
                                    xxm[:, 33 * g:33 * g + 32],
                                    rden[:, g:g + 1])
                    return epilogue

                for mt in range(NMT):
                    for fn in hooks.get((gi, mt), []):
                        fn()
                    if mt == 6:
                        # whole-bank start=True clear of this group's AV
                        # accumulator; deferred here so it lands after the
                        # previous group's epilogue (relu) was emitted —
                        # avp has a single zone. The AV matmuls use
                        # start=False (first per-element write overwrites,
                        # later ones accumulate).
                        nc.tensor.matmul(avt[:], zrow[:], vto[0:1, 0:264],
                                         start=True, stop=False)
                    # deeper flush threshold early in the group: the first
                    # own-AV flush (needs the cleared accumulator) waits
                    # until mt=8, well past the clear
                    limit = 7 if mt < 8 else LAG
                    while len(pend) > limit:
                        flush_one()
                    sc = scp.tile([128, 1024], F32, name="sc", tag="w")
                    for j2 in range(2):
                        j = 2 * pr + j2
                        nc.tensor.matmul(
                            sc[:, 512 * j2:512 * (j2 + 1)],
                            kk[x][32 * j:32 * j + KD, 128 * mt:128 * (mt + 1)],
                            qq[x][32 * j:32 * j + KD, q0:q0 + 512],
                            start=True, stop=True,
                            tile_position=(32 * j, 0))
                    et = ep.tile([128, 1024], BF16, name="et", tag="et")
                    emit_exp(et, sc, mt)
                    post = make_epilogue(avt, gi, qc, x, pr) if mt == NMT - 1 else None
                    pend.append((make_av(avt, x, pr, mt, et), post))

            # tail: flush remaining AVs (fires the last epilogue), then
            # the final transposes + output projection
            while pend:
                flush_one()
            transposes(1)
            outproj(1)

            for ct in range(2):
                nc.sync.dma_start(out[128 * ct:128 * (ct + 1), 512:1024],
                                  outb[ct][:, 512:1024])
            for _p in (sp, ep, psv, avp, scp):
                _p.release()

    # walrus codegen accepts only ONE sync wait on compute instructions
    # (Matmult / Activation / TensorTensor / ...). The multi-wait cases
    # Tile emits here are all {self-engine, other}: a self-engine wait
    # orders an instruction against an earlier instruction on the SAME
    # in-order engine (WAW through PE's single PSUM write port, ACT/DVE
    # pipeline order), which the hardware already guarantees — drop it.
    _self_prefix = {
        "EngineType.PE": "PE",
        "EngineType.Activation": "Activation",
        "EngineType.DVE": "DVE",
        "EngineType.Pool": "Pool",
        "EngineType.SP": "SP",
    }
    for f in nc.m.functions:
        for bb in f.blocks:
            for inst in bb.instructions:
                si = inst.sync_info
                if si is None or not si.on_wait or len(si.on_wait) < 2:
                    continue
                pref = _self_prefix.get(str(getattr(inst, "engine", "")), None)
                if pref is None:
                    continue
                kept = [w for w in si.on_wait
                        if not str(w.ant_name).startswith(pref)]
                if not kept or len(kept) == len(si.on_wait):
                    continue
                si.on_wait = kept

    # Safety net: any instruction still carrying >1 wait gets all but its
    # last wait hoisted into preceding same-engine NoOps (1 wait each).
    uid = [0]
    for f in nc.m.functions:
        for bb in f.blocks:
            new_insts = []
            for inst in bb.instructions:
                si = inst.sync_info
                if si is not None and si.on_wait and len(si.on_wait) > 1:
                    for w in si.on_wait[:-1]:
                        uid[0] += 1
                        nop = mybir.InstNoOp(
                            name=f"I-waitsplit-{uid[0]}", ins=[], outs=[])
                        nop.engine = inst.engine
                        nop.sync_info = mybir.SyncInfo(
                            on_wait=[w], on_update=[])
                        new_insts.append(nop)
                    si.on_wait = [si.on_wait[-1]]
                new_insts.append(inst)
            bb.instructions = new_insts
    return nc


_CACHE = {}


def _prep_host(inputs):
    """Fold BN into weights; build head-split layouts shared by all cores."""
    import ml_dtypes
    f = np.float32
    Wq = (inputs["Wq"] * inputs["sq"][:, None]).astype(f)
    Wk = (inputs["Wk"] * inputs["sk"][:, None]).astype(f)
    Wv = (inputs["Wv"] * inputs["sv"][:, None]).astype(f)
    Wp = (inputs["Wp"] * inputs["sp"][:, None]).astype(f)

    def split(Wt, b):
        o = []
        for g in range(2):
            Wx = np.zeros((C, 128), f)
            bx = np.zeros((128, 1), f)
            for j in range(4):
                h = 4 * g + j
                Wx[:, 32 * j:32 * j + KD] = Wt[:, KD * h:KD * (h + 1)]
                bx[32 * j:32 * j + KD, 0] = b[KD * h:KD * (h + 1)]
            o.append((np.ascontiguousarray(Wx), bx))
        return o

    (wqA, bqA), (wqB, bqB) = split(Wq.T.astype(f), inputs["bq"])
    (wkA, bkA), (wkB, bkB) = split(Wk.T.astype(f), inputs["bk"])
    WvT = Wv.T.astype(f)                      # [C, 256] cols (h, d)
    ident_bf16 = np.eye(128, dtype=ml_dtypes.bfloat16)
    return dict(
        w_qA=wqA, w_qB=wqB, w_kA=wkA, w_kB=wkB,
        w_v=np.ascontiguousarray(WvT),
        w_p=np.ascontiguousarray(Wp.T).astype(ml_dtypes.bfloat16),
        b_qA=bqA, b_qB=bqB, b_kA=bkA, b_kB=bkB,
        b_v=np.ascontiguousarray(np.broadcast_to(inputs["bv"].astype(f), (128, 256))),
        b_p=inputs["bp"].astype(f).reshape(C, 1),
        ident=ident_bf16,
    )


def kernel(**inputs) -> np.ndarray:
    inputs = {k: np.asarray(v) for k, v in inputs.items()}
    if "nc" not in _CACHE:
        _CACHE["nc"] = build_nc()
    nc = _CACHE["nc"]

    shared = _prep_host(inputs)
    rgb = np.ascontiguousarray(inputs["rgb"].astype(np.float32).reshape(B, C, N))
    edge = np.ascontiguousarray(inputs["edge"].astype(np.float32).reshape(B, C, N))

    in_maps = []
    for core in range(8):
        b, qs = core // 4, core % 4
        m = dict(shared)
        m["rgb_s"] = np.ascontiguousarray(rgb[b][:, QCH * qs:QCH * (qs + 1)])
        m["edge"] = edge[b]
        in_maps.append(m)

    res = run_bass_kernel_spmd(nc, in_maps, core_ids=list(range(8)))
    full = np.zeros((B, C, N), np.float32)
    for core in range(8):
        b, qs = core // 4, core % 4
        full[b][:, QCH * qs:QCH * (qs + 1)] = res.results[core]["out"]
    return full.reshape(B, C, H, W)
